# revision 1
# baseline (speedup 1.0000x reference)
"""Trainium2 Bass kernel for a Neural CDE forward pass.

Model (see reference): 2000 fixed Euler steps of
    y_{t+1} = y_t + dt * einsum('bhd,bd->bh', tanh-MLP(y_t).reshape(B,H,D), dX_t)
with a 3-layer softplus MLP (32 -> 128 -> 128 -> 256/tanh), batch B=128,
followed by a linear readout.

Strategy:
  * Pure data parallel over 8 NeuronCores (16 batch elements per core).
  * Feature-major activation layout (features on partitions, batch on the
    free dim) so every layer is a single PE matmul with a constant lhsT.
  * The cubic-spline derivative dX (and the dt factor) is precomputed on
    the host for all 2000 steps, pre-broadcast to the 256-feature layout
    the einsum needs, and streamed to SBUF in big chunks.
  * softplus(x) = Ln(Exp(x) + 1): two ScalarE ops from the single
    natural_log_exp activation table (gen3 has no softplus entry).
  * tanh(v) = 2/(1+exp(-2v)) - 1: one ScalarE Exp + DVE reciprocal,
    with the affine part fused into the dX multiply (one DVE op).
  * y is never materialized per step.  PSUM bank `psum1` accumulates
    A @ y_t (A = F0) directly across all steps:  psum1 += [A A .. A] @ g_t
    where g_t = (sigma - 1/2) * (2 dX dt) in a d-major 256-feature layout.
    A second PSUM bank (`psum_y`) accumulates Sel @ g_t per step (queued in
    a PE bubble, off the critical chain) and yields y_T at the end.
  * The activation-table registry is pinned so Exp/Ln/Identity resolve to
    the single natural_log_exp_and_others table (one ACT_TABLE_LOAD total;
    the default chooser alternates tables and costs ~5 us/step).

Measured on trn2 (8 cores): ~3.0 us/step critical chain, 6.05 ms total,
rel err ~1e-4 vs the fp32 reference (fp16 matmul weights/activations).
"""

import os
import numpy as np

B = 128
NP_KNOTS = 128
D = 8
H = 32
WID = 128
NCLS = 10
T0, T1 = 0.0, 20.0
DT0 = 0.01
NUM_STEPS = 2000
NCORES = 8
BS = B // NCORES  # 16 batch per core

_F32 = np.float32


# --------------------------------------------------------------------------
# Host-side precompute
# --------------------------------------------------------------------------

def _spline_dx(ts, coeff_d, coeff_c, coeff_b, num_steps):
    """dX/dt at each Euler step start time, with the (clipped) dt folded in.

    Mirrors the reference computation in fp32.  Returns (S, B, D)."""
    t_grid = (ts[0] + _F32(DT0) * np.arange(num_steps, dtype=_F32)).astype(_F32)
    dts = np.minimum(_F32(DT0), ts[-1] - t_grid).astype(_F32)
    idx = np.clip(np.searchsorted(ts, t_grid, side="right") - 1, 0, NP_KNOTS - 2)
    fr = (t_grid - ts[idx]).astype(_F32)[None, :, None]
    dX = (coeff_b[:, idx] + _F32(2.0) * coeff_c[:, idx] * fr
          + _F32(3.0) * coeff_d[:, idx] * fr * fr)          # (B, S, D)
    dX = np.transpose(dX, (1, 0, 2)).astype(_F32)           # (S, B, D)
    return dX * dts[:, None, None]


def _dxb_layout(dx_core, steps_per_chunk):
    """(S, BS, D) -> (CH, 128, C*32) chunked, d-major, h-broadcast layout.

    Feature p = d*32 + h lives in col-block cb = d // 4 (d_local = d % 4...
    precisely: partition p in col-block cb holds global feature cb*128 + p,
    i.e. d = cb*4 + p//32, h = p % 32).  Includes the factor 2 used by the
    fused tanh DVE op."""
    S = dx_core.shape[0]
    C = steps_per_chunk
    CH = S // C
    # [s, j, d] -> [s, j, cb, dblk] with d = cb*4 + dblk
    tmp = dx_core.reshape(S, BS, 2, 4)
    # -> [s, dblk, cb, j]
    tmp = np.transpose(tmp, (0, 3, 2, 1))
    # broadcast over h (32): [s, dblk, h, cb, j]
    tmp = np.broadcast_to(tmp[:, :, None, :, :], (S, 4, 32, 2, BS))
    arr = tmp.reshape(S, 128, 32)                      # [s, p, cb*16 + j]
    arr = arr.reshape(CH, C, 128, 32).transpose(0, 2, 1, 3).reshape(CH, 128, C * 32)
    return np.ascontiguousarray(_F32(2.0) * arr)


MM_DT = np.float16  # dtype of the per-step matmuls (fp16: 1 cyc/row + FWL)


def _host_weights(W0, b0, W1, b1, W2, b2, F0, f0, F1, f1, F2, f2, R, rb):
    """All constant tensors, already transposed/permuted for the kernel."""
    f32 = lambda a: np.ascontiguousarray(a, dtype=_F32)
    f16 = lambda a: np.ascontiguousarray(a, dtype=MM_DT)
    # d-major permutation of the 256 func-MLP output features
    p = np.arange(256)
    perm = (p % 32) * 8 + p // 32          # F2p[p] = F2[(p%32)*8 + p//32]
    F2p = F2[perm]
    f2p = f2[perm]
    W = {
        "ATt":   f16(np.tile(F0.T, (4, 1))),          # (128,128) lhsT for psum1 += [A..A] @ g
        "F1T":   f16(F1.T),                            # (128,128)
        "F2aT":  f16(F2p[:128].T),                     # (128,128)
        "F2bT":  f16(F2p[128:].T),                     # (128,128)
        "f2rows": f16(np.stack([f2p[:128], f2p[128:]])),   # (2,128) bias lhsT
        "Sel":   f16(np.tile(np.eye(32, dtype=_F32), (4, 1))),  # (128,32)
        "W0T":   f32(W0.T),                            # (8,128)
        "W1T":   f32(W1.T),                            # (128,128)
        "W2T":   f32(W2.T),                            # (128,32)
        "AW2T":  f32((F0 @ W2).T),                     # (128,128)
        "Ab2":   f32((F0 @ b2)[None, :]),              # (1,128)
        "RT":    f32(R.T),                             # (32,10)
        "b0c":   f32(b0[:, None]),                     # (128,1)
        "b1c":   f32(b1[:, None]),
        "f0c":   f32(f0[:, None]),
        "f1c":   f32(f1[:, None]),
        "b2c":   f32(b2[:, None]),                     # (32,1)
        "rbc":   f32(rb[:, None]),                     # (10,1)
        "ones2": f16(np.stack([np.r_[np.ones(16), np.zeros(16)],
                               np.r_[np.zeros(16), np.ones(16)]])),  # (2,32)
        "ones16": f32(np.ones((1, 16))),
    }
    return W


# --------------------------------------------------------------------------
# Bass kernel build
# --------------------------------------------------------------------------

_NC_CACHE = {}


def _build_nc(num_steps, steps_per_chunk):
    key = (num_steps, steps_per_chunk)
    if key in _NC_CACHE:
        return _NC_CACHE[key]

    import concourse.bacc as bacc
    import concourse.bass as bass
    import concourse.mybir as mybir
    import concourse.tile as tile
    from contextlib import ExitStack

    f32 = mybir.dt.float32
    mmdt = mybir.dt.from_np(np.dtype(MM_DT))
    AF = mybir.ActivationFunctionType
    OP = mybir.AluOpType

    # Pin the activation-function table: everything we use (Exp, Ln,
    # Identity) lives in natural_log_exp_and_others.  Without this the
    # table chooser may alternate tables between Exp and Ln, inserting a
    # ~1.3us ACT_TABLE_LOAD several times per step.  The act_func_set_id
    # is an index into the FULL ordered table list, so keep all names and
    # positions, but strip our functions from every other table so the
    # chooser has exactly one option.
    import concourse.hw_specs as hw_specs
    _full_tabs = hw_specs.get_activation_tables("gen3")
    _ours = {AF.Exp, AF.Ln, AF.Identity, AF.Copy}
    _pinned = {
        name: (set(funcs) if name == "natural_log_exp_and_others"
               else set(funcs) - _ours)
        for name, funcs in _full_tabs.items()
    }
    bacc.get_activation_tables = lambda arch: _pinned

    S = num_steps
    C = steps_per_chunk
    assert S % C == 0
    CH = S // C

    nc = bacc.Bacc("TRN2", target_bir_lowering=False, debug=False)

    # ---- DRAM I/O ----
    dram = {}
    wshapes = {
        "ATt": (128, 128), "F1T": (128, 128), "F2aT": (128, 128),
        "F2bT": (128, 128), "f2rows": (2, 128), "Sel": (128, 32),
        "W0T": (8, 128), "W1T": (128, 128), "W2T": (128, 32),
        "AW2T": (128, 128), "Ab2": (1, 128), "RT": (32, 10),
        "b0c": (128, 1), "b1c": (128, 1), "f0c": (128, 1), "f1c": (128, 1),
        "b2c": (32, 1), "rbc": (10, 1), "ones2": (2, 32), "ones16": (1, 16),
    }
    mm_names = {"ATt", "F1T", "F2aT", "F2bT", "f2rows", "Sel", "ones2"}
    for name, shp in wshapes.items():
        dt_ = mmdt if name in mm_names else f32
        dram[name] = nc.dram_tensor(name, list(shp), dt_, kind="ExternalInput")
    dram["x0"] = nc.dram_tensor("x0", [8, BS], f32, kind="ExternalInput")
    dram["dxb"] = nc.dram_tensor("dxb", [CH, 128, C * 32], f32, kind="ExternalInput")
    out_dram = nc.dram_tensor("logits", [NCLS, BS], f32, kind="ExternalOutput")

    with tile.TileContext(nc) as tc, ExitStack() as ctx:
        const = ctx.enter_context(tc.tile_pool(name="const", bufs=1))
        dxbp = ctx.enter_context(tc.tile_pool(name="dxbp", bufs=2))
        work = ctx.enter_context(tc.tile_pool(name="work", bufs=3))
        psum = ctx.enter_context(
            tc.tile_pool(name="psum", bufs=1, space="PSUM"))
        ptmp = ctx.enter_context(
            tc.tile_pool(name="ptmp", bufs=2, space="PSUM"))

        # ---- constants into SBUF ----
        ct = {}
        for name, shp in wshapes.items():
            dt_ = mmdt if name in mm_names else f32
            ct[name] = const.tile(list(shp), dt_, tag=name, name=f"c_{name}")
            nc.sync.dma_start(ct[name][:], dram[name][:])
        x0_t = const.tile([8, BS], f32, tag="x0")
        nc.sync.dma_start(x0_t[:], dram["x0"][:])

        # ---- persistent PSUM tiles ----
        psum1 = psum.tile([128, BS], f32, tag="psum1")   # A @ y_t accumulator
        psum2 = psum.tile([128, BS], f32, tag="psum2")
        psum3 = psum.tile([128, 2 * BS], f32, tag="psum3")
        psum_y = psum.tile([32, BS], f32, tag="psum_y")  # y_T (minus b2)

        def softplus(ps_in, bias_ap, out_tile):
            """out = ln(1 + exp(ps_in + bias)); two ACT ops, one table."""
            e = ptmp.tile([128, BS], f32, tag="ptmp")
            nc.scalar.activation(e[:], ps_in, AF.Exp, bias=bias_ap)
            nc.scalar.activation(out_tile[:], e[:], AF.Ln, bias=1.0)

        # ---- initial MLP: y0 = W2 @ sp(W1 @ sp(W0 @ x0 + b0) + b1) (+ b2) ----
        psA = ptmp.tile([128, BS], f32, tag="ptmp")
        nc.tensor.matmul(psA[:], ct["W0T"][:], x0_t[:], start=True, stop=True)
        hA = work.tile([128, BS], f32, tag="h1")
        softplus(psA[:], ct["b0c"][:], hA)
        psB = ptmp.tile([128, BS], f32, tag="ptmp")
        nc.tensor.matmul(psB[:], ct["W1T"][:], hA[:], start=True, stop=True)
        hB = work.tile([128, BS], f32, tag="h2")
        softplus(psB[:], ct["b1c"][:], hB)

        # psum_y <- W2 @ hB   (b2 is added at the end)
        nc.tensor.matmul(psum_y[:], ct["W2T"][:], hB[:], start=True, stop=False,
                         skip_group_check=True)
        # psum1 <- A @ y0 = (F0 @ W2) @ hB + F0 @ b2
        nc.tensor.matmul(psum1[:], ct["AW2T"][:], hB[:], start=True, stop=False,
                         skip_group_check=True)
        nc.tensor.matmul(psum1[:], ct["Ab2"][:], ct["ones16"][:],
                         start=False, stop=False, skip_group_check=True)

        # ---- the 2000-step Euler scan ----
        g_prev = None
        for ch in range(CH):
            dxb_t = dxbp.tile([128, C * 32], f32, tag="dxb")
            nc.sync.dma_start(dxb_t[:], dram["dxb"][ch])
            for c in range(C):
                t = ch * C + c
                if t > 0:
                    # psum1 += [A .. A] @ g_{t-1}   (both 128-col halves)
                    nc.tensor.matmul(psum1[:], ct["ATt"][:], g_prev[:, 0:BS],
                                     start=False, stop=False, skip_group_check=True)
                    nc.tensor.matmul(psum1[:], ct["ATt"][:], g_prev[:, BS:2 * BS],
                                     start=False, stop=False, skip_group_check=True)
                # layer 1: h1 = sp(psum1 + f0)
                h1 = work.tile([128, BS], mmdt, tag="h1s")
                softplus(psum1[:], ct["f0c"][:], h1)
                # layer 2
                nc.tensor.matmul(psum2[:], ct["F1T"][:], h1[:], start=True, stop=True)
                if t > 0:
                    # psum_y += Sel @ g_{t-1}; queued after mm2 so it runs in
                    # the PE bubble while ACT does layer-2 softplus, keeping
                    # it off the critical chain.
                    nc.tensor.matmul(psum_y[:], ct["Sel"][:], g_prev[:, 0:BS],
                                     start=False, stop=False, skip_group_check=True)
                    nc.tensor.matmul(psum_y[:], ct["Sel"][:], g_prev[:, BS:2 * BS],
                                     start=False, stop=False, skip_group_check=True)
                h2 = work.tile([128, BS], mmdt, tag="h2s")
                softplus(psum2[:], ct["f1c"][:], h2)
                # layer 3: psum3 = F2p @ h2 + f2p   (bias via K=2 matmul)
                nc.tensor.matmul(psum3[:], ct["f2rows"][:], ct["ones2"][:],
                                 start=True, stop=False, skip_group_check=True)
                nc.tensor.matmul(psum3[:, 0:BS], ct["F2aT"][:], h2[:],
                                 start=False, stop=False, skip_group_check=True)
                nc.tensor.matmul(psum3[:, BS:2 * BS], ct["F2bT"][:], h2[:],
                                 start=False, stop=True, skip_group_check=True)
                # tanh(z) * (2 dX dt)  =  (1/(1+exp(-2z)) - 0.5) * (4 dX dt) ... :
                #   t3 = exp(-2 z); w = min(1+t3, 1e30); r ~= 1/w;
                #   g  = (r - 0.5) * dxb2          (dxb2 = 2 dt dX, d-major)
                t3 = work.tile([128, 2 * BS], f32, tag="t3")
                nc.scalar.activation(t3[:], psum3[:], AF.Exp, scale=-2.0)
                w = work.tile([128, 2 * BS], f32, tag="w")
                nc.vector.tensor_scalar(w[:], t3[:], 1.0, 1.0e30, OP.add, OP.min)
                r = work.tile([128, 2 * BS], f32, tag="r")
                if os.environ.get("NCDE_NATIVE_RECIP"):
                    nc.vector.reciprocal(r[:], w[:])
                else:
                    nc.vector.reciprocal_approx_fast(r[:], w[:])
                g = work.tile([128, 2 * BS], mmdt, tag="g")
                nc.vector.scalar_tensor_tensor(
                    g[:], r[:], -0.5, dxb_t[:, c * 32:(c + 1) * 32],
                    OP.add, OP.mult)
                g_prev = g

        # ---- finish: y_T = y0 + sum_t Sel @ g_t ----
        nc.tensor.matmul(psum_y[:], ct["Sel"][:], g_prev[:, 0:BS],
                         start=False, stop=False, skip_group_check=True)
        nc.tensor.matmul(psum_y[:], ct["Sel"][:], g_prev[:, BS:2 * BS],
                         start=False, stop=True, skip_group_check=True)
        y_sb = work.tile([32, BS], f32, tag="y_sb")
        nc.scalar.activation(y_sb[:], psum_y[:], AF.Identity, bias=ct["b2c"][:])
        # readout
        psl = ptmp.tile([NCLS, BS], f32, tag="ptmp")
        nc.tensor.matmul(psl[:], ct["RT"][:], y_sb[:], start=True, stop=True)
        out_sb = work.tile([NCLS, BS], f32, tag="out_sb")
        nc.scalar.activation(out_sb[:], psl[:], AF.Identity, bias=ct["rbc"][:])
        nc.sync.dma_start(out_dram[:], out_sb[:])

    nc.compile()
    _NC_CACHE[key] = nc
    return nc


# --------------------------------------------------------------------------
# Public entry point
# --------------------------------------------------------------------------

def _prepare_inputs(ts, coeff_d, coeff_c, coeff_b, coeff_a,
                    W0, b0, W1, b1, W2, b2, F0, f0, F1, f1, F2, f2, R, rb,
                    num_steps, steps_per_chunk):
    ts = np.asarray(ts, dtype=_F32)
    coeff_a = np.asarray(coeff_a, dtype=_F32)
    dx = _spline_dx(ts, np.asarray(coeff_d, _F32), np.asarray(coeff_c, _F32),
                    np.asarray(coeff_b, _F32), num_steps)          # (S,B,D), dt folded
    W = _host_weights(*[np.asarray(a, _F32) for a in
                        (W0, b0, W1, b1, W2, b2, F0, f0, F1, f1, F2, f2, R, rb)])
    in_maps = []
    for core in range(NCORES):
        bs = slice(core * BS, (core + 1) * BS)
        m = dict(W)
        m["x0"] = np.ascontiguousarray(coeff_a[bs, 0, :].T)        # (8,16)
        m["dxb"] = _dxb_layout(dx[:, bs, :], steps_per_chunk)      # (CH,128,C*32)
        in_maps.append(m)
    return in_maps


def kernel(ts, coeff_d, coeff_c, coeff_b, coeff_a,
           W0, b0, W1, b1, W2, b2, F0, f0, F1, f1, F2, f2, R, rb):
    from concourse.bass_utils import run_bass_kernel_spmd

    num_steps = NUM_STEPS
    steps_per_chunk = 250
    nc = _build_nc(num_steps, steps_per_chunk)
    in_maps = _prepare_inputs(ts, coeff_d, coeff_c, coeff_b, coeff_a,
                              W0, b0, W1, b1, W2, b2, F0, f0, F1, f1, F2, f2,
                              R, rb, num_steps, steps_per_chunk)
    res = run_bass_kernel_spmd(nc, in_maps, list(range(NCORES)))
    logits = np.concatenate(
        [res.results[i]["logits"].T for i in range(NCORES)], axis=0)
    return np.ascontiguousarray(logits.astype(np.float32))



# revision 2
# speedup vs baseline: 2.9841x; 2.9841x over previous
"""Trainium2 Bass kernel for a Neural CDE forward pass — parareal edition.

Model (see reference): 2000 fixed Euler steps (h=0.01) of
    y_{t+1} = y_t + dt * einsum('bhd,bd->bh', tanh-MLP(y_t).reshape(B,H,D), dX_t)
with a 3-layer softplus MLP (32 -> 128 -> 128 -> 256/tanh), batch B=128,
followed by a linear readout.

The reference trajectory is sensitive (its own discretization error vs the
true flow is ~5e-2), so the only way to match it within 2e-2 is to reproduce
its exact discrete Euler map.  The serial 2000-step chain is latency-bound
(~3us/step: every instruction has a ~200-600ns mostly-size-independent cost).

Strategy:
  * Pure data parallel over 8 NeuronCores (16 batch elements per core).
  * PARAREAL over time inside each core: split the 2000 steps into P=25
    slabs of 80 steps.  The fine propagator (exact Euler, h=0.01) advances
    ALL slabs simultaneously, packed side by side in the free dimension
    (width 25*16=400), so each 80-step fine sweep costs barely more than 80
    narrow steps.  A serial coarse propagator (Euler with m substeps per
    slab) plus the parareal correction
        U_{s+1} <- G(U_s) + F(U_s_prev) - G(U_s_prev)
    stitches the slabs together.  J iterations converge toward the exact
    fine (reference) map; float64 simulation of this exact configuration
    gives rel err 3.9e-3 (P=25, m=2, J=2) / 3.0e-4 (m=4) vs the reference.
  * Feature-major layout: features on partitions, (slab, batch) on the free
    dim; every layer is one PE matmul with a constant fp16 lhsT.
  * softplus(x) = Ln(Exp(x)+1): two ACT ops (natural_log_exp table).
  * tanh section via the softplus identity  sigma(2z) = exp(-ln(1+e^{-2z})):
    Exp -> Ln -> Exp on ACT (cheaper than DVE reciprocal at width 400),
    then one DVE op  g = (sigma - 1/2) * (2 dX dt).
  * The -2*f2 bias of the func-MLP output layer rides the Exp bias port, so
    layer 3 needs no bias matmul.
  * The activation-table registry is pinned so Exp/Ln/Identity resolve to
    the single natural_log_exp_and_others table (one ACT_TABLE_LOAD total).
"""

import numpy as np

B = 128
NP_KNOTS = 128
D = 8
H = 32
WID = 128
NCLS = 10
T0, T1 = 0.0, 20.0
DT0 = 0.01
NUM_STEPS = 2000
NCORES = 8
BS = B // NCORES          # 16 batch per core

# parareal configuration
P_SLABS = 25              # time slabs per core
NS = NUM_STEPS // P_SLABS  # fine steps per slab (80)
M_COARSE = 2              # coarse Euler substeps per slab
J_ITERS = 2               # parareal iterations
CHUNK = 10                # fine steps per DMA chunk
W = P_SLABS * BS          # packed fine width (400)

_F32 = np.float32
MM_DT = np.float16


# --------------------------------------------------------------------------
# Host-side precompute
# --------------------------------------------------------------------------

def _dx_eval(ts, coeff_d, coeff_c, coeff_b, t_grid):
    """Spline derivative dX/dt at the given times.  Returns (T, B, D)."""
    idx = np.clip(np.searchsorted(ts, t_grid, side="right") - 1, 0, NP_KNOTS - 2)
    fr = (t_grid - ts[idx]).astype(_F32)[None, :, None]
    dX = (coeff_b[:, idx] + _F32(2.0) * coeff_c[:, idx] * fr
          + _F32(3.0) * coeff_d[:, idx] * fr * fr)          # (B, T, D)
    return np.transpose(dX, (1, 0, 2)).astype(_F32)         # (T, B, D)


def _pack_layout(dx_scaled, cols):
    """(T, cols, D) -> (T, 128, 2*cols) d-major, h-broadcast layout.

    Partition p in col-half cb holds (d = cb*4 + p//32, h = p%32)."""
    T = dx_scaled.shape[0]
    tmp = dx_scaled.reshape(T, cols, 2, 4)          # [t, j, cb, dblk]
    tmp = np.transpose(tmp, (0, 3, 2, 1))           # [t, dblk, cb, j]
    tmp = np.broadcast_to(tmp[:, :, None, :, :], (T, 4, 32, 2, cols))
    return np.ascontiguousarray(tmp.reshape(T, 128, 2 * cols))


def _host_weights(W0, b0, W1, b1, W2, b2, F0, f0, F1, f1, F2, f2, R, rb):
    f32 = lambda a: np.ascontiguousarray(a, dtype=_F32)
    f16 = lambda a: np.ascontiguousarray(a, dtype=MM_DT)
    p = np.arange(256)
    perm = (p % 32) * 8 + p // 32
    F2p = F2[perm]
    f2p = f2[perm]
    return {
        "ATt":   f16(np.tile(F0.T, (4, 1))),          # (128,128)
        "F1T":   f16(F1.T),
        "F2aT":  f16(F2p[:128].T),
        "F2bT":  f16(F2p[128:].T),
        "Sel":   f16(np.tile(np.eye(32, dtype=_F32), (4, 1))),  # (128,32)
        "F0T32": f32(F0.T),                            # (32,128) fp32 lhsT
        "Id32":  f32(np.eye(32, dtype=_F32)),          # (32,32)
        "W0T":   f32(W0.T), "W1T": f32(W1.T), "W2T": f32(W2.T),
        "RT":    f32(R.T),                             # (32,10)
        "b0c":   f32(b0[:, None]), "b1c": f32(b1[:, None]),
        "b2c":   f32(b2[:, None]),
        "f0c":   f32(f0[:, None]), "f1c": f32(f1[:, None]),
        "f2nA":  f32(-2.0 * f2p[:128, None]),
        "f2nB":  f32(-2.0 * f2p[128:, None]),
        "rbc":   f32(rb[:, None]),
    }


# --------------------------------------------------------------------------
# Bass kernel build
# --------------------------------------------------------------------------

_NC_CACHE = {}


def _build_nc():
    key = (P_SLABS, M_COARSE, J_ITERS, CHUNK)
    if key in _NC_CACHE:
        return _NC_CACHE[key]

    import concourse.bacc as bacc
    import concourse.mybir as mybir
    import concourse.tile as tile
    from contextlib import ExitStack

    f32 = mybir.dt.float32
    mmdt = mybir.dt.from_np(np.dtype(MM_DT))
    AF = mybir.ActivationFunctionType
    OP = mybir.AluOpType

    import concourse.hw_specs as hw_specs
    _full_tabs = hw_specs.get_activation_tables("gen3")
    _ours = {AF.Exp, AF.Ln, AF.Identity, AF.Copy}
    _pinned = {
        name: (set(funcs) if name == "natural_log_exp_and_others"
               else set(funcs) - _ours)
        for name, funcs in _full_tabs.items()
    }
    bacc.get_activation_tables = lambda arch: _pinned

    P, m, J, C = P_SLABS, M_COARSE, J_ITERS, CHUNK
    NCH = NS // C
    assert NS % C == 0

    nc = bacc.Bacc("TRN2", target_bir_lowering=False, debug=False)

    # ---- DRAM I/O ----
    wshapes = {
        "ATt": (128, 128), "F1T": (128, 128), "F2aT": (128, 128),
        "F2bT": (128, 128), "Sel": (128, 32),
        "F0T32": (32, 128), "Id32": (32, 32),
        "W0T": (8, 128), "W1T": (128, 128), "W2T": (128, 32),
        "RT": (32, 10),
        "b0c": (128, 1), "b1c": (128, 1), "b2c": (32, 1),
        "f0c": (128, 1), "f1c": (128, 1),
        "f2nA": (128, 1), "f2nB": (128, 1), "rbc": (10, 1),
    }
    mm_names = {"ATt", "F1T", "F2aT", "F2bT", "Sel"}
    dram = {}
    for name, shp in wshapes.items():
        dt_ = mmdt if name in mm_names else f32
        dram[name] = nc.dram_tensor(name, list(shp), dt_, kind="ExternalInput")
    dram["x0"] = nc.dram_tensor("x0", [8, BS], f32, kind="ExternalInput")
    dram["dxbf"] = nc.dram_tensor("dxbf", [NCH, 128, C * 2 * W], f32,
                                  kind="ExternalInput")
    dram["dxbc"] = nc.dram_tensor("dxbc", [128, m * P * 2 * BS], f32,
                                  kind="ExternalInput")
    out_dram = nc.dram_tensor("logits", [NCLS, BS], f32, kind="ExternalOutput")

    with tile.TileContext(nc) as tc, ExitStack() as ctx:
        const = ctx.enter_context(tc.tile_pool(name="const", bufs=1))
        dxbp = ctx.enter_context(tc.tile_pool(name="dxbp", bufs=2))
        work = ctx.enter_context(tc.tile_pool(name="work", bufs=2))
        state = ctx.enter_context(tc.tile_pool(name="state", bufs=1))
        psum = ctx.enter_context(tc.tile_pool(name="psum", bufs=1, space="PSUM"))

        ct = {}
        for name, shp in wshapes.items():
            dt_ = mmdt if name in mm_names else f32
            ct[name] = const.tile(list(shp), dt_, tag=name, name=f"c_{name}")
            nc.sync.dma_start(ct[name][:], dram[name][:])
        x0_t = const.tile([8, BS], f32, tag="x0")
        nc.sync.dma_start(x0_t[:], dram["x0"][:])
        dxbc_t = const.tile([128, m * P * 2 * BS], f32, tag="dxbc")
        nc.sync.dma_start(dxbc_t[:], dram["dxbc"][:])

        # ---- PSUM tiles (5 banks + readout) ----
        psum1 = psum.tile([128, W], f32, tag="psum1")
        psum2 = psum.tile([128, W], f32, tag="psum2")
        ps3a = psum.tile([128, W], f32, tag="ps3a")
        ps3b = psum.tile([128, W], f32, tag="ps3b")
        psum_y = psum.tile([32, W], f32, tag="psum_y")
        psl = psum.tile([NCLS, BS], f32, tag="psl")

        # ---- persistent state ----
        UpkA = state.tile([32, (P + 1) * BS], f32, tag="UpkA")
        UpkB = state.tile([32, (P + 1) * BS], f32, tag="UpkB")
        G_old = state.tile([32, W], f32, tag="G_old")
        F_sb = state.tile([32, W], f32, tag="F_sb")

        MM = dict(skip_group_check=True)

        def cs(t, s, n=1):
            """column slice of packed tile: slab block s, n blocks"""
            return t[:, s * BS:(s + n) * BS]

        # ================= shared eval bodies =================
        def eval_packed(dxb_ap, w, tagsfx, last):
            """One packed Euler eval at width w reading psum1[:, :w].
            Returns g [128, 2w] fp16; accumulates psum1 += A@g, psum_y += Sel@g."""
            e1 = work.tile([128, w], f32, tag="e1" + tagsfx)
            nc.scalar.activation(e1[:], psum1[:, 0:w], AF.Exp, bias=ct["f0c"][:])
            h1 = work.tile([128, w], mmdt, tag="h1" + tagsfx)
            nc.scalar.activation(h1[:], e1[:], AF.Ln, bias=1.0)
            nc.tensor.matmul(psum2[:, 0:w], ct["F1T"][:], h1[:],
                             start=True, stop=True, **MM)
            e2 = work.tile([128, w], f32, tag="e2" + tagsfx)
            nc.scalar.activation(e2[:], psum2[:, 0:w], AF.Exp, bias=ct["f1c"][:])
            h2 = work.tile([128, w], mmdt, tag="h2" + tagsfx)
            nc.scalar.activation(h2[:], e2[:], AF.Ln, bias=1.0)
            nc.tensor.matmul(ps3a[:, 0:w], ct["F2aT"][:], h2[:],
                             start=True, stop=True, **MM)
            nc.tensor.matmul(ps3b[:, 0:w], ct["F2bT"][:], h2[:],
                             start=True, stop=True, **MM)
            u = work.tile([128, 2 * w], f32, tag="u" + tagsfx)
            nc.scalar.activation(u[:, 0:w], ps3a[:, 0:w], AF.Exp,
                                 scale=-2.0, bias=ct["f2nA"][:])
            nc.scalar.activation(u[:, w:2 * w], ps3b[:, 0:w], AF.Exp,
                                 scale=-2.0, bias=ct["f2nB"][:])
            v = work.tile([128, 2 * w], f32, tag="v" + tagsfx)
            nc.scalar.activation(v[:], u[:], AF.Ln, bias=1.0)
            r = work.tile([128, 2 * w], f32, tag="r" + tagsfx)
            nc.scalar.activation(r[:], v[:], AF.Exp, scale=-1.0)
            g = work.tile([128, 2 * w], mmdt, tag="g" + tagsfx)
            nc.vector.scalar_tensor_tensor(g[:], r[:], -0.5, dxb_ap,
                                           OP.add, OP.mult)
            nc.tensor.matmul(psum1[:, 0:w], ct["ATt"][:], g[:, 0:w],
                             start=False, stop=False, **MM)
            nc.tensor.matmul(psum1[:, 0:w], ct["ATt"][:], g[:, w:2 * w],
                             start=False, stop=last, **MM)
            nc.tensor.matmul(psum_y[:, 0:w], ct["Sel"][:], g[:, 0:w],
                             start=False, stop=False, **MM)
            nc.tensor.matmul(psum_y[:, 0:w], ct["Sel"][:], g[:, w:2 * w],
                             start=False, stop=last, **MM)

        def softplus16(ps_in, bias_ap, out_tile, tagsfx):
            e = work.tile([128, BS], f32, tag="esp" + tagsfx)
            nc.scalar.activation(e[:], ps_in, AF.Exp, bias=bias_ap)
            nc.scalar.activation(out_tile[:], e[:], AF.Ln, bias=1.0)

        def coarse_G(ys_ap, s, out_ap):
            """m coarse Euler substeps from ys_ap ([32,16]); writes G(y) to
            out_ap via ACT."""
            nc.tensor.matmul(psum1[:, 0:BS], ct["F0T32"][:], ys_ap,
                             start=True, stop=False, **MM)
            nc.tensor.matmul(psum_y[:, 0:BS], ct["Id32"][:], ys_ap,
                             start=True, stop=False, **MM)
            for k in range(m):
                q = s * m + k
                last = (k == m - 1)
                eval_packed(dxbc_t[:, q * 2 * BS:(q + 1) * 2 * BS], BS, "c", last)
            nc.scalar.activation(out_ap, psum_y[:, 0:BS], AF.Identity)

        # ================= initial MLP =================
        nc.tensor.matmul(psum1[:, 0:BS], ct["W0T"][:], x0_t[:],
                         start=True, stop=True, **MM)
        hA = work.tile([128, BS], f32, tag="hA")
        softplus16(psum1[:, 0:BS], ct["b0c"][:], hA, "A")
        nc.tensor.matmul(psum2[:, 0:BS], ct["W1T"][:], hA[:],
                         start=True, stop=True, **MM)
        hB = work.tile([128, BS], f32, tag="hB")
        softplus16(psum2[:, 0:BS], ct["b1c"][:], hB, "B")
        nc.tensor.matmul(psum_y[:, 0:BS], ct["W2T"][:], hB[:],
                         start=True, stop=True, **MM)
        nc.scalar.activation(cs(UpkA, 0), psum_y[:, 0:BS], AF.Identity,
                             bias=ct["b2c"][:])
        nc.vector.tensor_copy(cs(UpkB, 0), cs(UpkA, 0))

        # ================= coarse init sweep =================
        for s in range(P):
            coarse_G(cs(UpkA, s), s, cs(G_old, s))
            nc.vector.tensor_copy(cs(UpkA, s + 1), cs(G_old, s))

        # ================= parareal iterations =================
        cur, nxt = UpkA, UpkB
        for j in range(J):
            # ---- fine sweep on cur ----
            nc.tensor.matmul(psum1[:, 0:W], ct["F0T32"][:], cur[:, 0:W],
                             start=True, stop=False, **MM)
            nc.tensor.matmul(psum_y[:, 0:W], ct["Id32"][:], cur[:, 0:W],
                             start=True, stop=False, **MM)
            for ch in range(NCH):
                dxb_t = dxbp.tile([128, C * 2 * W], f32, tag="dxbf")
                nc.sync.dma_start(dxb_t[:], dram["dxbf"][ch])
                for c in range(C):
                    i = ch * C + c
                    eval_packed(dxb_t[:, c * 2 * W:(c + 1) * 2 * W], W, "f",
                                i == NS - 1)
            nc.scalar.activation(F_sb[:], psum_y[:, 0:W], AF.Identity)

            # ---- correction sweep: cur -> nxt ----
            for s in range(P):
                t1 = work.tile([32, BS], f32, tag="t1")
                nc.vector.tensor_tensor(t1[:], cs(F_sb, s), cs(G_old, s),
                                        OP.subtract)
                coarse_G(cs(nxt, s), s, cs(G_old, s))
                nc.vector.tensor_tensor(cs(nxt, s + 1), cs(G_old, s), t1[:],
                                        OP.add)
            cur, nxt = nxt, cur

        # ================= readout =================
        nc.tensor.matmul(psl[:], ct["RT"][:], cs(cur, P),
                         start=True, stop=True, **MM)
        out_sb = work.tile([NCLS, BS], f32, tag="out_sb")
        nc.scalar.activation(out_sb[:], psl[:], AF.Identity, bias=ct["rbc"][:])
        nc.sync.dma_start(out_dram[:], out_sb[:])

    nc.compile()
    _NC_CACHE[key] = nc
    return nc


# --------------------------------------------------------------------------
# Public entry point
# --------------------------------------------------------------------------

def _prepare_inputs(ts, coeff_d, coeff_c, coeff_b, coeff_a,
                    W0, b0, W1, b1, W2, b2, F0, f0, F1, f1, F2, f2, R, rb):
    P, m, C = P_SLABS, M_COARSE, CHUNK
    NCH = NS // C
    ts = np.asarray(ts, dtype=_F32)
    coeff_a = np.asarray(coeff_a, _F32)
    cd, cc, cb = (np.asarray(a, _F32) for a in (coeff_d, coeff_c, coeff_b))

    # fine-step times (exactly the reference's grid) and coarse times
    t_fine = (ts[0] + _F32(DT0) * np.arange(NUM_STEPS, dtype=_F32)).astype(_F32)
    dts = np.minimum(_F32(DT0), ts[-1] - t_fine).astype(_F32)
    dx_fine = _dx_eval(ts, cd, cc, cb, t_fine)            # (2000, B, D)
    dx_fine = dx_fine * (2.0 * dts[:, None, None])

    slab_len = T1 / P
    h_c = slab_len / m
    t_coarse = np.array([s * slab_len + k * h_c
                         for s in range(P) for k in range(m)], dtype=_F32)
    dx_coarse = _dx_eval(ts, cd, cc, cb, t_coarse)        # (m*P, B, D)
    dx_coarse = dx_coarse * _F32(2.0 * h_c)

    Wt = _host_weights(*[np.asarray(a, _F32) for a in
                         (W0, b0, W1, b1, W2, b2, F0, f0, F1, f1, F2, f2, R, rb)])
    in_maps = []
    for core in range(NCORES):
        bsl = slice(core * BS, (core + 1) * BS)
        mmap = dict(Wt)
        mmap["x0"] = np.ascontiguousarray(coeff_a[bsl, 0, :].T)
        # fine dxb: [i, (s, j), d] packed layout
        dxc = dx_fine[:, bsl, :]                          # (2000, 16, 8)
        dxp = dxc.reshape(P, NS, BS, D).transpose(1, 0, 2, 3).reshape(NS, W, D)
        arr = _pack_layout(dxp, W)                        # (NS, 128, 2W)
        arr = arr.reshape(NCH, C, 128, 2 * W).transpose(0, 2, 1, 3)
        mmap["dxbf"] = np.ascontiguousarray(arr.reshape(NCH, 128, C * 2 * W))
        # coarse dxb: [q, j, d] -> [128, q-blocks of 2*BS]
        arrc = _pack_layout(dx_coarse[:, bsl, :], BS)     # (m*P, 128, 32)
        mmap["dxbc"] = np.ascontiguousarray(
            arrc.transpose(1, 0, 2).reshape(128, m * P * 2 * BS))
        in_maps.append(mmap)
    return in_maps


def kernel(ts, coeff_d, coeff_c, coeff_b, coeff_a,
           W0, b0, W1, b1, W2, b2, F0, f0, F1, f1, F2, f2, R, rb):
    from concourse.bass_utils import run_bass_kernel_spmd

    nc = _build_nc()
    in_maps = _prepare_inputs(ts, coeff_d, coeff_c, coeff_b, coeff_a,
                              W0, b0, W1, b1, W2, b2,
                              F0, f0, F1, f1, F2, f2, R, rb)
    res = run_bass_kernel_spmd(nc, in_maps, list(range(NCORES)))
    logits = np.concatenate(
        [res.results[i]["logits"].T for i in range(NCORES)], axis=0)
    return np.ascontiguousarray(logits.astype(np.float32))


# revision 5
# speedup vs baseline: 5.2577x; 1.7619x over previous
"""Trainium2 Bass kernel for a Neural CDE forward pass — parareal edition.

Model (see reference): 2000 fixed Euler steps (h=0.01) of
    y_{t+1} = y_t + dt * einsum('bhd,bd->bh', tanh-MLP(y_t).reshape(B,H,D), dX_t)
with a 3-layer softplus MLP (32 -> 128 -> 128 -> 256/tanh), batch B=128,
followed by a linear readout.

The reference trajectory is sensitive (its own discretization error vs the
true flow is ~5e-2), so the only way to match it within 2e-2 is to reproduce
its exact discrete Euler map.  The serial 2000-step chain is latency-bound
(~3us/step: every instruction has a ~200-600ns mostly-size-independent cost).

Strategy:
  * Pure data parallel over 8 NeuronCores (16 batch elements per core).
  * PARAREAL over time inside each core: split the 2000 steps into P=25
    slabs of 80 steps.  The fine propagator (exact Euler, h=0.01) advances
    ALL slabs simultaneously, packed side by side in the free dimension
    (width 25*16=400), so each 80-step fine sweep costs barely more than 80
    narrow steps.  A serial coarse propagator (Euler with m substeps per
    slab) plus the parareal correction
        U_{s+1} <- G(U_s) + F(U_s_prev) - G(U_s_prev)
    stitches the slabs together.  J iterations converge toward the exact
    fine (reference) map.  The coarse increment is the EXACT integral of dX
    over the slab (not an Euler sample), which tracks F so well that one
    iteration (J=1, m=1) reaches rel err 3.7e-5 in float64 simulation.
  * Feature-major layout: features on partitions, (slab, batch) on the free
    dim; every layer is one PE matmul with a constant fp16 lhsT.
  * softplus(x) = Ln(Exp(x)+1): two ACT ops (natural_log_exp table).
  * tanh section via the softplus identity  sigma(2z) = exp(-ln(1+e^{-2z})):
    Exp -> Ln -> Exp on ACT (cheaper than DVE reciprocal at width 400),
    then one DVE op  g = (sigma - 1/2) * (2 dX dt).
  * The -2*f2 bias of the func-MLP output layer rides the Exp bias port, so
    layer 3 needs no bias matmul.
  * The activation-table registry is pinned so Exp/Ln/Identity resolve to
    the single natural_log_exp_and_others table (one ACT_TABLE_LOAD total).
"""

import numpy as np

B = 128
NP_KNOTS = 128
D = 8
H = 32
WID = 128
NCLS = 10
T0, T1 = 0.0, 20.0
DT0 = 0.01
NUM_STEPS = 2000
NCORES = 8
BS = B // NCORES          # 16 batch per core

# parareal configuration
P_SLABS = 25              # time slabs per core
NS = NUM_STEPS // P_SLABS  # fine steps per slab (80)
M_COARSE = 1              # coarse substeps per slab
J_ITERS = 1               # parareal iterations
CHUNK = 10                # fine steps per DMA chunk
W = P_SLABS * BS          # packed fine width (400)

_F32 = np.float32
MM_DT = np.float16


# --------------------------------------------------------------------------
# Host-side precompute
# --------------------------------------------------------------------------

def _dx_eval(ts, coeff_d, coeff_c, coeff_b, t_grid):
    """Spline derivative dX/dt at the given times.  Returns (T, B, D)."""
    idx = np.clip(np.searchsorted(ts, t_grid, side="right") - 1, 0, NP_KNOTS - 2)
    fr = (t_grid - ts[idx]).astype(_F32)[None, :, None]
    dX = (coeff_b[:, idx] + _F32(2.0) * coeff_c[:, idx] * fr
          + _F32(3.0) * coeff_d[:, idx] * fr * fr)          # (B, T, D)
    return np.transpose(dX, (1, 0, 2)).astype(_F32)         # (T, B, D)


def _pack_layout(dx_scaled, cols):
    """(T, cols, D) -> (T, 128, 2*cols) d-major, h-broadcast layout.

    Partition p in col-half cb holds (d = cb*4 + p//32, h = p%32)."""
    T = dx_scaled.shape[0]
    tmp = dx_scaled.reshape(T, cols, 2, 4)          # [t, j, cb, dblk]
    tmp = np.transpose(tmp, (0, 3, 2, 1))           # [t, dblk, cb, j]
    tmp = np.broadcast_to(tmp[:, :, None, :, :], (T, 4, 32, 2, cols))
    return np.ascontiguousarray(tmp.reshape(T, 128, 2 * cols))


def _host_weights(W0, b0, W1, b1, W2, b2, F0, f0, F1, f1, F2, f2, R, rb):
    f32 = lambda a: np.ascontiguousarray(a, dtype=_F32)
    f16 = lambda a: np.ascontiguousarray(a, dtype=MM_DT)
    p = np.arange(256)
    perm = (p % 32) * 8 + p // 32
    F2p = F2[perm]
    f2p = f2[perm]
    return {
        "ATt":   f16(np.tile(F0.T, (4, 1))),          # (128,128)
        "F1T":   f16(F1.T),
        "F2aT":  f16(F2p[:128].T),
        "F2bT":  f16(F2p[128:].T),
        "Sel":   f16(np.tile(np.eye(32, dtype=_F32), (4, 1))),  # (128,32)
        "F0T32": f32(F0.T),                            # (32,128) fp32 lhsT
        "Id32":  f32(np.eye(32, dtype=_F32)),          # (32,32)
        "W0T":   f32(W0.T), "W1T": f32(W1.T), "W2T": f32(W2.T),
        "RT":    f32(R.T),                             # (32,10)
        "b0c":   f32(b0[:, None]), "b1c": f32(b1[:, None]),
        "b2c":   f32(b2[:, None]),
        "f0c":   f32(f0[:, None]), "f1c": f32(f1[:, None]),
        "f2nA":  f32(-2.0 * f2p[:128, None]),
        "f2nB":  f32(-2.0 * f2p[128:, None]),
        "rbc":   f32(rb[:, None]),
    }


# --------------------------------------------------------------------------
# Bass kernel build
# --------------------------------------------------------------------------

_NC_CACHE = {}


def _build_nc():
    key = (P_SLABS, M_COARSE, J_ITERS, CHUNK)
    if key in _NC_CACHE:
        return _NC_CACHE[key]

    import concourse.bacc as bacc
    import concourse.mybir as mybir
    import concourse.tile as tile
    from contextlib import ExitStack

    f32 = mybir.dt.float32
    mmdt = mybir.dt.from_np(np.dtype(MM_DT))
    AF = mybir.ActivationFunctionType
    OP = mybir.AluOpType

    import concourse.hw_specs as hw_specs
    _full_tabs = hw_specs.get_activation_tables("gen3")
    _ours = {AF.Exp, AF.Ln, AF.Identity, AF.Copy}
    _pinned = {
        name: (set(funcs) if name == "natural_log_exp_and_others"
               else set(funcs) - _ours)
        for name, funcs in _full_tabs.items()
    }
    bacc.get_activation_tables = lambda arch: _pinned

    P, m, J, C = P_SLABS, M_COARSE, J_ITERS, CHUNK
    NCH = NS // C
    assert NS % C == 0

    nc = bacc.Bacc("TRN2", target_bir_lowering=False, debug=False)

    # ---- DRAM I/O ----
    wshapes = {
        "ATt": (128, 128), "F1T": (128, 128), "F2aT": (128, 128),
        "F2bT": (128, 128), "Sel": (128, 32),
        "F0T32": (32, 128), "Id32": (32, 32),
        "W0T": (8, 128), "W1T": (128, 128), "W2T": (128, 32),
        "RT": (32, 10),
        "b0c": (128, 1), "b1c": (128, 1), "b2c": (32, 1),
        "f0c": (128, 1), "f1c": (128, 1),
        "f2nA": (128, 1), "f2nB": (128, 1), "rbc": (10, 1),
    }
    mm_names = {"ATt", "F1T", "F2aT", "F2bT", "Sel"}
    dram = {}
    for name, shp in wshapes.items():
        dt_ = mmdt if name in mm_names else f32
        dram[name] = nc.dram_tensor(name, list(shp), dt_, kind="ExternalInput")
    dram["x0"] = nc.dram_tensor("x0", [8, BS], f32, kind="ExternalInput")
    dram["dxbf"] = nc.dram_tensor("dxbf", [NCH, 128, C * 2 * W], f32,
                                  kind="ExternalInput")
    dram["dxbc"] = nc.dram_tensor("dxbc", [128, m * P * 2 * BS], f32,
                                  kind="ExternalInput")
    out_dram = nc.dram_tensor("logits", [NCLS, BS], f32, kind="ExternalOutput")

    with tile.TileContext(nc) as tc, ExitStack() as ctx:
        const = ctx.enter_context(tc.tile_pool(name="const", bufs=1))
        dxbp = ctx.enter_context(tc.tile_pool(name="dxbp", bufs=2))
        work = ctx.enter_context(tc.tile_pool(name="work", bufs=2))
        state = ctx.enter_context(tc.tile_pool(name="state", bufs=1))
        psum = ctx.enter_context(tc.tile_pool(name="psum", bufs=1, space="PSUM"))

        ct = {}
        for name, shp in wshapes.items():
            dt_ = mmdt if name in mm_names else f32
            ct[name] = const.tile(list(shp), dt_, tag=name, name=f"c_{name}")
            nc.sync.dma_start(ct[name][:], dram[name][:])
        x0_t = const.tile([8, BS], f32, tag="x0")
        nc.sync.dma_start(x0_t[:], dram["x0"][:])
        dxbc_t = const.tile([128, m * P * 2 * BS], f32, tag="dxbc")
        nc.sync.dma_start(dxbc_t[:], dram["dxbc"][:])

        # ---- PSUM tiles (5 banks + readout) ----
        psum1 = psum.tile([128, W], f32, tag="psum1")
        psum2 = psum.tile([128, W], f32, tag="psum2")
        ps3a = psum.tile([128, W], f32, tag="ps3a")
        ps3b = psum.tile([128, W], f32, tag="ps3b")
        psum_y = psum.tile([32, W], f32, tag="psum_y")
        psl = psum.tile([NCLS, BS], f32, tag="psl")

        # ---- persistent state ----
        UpkA = state.tile([32, (P + 1) * BS], f32, tag="UpkA")
        UpkB = state.tile([32, (P + 1) * BS], f32, tag="UpkB")
        G_old = state.tile([32, W], f32, tag="G_old")
        F_sb = state.tile([32, W], f32, tag="F_sb")

        MM = dict(skip_group_check=True)

        def cs(t, s, n=1):
            """column slice of packed tile: slab block s, n blocks"""
            return t[:, s * BS:(s + n) * BS]

        # ================= shared eval bodies =================
        def eval_packed(dxb_ap, w, tagsfx, last):
            """One packed Euler eval at width w reading psum1[:, :w].
            Returns g [128, 2w] fp16; accumulates psum1 += A@g, psum_y += Sel@g."""
            e1 = work.tile([128, w], f32, tag="e1" + tagsfx)
            nc.scalar.activation(e1[:], psum1[:, 0:w], AF.Exp, bias=ct["f0c"][:])
            h1 = work.tile([128, w], mmdt, tag="h1" + tagsfx)
            nc.scalar.activation(h1[:], e1[:], AF.Ln, bias=1.0)
            nc.tensor.matmul(psum2[:, 0:w], ct["F1T"][:], h1[:],
                             start=True, stop=True, **MM)
            e2 = work.tile([128, w], f32, tag="e2" + tagsfx)
            nc.scalar.activation(e2[:], psum2[:, 0:w], AF.Exp, bias=ct["f1c"][:])
            h2 = work.tile([128, w], mmdt, tag="h2" + tagsfx)
            nc.scalar.activation(h2[:], e2[:], AF.Ln, bias=1.0)
            nc.tensor.matmul(ps3a[:, 0:w], ct["F2aT"][:], h2[:],
                             start=True, stop=True, **MM)
            nc.tensor.matmul(ps3b[:, 0:w], ct["F2bT"][:], h2[:],
                             start=True, stop=True, **MM)
            u = work.tile([128, 2 * w], f32, tag="u" + tagsfx)
            nc.scalar.activation(u[:, 0:w], ps3a[:, 0:w], AF.Exp,
                                 scale=-2.0, bias=ct["f2nA"][:])
            nc.scalar.activation(u[:, w:2 * w], ps3b[:, 0:w], AF.Exp,
                                 scale=-2.0, bias=ct["f2nB"][:])
            v = work.tile([128, 2 * w], f32, tag="v" + tagsfx)
            nc.scalar.activation(v[:], u[:], AF.Ln, bias=1.0)
            r = work.tile([128, 2 * w], f32, tag="r" + tagsfx)
            nc.scalar.activation(r[:], v[:], AF.Exp, scale=-1.0)
            g = work.tile([128, 2 * w], mmdt, tag="g" + tagsfx)
            nc.vector.scalar_tensor_tensor(g[:], r[:], -0.5, dxb_ap,
                                           OP.add, OP.mult)
            nc.tensor.matmul(psum1[:, 0:w], ct["ATt"][:], g[:, 0:w],
                             start=False, stop=False, **MM)
            nc.tensor.matmul(psum1[:, 0:w], ct["ATt"][:], g[:, w:2 * w],
                             start=False, stop=last, **MM)
            nc.tensor.matmul(psum_y[:, 0:w], ct["Sel"][:], g[:, 0:w],
                             start=False, stop=False, **MM)
            nc.tensor.matmul(psum_y[:, 0:w], ct["Sel"][:], g[:, w:2 * w],
                             start=False, stop=last, **MM)

        def softplus16(ps_in, bias_ap, out_tile, tagsfx):
            e = work.tile([128, BS], f32, tag="esp" + tagsfx)
            nc.scalar.activation(e[:], ps_in, AF.Exp, bias=bias_ap)
            nc.scalar.activation(out_tile[:], e[:], AF.Ln, bias=1.0)

        def coarse_G(ys_ap, s, out_ap):
            """m coarse Euler substeps from ys_ap ([32,16]); writes G(y) to
            out_ap via ACT."""
            nc.tensor.matmul(psum1[:, 0:BS], ct["F0T32"][:], ys_ap,
                             start=True, stop=False, **MM)
            nc.tensor.matmul(psum_y[:, 0:BS], ct["Id32"][:], ys_ap,
                             start=True, stop=False, **MM)
            for k in range(m):
                q = s * m + k
                last = (k == m - 1)
                eval_packed(dxbc_t[:, q * 2 * BS:(q + 1) * 2 * BS], BS, "c", last)
            nc.scalar.activation(out_ap, psum_y[:, 0:BS], AF.Identity)

        # ================= initial MLP =================
        nc.tensor.matmul(psum1[:, 0:BS], ct["W0T"][:], x0_t[:],
                         start=True, stop=True, **MM)
        hA = work.tile([128, BS], f32, tag="hA")
        softplus16(psum1[:, 0:BS], ct["b0c"][:], hA, "A")
        nc.tensor.matmul(psum2[:, 0:BS], ct["W1T"][:], hA[:],
                         start=True, stop=True, **MM)
        hB = work.tile([128, BS], f32, tag="hB")
        softplus16(psum2[:, 0:BS], ct["b1c"][:], hB, "B")
        nc.tensor.matmul(psum_y[:, 0:BS], ct["W2T"][:], hB[:],
                         start=True, stop=True, **MM)
        nc.scalar.activation(cs(UpkA, 0), psum_y[:, 0:BS], AF.Identity,
                             bias=ct["b2c"][:])
        nc.vector.tensor_copy(cs(UpkB, 0), cs(UpkA, 0))

        # ================= coarse init sweep =================
        for s in range(P):
            coarse_G(cs(UpkA, s), s, cs(G_old, s))
            nc.vector.tensor_copy(cs(UpkA, s + 1), cs(G_old, s))

        # ================= parareal iterations =================
        cur, nxt = UpkA, UpkB
        for j in range(J):
            # ---- fine sweep on cur ----
            nc.tensor.matmul(psum1[:, 0:W], ct["F0T32"][:], cur[:, 0:W],
                             start=True, stop=False, **MM)
            nc.tensor.matmul(psum_y[:, 0:W], ct["Id32"][:], cur[:, 0:W],
                             start=True, stop=False, **MM)
            for ch in range(NCH):
                dxb_t = dxbp.tile([128, C * 2 * W], f32, tag="dxbf")
                nc.sync.dma_start(dxb_t[:], dram["dxbf"][ch])
                for c in range(C):
                    i = ch * C + c
                    eval_packed(dxb_t[:, c * 2 * W:(c + 1) * 2 * W], W, "f",
                                i == NS - 1)
            nc.scalar.activation(F_sb[:], psum_y[:, 0:W], AF.Identity)

            # ---- correction sweep: cur -> nxt ----
            for s in range(P):
                t1 = work.tile([32, BS], f32, tag="t1")
                nc.vector.tensor_tensor(t1[:], cs(F_sb, s), cs(G_old, s),
                                        OP.subtract)
                coarse_G(cs(nxt, s), s, cs(G_old, s))
                nc.vector.tensor_tensor(cs(nxt, s + 1), cs(G_old, s), t1[:],
                                        OP.add)
            cur, nxt = nxt, cur

        # ================= readout =================
        nc.tensor.matmul(psl[:], ct["RT"][:], cs(cur, P),
                         start=True, stop=True, **MM)
        out_sb = work.tile([NCLS, BS], f32, tag="out_sb")
        nc.scalar.activation(out_sb[:], psl[:], AF.Identity, bias=ct["rbc"][:])
        nc.sync.dma_start(out_dram[:], out_sb[:])

    nc.compile()
    _NC_CACHE[key] = nc
    return nc


# --------------------------------------------------------------------------
# Public entry point
# --------------------------------------------------------------------------

def _prepare_inputs(ts, coeff_d, coeff_c, coeff_b, coeff_a,
                    W0, b0, W1, b1, W2, b2, F0, f0, F1, f1, F2, f2, R, rb):
    P, m, C = P_SLABS, M_COARSE, CHUNK
    NCH = NS // C
    ts = np.asarray(ts, dtype=_F32)
    coeff_a = np.asarray(coeff_a, _F32)
    cd, cc, cb = (np.asarray(a, _F32) for a in (coeff_d, coeff_c, coeff_b))

    # fine-step times (exactly the reference's grid) and coarse times
    t_fine = (ts[0] + _F32(DT0) * np.arange(NUM_STEPS, dtype=_F32)).astype(_F32)
    dts = np.minimum(_F32(DT0), ts[-1] - t_fine).astype(_F32)
    dx_fine = _dx_eval(ts, cd, cc, cb, t_fine)            # (2000, B, D)
    dx_fine = dx_fine * (2.0 * dts[:, None, None])

    # coarse increments: EXACT integrals of the piecewise-quadratic dX over
    # each coarse sub-interval (the control increment the reference's Euler
    # steps are a Riemann sum of).  This makes G so close to F that a single
    # parareal iteration converges to ~4e-5.
    slab_len = T1 / P
    h_c = slab_len / m
    knot_d = np.diff(ts).astype(np.float64)
    cbd, ccd, cdd = (a.astype(np.float64) for a in (cb, cc, cd))
    # full-interval integrals of dX: T_k = b*d + c*d^2 + d*d^3, prefix-summed
    Tk = (cbd * knot_d[None, :, None] + ccd * knot_d[None, :, None] ** 2
          + cdd * knot_d[None, :, None] ** 3)             # (B, 127, D)
    pref = np.concatenate([np.zeros((B, 1, D)), np.cumsum(Tk, axis=1)], axis=1)

    def antider(t):  # prefix up to t's interval + local part -> (B, D)
        idx = int(np.clip(np.searchsorted(ts, t, side="right") - 1, 0, NP_KNOTS - 2))
        u = float(t) - float(ts[idx])
        return pref[:, idx] + cbd[:, idx] * u + ccd[:, idx] * u * u + cdd[:, idx] * u ** 3

    bounds = [s * slab_len + k * h_c for s in range(P) for k in range(m)]
    bounds.append(T1)
    A = np.stack([antider(t) for t in bounds])            # (m*P+1, B, D)
    dx_coarse = (A[1:] - A[:-1]).astype(_F32) * _F32(2.0)  # (m*P, B, D)

    Wt = _host_weights(*[np.asarray(a, _F32) for a in
                         (W0, b0, W1, b1, W2, b2, F0, f0, F1, f1, F2, f2, R, rb)])
    in_maps = []
    for core in range(NCORES):
        bsl = slice(core * BS, (core + 1) * BS)
        mmap = dict(Wt)
        mmap["x0"] = np.ascontiguousarray(coeff_a[bsl, 0, :].T)
        # fine dxb: [i, (s, j), d] packed layout
        dxc = dx_fine[:, bsl, :]                          # (2000, 16, 8)
        dxp = dxc.reshape(P, NS, BS, D).transpose(1, 0, 2, 3).reshape(NS, W, D)
        arr = _pack_layout(dxp, W)                        # (NS, 128, 2W)
        arr = arr.reshape(NCH, C, 128, 2 * W).transpose(0, 2, 1, 3)
        mmap["dxbf"] = np.ascontiguousarray(arr.reshape(NCH, 128, C * 2 * W))
        # coarse dxb: [q, j, d] -> [128, q-blocks of 2*BS]
        arrc = _pack_layout(dx_coarse[:, bsl, :], BS)     # (m*P, 128, 32)
        mmap["dxbc"] = np.ascontiguousarray(
            arrc.transpose(1, 0, 2).reshape(128, m * P * 2 * BS))
        in_maps.append(mmap)
    return in_maps


def kernel(ts, coeff_d, coeff_c, coeff_b, coeff_a,
           W0, b0, W1, b1, W2, b2, F0, f0, F1, f1, F2, f2, R, rb):
    from concourse.bass_utils import run_bass_kernel_spmd

    nc = _build_nc()
    in_maps = _prepare_inputs(ts, coeff_d, coeff_c, coeff_b, coeff_a,
                              W0, b0, W1, b1, W2, b2,
                              F0, f0, F1, f1, F2, f2, R, rb)
    res = run_bass_kernel_spmd(nc, in_maps, list(range(NCORES)))
    logits = np.concatenate(
        [res.results[i]["logits"].T for i in range(NCORES)], axis=0)
    return np.ascontiguousarray(logits.astype(np.float32))


# revision 6
# speedup vs baseline: 5.3432x; 1.0163x over previous
"""Trainium2 Bass kernel for a Neural CDE forward pass — parareal edition.

Model (see reference): 2000 fixed Euler steps (h=0.01) of
    y_{t+1} = y_t + dt * einsum('bhd,bd->bh', tanh-MLP(y_t).reshape(B,H,D), dX_t)
with a 3-layer softplus MLP (32 -> 128 -> 128 -> 256/tanh), batch B=128,
followed by a linear readout.

The reference trajectory is sensitive (its own discretization error vs the
true flow is ~5e-2), so the only way to match it within 2e-2 is to reproduce
its exact discrete Euler map.  The serial 2000-step chain is latency-bound
(~3us/step: every instruction has a ~200-600ns mostly-size-independent cost).

Strategy:
  * Pure data parallel over 8 NeuronCores (16 batch elements per core).
  * PARAREAL over time inside each core: split the 2000 steps into P=25
    slabs of 80 steps.  The fine propagator (exact Euler, h=0.01) advances
    ALL slabs simultaneously, packed side by side in the free dimension
    (width 25*16=400), so each 80-step fine sweep costs barely more than 80
    narrow steps.  A serial coarse propagator (Euler with m substeps per
    slab) plus the parareal correction
        U_{s+1} <- G(U_s) + F(U_s_prev) - G(U_s_prev)
    stitches the slabs together.  J iterations converge toward the exact
    fine (reference) map.  The coarse increment is the EXACT integral of dX
    over the slab (not an Euler sample), which tracks F so well that one
    iteration (J=1, m=1) reaches rel err 3.7e-5 in float64 simulation.
  * Feature-major layout: features on partitions, (slab, batch) on the free
    dim; every layer is one PE matmul with a constant fp16 lhsT.
  * softplus(x) = Ln(Exp(x)+1): two ACT ops (natural_log_exp table).
  * tanh section via the softplus identity  sigma(2z) = exp(-ln(1+e^{-2z})):
    Exp -> Ln -> Exp on ACT (cheaper than DVE reciprocal at width 400),
    then one DVE op  g = (sigma - 1/2) * (2 dX dt).
  * The -2*f2 bias of the func-MLP output layer rides the Exp bias port, so
    layer 3 needs no bias matmul.
  * The activation-table registry is pinned so Exp/Ln/Identity resolve to
    the single natural_log_exp_and_others table (one ACT_TABLE_LOAD total).
"""

import numpy as np

B = 128
NP_KNOTS = 128
D = 8
H = 32
WID = 128
NCLS = 10
T0, T1 = 0.0, 20.0
DT0 = 0.01
NUM_STEPS = 2000
NCORES = 8
BS = B // NCORES          # 16 batch per core

# parareal configuration
P_SLABS = 25              # time slabs per core
NS = NUM_STEPS // P_SLABS  # fine steps per slab (80)
M_COARSE = 1              # coarse substeps per slab
J_ITERS = 1               # parareal iterations
CHUNK = 10                # fine steps per DMA chunk
W = P_SLABS * BS          # packed fine width (400)

_F32 = np.float32
MM_DT = np.float16


# --------------------------------------------------------------------------
# Host-side precompute
# --------------------------------------------------------------------------

def _dx_eval(ts, coeff_d, coeff_c, coeff_b, t_grid):
    """Spline derivative dX/dt at the given times.  Returns (T, B, D)."""
    idx = np.clip(np.searchsorted(ts, t_grid, side="right") - 1, 0, NP_KNOTS - 2)
    fr = (t_grid - ts[idx]).astype(_F32)[None, :, None]
    dX = (coeff_b[:, idx] + _F32(2.0) * coeff_c[:, idx] * fr
          + _F32(3.0) * coeff_d[:, idx] * fr * fr)          # (B, T, D)
    return np.transpose(dX, (1, 0, 2)).astype(_F32)         # (T, B, D)


def _pack_layout(dx_scaled, cols):
    """(T, cols, D) -> (T, 128, 2*cols) d-major, h-broadcast layout.

    Partition p in col-half cb holds (d = cb*4 + p//32, h = p%32)."""
    T = dx_scaled.shape[0]
    tmp = dx_scaled.reshape(T, cols, 2, 4)          # [t, j, cb, dblk]
    tmp = np.transpose(tmp, (0, 3, 2, 1))           # [t, dblk, cb, j]
    tmp = np.broadcast_to(tmp[:, :, None, :, :], (T, 4, 32, 2, cols))
    return np.ascontiguousarray(tmp.reshape(T, 128, 2 * cols))


def _host_weights(W0, b0, W1, b1, W2, b2, F0, f0, F1, f1, F2, f2, R, rb):
    f32 = lambda a: np.ascontiguousarray(a, dtype=_F32)
    f16 = lambda a: np.ascontiguousarray(a, dtype=MM_DT)
    p = np.arange(256)
    perm = (p % 32) * 8 + p // 32
    F2p = F2[perm]
    f2p = f2[perm]
    return {
        "ATt":   f16(np.tile(F0.T, (4, 1))),          # (128,128)
        "F1T":   f16(F1.T),
        "F2aT":  f16(F2p[:128].T),
        "F2bT":  f16(F2p[128:].T),
        "Sel":   f16(np.tile(np.eye(32, dtype=_F32), (4, 1))),  # (128,32)
        "F0T32": f32(F0.T),                            # (32,128) fp32 lhsT
        "Id32":  f32(np.eye(32, dtype=_F32)),          # (32,32)
        "W0T":   f32(W0.T), "W1T": f32(W1.T), "W2T": f32(W2.T),
        "RT":    f32(R.T),                             # (32,10)
        "b0c":   f32(b0[:, None]), "b1c": f32(b1[:, None]),
        "b2c":   f32(b2[:, None]),
        "f0c":   f32(f0[:, None]), "f1c": f32(f1[:, None]),
        "f2nA":  f32(-2.0 * f2p[:128, None]),
        "f2nB":  f32(-2.0 * f2p[128:, None]),
        "rbc":   f32(rb[:, None]),
    }


# --------------------------------------------------------------------------
# Bass kernel build
# --------------------------------------------------------------------------

_NC_CACHE = {}


def _build_nc():
    key = (P_SLABS, M_COARSE, J_ITERS, CHUNK)
    if key in _NC_CACHE:
        return _NC_CACHE[key]

    import concourse.bacc as bacc
    import concourse.mybir as mybir
    import concourse.tile as tile
    from contextlib import ExitStack

    f32 = mybir.dt.float32
    mmdt = mybir.dt.from_np(np.dtype(MM_DT))
    AF = mybir.ActivationFunctionType
    OP = mybir.AluOpType

    import concourse.hw_specs as hw_specs
    _full_tabs = hw_specs.get_activation_tables("gen3")
    _ours = {AF.Exp, AF.Ln, AF.Identity, AF.Copy}
    _pinned = {
        name: (set(funcs) if name == "natural_log_exp_and_others"
               else set(funcs) - _ours)
        for name, funcs in _full_tabs.items()
    }
    bacc.get_activation_tables = lambda arch: _pinned

    P, m, J, C = P_SLABS, M_COARSE, J_ITERS, CHUNK
    NCH = NS // C
    assert NS % C == 0

    nc = bacc.Bacc("TRN2", target_bir_lowering=False, debug=False)

    # ---- DRAM I/O ----
    wshapes = {
        "ATt": (128, 128), "F1T": (128, 128), "F2aT": (128, 128),
        "F2bT": (128, 128), "Sel": (128, 32),
        "F0T32": (32, 128), "Id32": (32, 32),
        "W0T": (8, 128), "W1T": (128, 128), "W2T": (128, 32),
        "RT": (32, 10),
        "b0c": (128, 1), "b1c": (128, 1), "b2c": (32, 1),
        "f0c": (128, 1), "f1c": (128, 1),
        "f2nA": (128, 1), "f2nB": (128, 1), "rbc": (10, 1),
    }
    mm_names = {"ATt", "F1T", "F2aT", "F2bT", "Sel"}
    dram = {}
    for name, shp in wshapes.items():
        dt_ = mmdt if name in mm_names else f32
        dram[name] = nc.dram_tensor(name, list(shp), dt_, kind="ExternalInput")
    dram["x0"] = nc.dram_tensor("x0", [8, BS], f32, kind="ExternalInput")
    dram["dxbf"] = nc.dram_tensor("dxbf", [NCH, 128, C * 2 * W], f32,
                                  kind="ExternalInput")
    dram["dxbc"] = nc.dram_tensor("dxbc", [128, m * P * 2 * BS], f32,
                                  kind="ExternalInput")
    out_dram = nc.dram_tensor("logits", [NCLS, BS], f32, kind="ExternalOutput")

    with tile.TileContext(nc) as tc, ExitStack() as ctx:
        const = ctx.enter_context(tc.tile_pool(name="const", bufs=1))
        dxbp = ctx.enter_context(tc.tile_pool(name="dxbp", bufs=2))
        work = ctx.enter_context(tc.tile_pool(name="work", bufs=2))
        state = ctx.enter_context(tc.tile_pool(name="state", bufs=1))
        psum = ctx.enter_context(tc.tile_pool(name="psum", bufs=1, space="PSUM"))

        ct = {}
        for name, shp in wshapes.items():
            dt_ = mmdt if name in mm_names else f32
            ct[name] = const.tile(list(shp), dt_, tag=name, name=f"c_{name}")
            nc.sync.dma_start(ct[name][:], dram[name][:])
        x0_t = const.tile([8, BS], f32, tag="x0")
        nc.sync.dma_start(x0_t[:], dram["x0"][:])
        dxbc_t = const.tile([128, m * P * 2 * BS], f32, tag="dxbc")
        nc.sync.dma_start(dxbc_t[:], dram["dxbc"][:])

        # ---- PSUM tiles (5 banks + readout) ----
        psum1 = psum.tile([128, W], f32, tag="psum1")
        psum2 = psum.tile([128, W], f32, tag="psum2")
        ps3a = psum.tile([128, W], f32, tag="ps3a")
        ps3b = psum.tile([128, W], f32, tag="ps3b")
        psum_y = psum.tile([32, W], f32, tag="psum_y")
        psl = psum.tile([NCLS, BS], f32, tag="psl")

        # ---- persistent state ----
        UpkA = state.tile([32, (P + 1) * BS], f32, tag="UpkA")
        UpkB = state.tile([32, (P + 1) * BS], f32, tag="UpkB")
        G_old = state.tile([32, W], f32, tag="G_old")
        F_sb = state.tile([32, W], f32, tag="F_sb")

        MM = dict(skip_group_check=True)

        def cs(t, s, n=1):
            """column slice of packed tile: slab block s, n blocks"""
            return t[:, s * BS:(s + n) * BS]

        # ================= shared eval bodies =================
        def eval_packed(dxb_ap, w, tagsfx, last):
            """One packed Euler eval at width w reading psum1[:, :w].
            Returns g [128, 2w] fp16; accumulates psum1 += A@g, psum_y += Sel@g."""
            e1 = work.tile([128, w], f32, tag="e1" + tagsfx)
            nc.scalar.activation(e1[:], psum1[:, 0:w], AF.Exp, bias=ct["f0c"][:])
            h1 = work.tile([128, w], mmdt, tag="h1" + tagsfx)
            nc.scalar.activation(h1[:], e1[:], AF.Ln, bias=1.0)
            nc.tensor.matmul(psum2[:, 0:w], ct["F1T"][:], h1[:],
                             start=True, stop=True, **MM)
            e2 = work.tile([128, w], f32, tag="e2" + tagsfx)
            nc.scalar.activation(e2[:], psum2[:, 0:w], AF.Exp, bias=ct["f1c"][:])
            h2 = work.tile([128, w], mmdt, tag="h2" + tagsfx)
            nc.scalar.activation(h2[:], e2[:], AF.Ln, bias=1.0)
            nc.tensor.matmul(ps3a[:, 0:w], ct["F2aT"][:], h2[:],
                             start=True, stop=True, **MM)
            nc.tensor.matmul(ps3b[:, 0:w], ct["F2bT"][:], h2[:],
                             start=True, stop=True, **MM)
            u = work.tile([128, 2 * w], f32, tag="u" + tagsfx)
            nc.scalar.activation(u[:, 0:w], ps3a[:, 0:w], AF.Exp,
                                 scale=-2.0, bias=ct["f2nA"][:])
            nc.scalar.activation(u[:, w:2 * w], ps3b[:, 0:w], AF.Exp,
                                 scale=-2.0, bias=ct["f2nB"][:])
            v = work.tile([128, 2 * w], f32, tag="v" + tagsfx)
            nc.scalar.activation(v[:], u[:], AF.Ln, bias=1.0)
            r = work.tile([128, 2 * w], f32, tag="r" + tagsfx)
            g = work.tile([128, 2 * w], mmdt, tag="g" + tagsfx)
            if w >= 128:
                # split halves so mm1 of half 1 overlaps ACT/DVE of half 2
                nc.scalar.activation(r[:, 0:w], v[:, 0:w], AF.Exp, scale=-1.0)
                nc.vector.scalar_tensor_tensor(g[:, 0:w], r[:, 0:w], -0.5,
                                               dxb_ap[:, 0:w], OP.add, OP.mult)
                nc.scalar.activation(r[:, w:2 * w], v[:, w:2 * w], AF.Exp,
                                     scale=-1.0)
                nc.tensor.matmul(psum1[:, 0:w], ct["ATt"][:], g[:, 0:w],
                                 start=False, stop=False, **MM)
                nc.vector.scalar_tensor_tensor(g[:, w:2 * w], r[:, w:2 * w],
                                               -0.5, dxb_ap[:, w:2 * w],
                                               OP.add, OP.mult)
                nc.tensor.matmul(psum1[:, 0:w], ct["ATt"][:], g[:, w:2 * w],
                                 start=False, stop=last, **MM)
            else:
                nc.scalar.activation(r[:], v[:], AF.Exp, scale=-1.0)
                nc.vector.scalar_tensor_tensor(g[:], r[:], -0.5, dxb_ap,
                                               OP.add, OP.mult)
                nc.tensor.matmul(psum1[:, 0:w], ct["ATt"][:], g[:, 0:w],
                                 start=False, stop=False, **MM)
                nc.tensor.matmul(psum1[:, 0:w], ct["ATt"][:], g[:, w:2 * w],
                                 start=False, stop=last, **MM)
            nc.tensor.matmul(psum_y[:, 0:w], ct["Sel"][:], g[:, 0:w],
                             start=False, stop=False, **MM)
            nc.tensor.matmul(psum_y[:, 0:w], ct["Sel"][:], g[:, w:2 * w],
                             start=False, stop=last, **MM)

        def softplus16(ps_in, bias_ap, out_tile, tagsfx):
            e = work.tile([128, BS], f32, tag="esp" + tagsfx)
            nc.scalar.activation(e[:], ps_in, AF.Exp, bias=bias_ap)
            nc.scalar.activation(out_tile[:], e[:], AF.Ln, bias=1.0)

        def coarse_G(ys_ap, s, out_ap):
            """m coarse Euler substeps from ys_ap ([32,16]); writes G(y) to
            out_ap via ACT."""
            nc.tensor.matmul(psum1[:, 0:BS], ct["F0T32"][:], ys_ap,
                             start=True, stop=False, **MM)
            nc.tensor.matmul(psum_y[:, 0:BS], ct["Id32"][:], ys_ap,
                             start=True, stop=False, **MM)
            for k in range(m):
                q = s * m + k
                last = (k == m - 1)
                eval_packed(dxbc_t[:, q * 2 * BS:(q + 1) * 2 * BS], BS, "c", last)
            nc.scalar.activation(out_ap, psum_y[:, 0:BS], AF.Identity)

        # ================= initial MLP =================
        nc.tensor.matmul(psum1[:, 0:BS], ct["W0T"][:], x0_t[:],
                         start=True, stop=True, **MM)
        hA = work.tile([128, BS], f32, tag="hA")
        softplus16(psum1[:, 0:BS], ct["b0c"][:], hA, "A")
        nc.tensor.matmul(psum2[:, 0:BS], ct["W1T"][:], hA[:],
                         start=True, stop=True, **MM)
        hB = work.tile([128, BS], f32, tag="hB")
        softplus16(psum2[:, 0:BS], ct["b1c"][:], hB, "B")
        nc.tensor.matmul(psum_y[:, 0:BS], ct["W2T"][:], hB[:],
                         start=True, stop=True, **MM)
        nc.scalar.activation(cs(UpkA, 0), psum_y[:, 0:BS], AF.Identity,
                             bias=ct["b2c"][:])
        nc.vector.tensor_copy(cs(UpkB, 0), cs(UpkA, 0))

        # ================= coarse init sweep =================
        for s in range(P):
            coarse_G(cs(UpkA, s), s, cs(G_old, s))
            nc.vector.tensor_copy(cs(UpkA, s + 1), cs(G_old, s))

        # ================= parareal iterations =================
        cur, nxt = UpkA, UpkB
        for j in range(J):
            # ---- fine sweep on cur ----
            nc.tensor.matmul(psum1[:, 0:W], ct["F0T32"][:], cur[:, 0:W],
                             start=True, stop=False, **MM)
            nc.tensor.matmul(psum_y[:, 0:W], ct["Id32"][:], cur[:, 0:W],
                             start=True, stop=False, **MM)
            for ch in range(NCH):
                dxb_t = dxbp.tile([128, C * 2 * W], f32, tag="dxbf")
                nc.sync.dma_start(dxb_t[:], dram["dxbf"][ch])
                for c in range(C):
                    i = ch * C + c
                    eval_packed(dxb_t[:, c * 2 * W:(c + 1) * 2 * W], W, "f",
                                i == NS - 1)
            nc.scalar.activation(F_sb[:], psum_y[:, 0:W], AF.Identity)

            # ---- correction sweep: cur -> nxt ----
            for s in range(P):
                t1 = work.tile([32, BS], f32, tag="t1")
                nc.vector.tensor_tensor(t1[:], cs(F_sb, s), cs(G_old, s),
                                        OP.subtract)
                coarse_G(cs(nxt, s), s, cs(G_old, s))
                nc.vector.tensor_tensor(cs(nxt, s + 1), cs(G_old, s), t1[:],
                                        OP.add)
            cur, nxt = nxt, cur

        # ================= readout =================
        nc.tensor.matmul(psl[:], ct["RT"][:], cs(cur, P),
                         start=True, stop=True, **MM)
        out_sb = work.tile([NCLS, BS], f32, tag="out_sb")
        nc.scalar.activation(out_sb[:], psl[:], AF.Identity, bias=ct["rbc"][:])
        nc.sync.dma_start(out_dram[:], out_sb[:])

    nc.compile()
    _NC_CACHE[key] = nc
    return nc


# --------------------------------------------------------------------------
# Public entry point
# --------------------------------------------------------------------------

def _prepare_inputs(ts, coeff_d, coeff_c, coeff_b, coeff_a,
                    W0, b0, W1, b1, W2, b2, F0, f0, F1, f1, F2, f2, R, rb):
    P, m, C = P_SLABS, M_COARSE, CHUNK
    NCH = NS // C
    ts = np.asarray(ts, dtype=_F32)
    coeff_a = np.asarray(coeff_a, _F32)
    cd, cc, cb = (np.asarray(a, _F32) for a in (coeff_d, coeff_c, coeff_b))

    # fine-step times (exactly the reference's grid) and coarse times
    t_fine = (ts[0] + _F32(DT0) * np.arange(NUM_STEPS, dtype=_F32)).astype(_F32)
    dts = np.minimum(_F32(DT0), ts[-1] - t_fine).astype(_F32)
    dx_fine = _dx_eval(ts, cd, cc, cb, t_fine)            # (2000, B, D)
    dx_fine = dx_fine * (2.0 * dts[:, None, None])

    # coarse increments: EXACT integrals of the piecewise-quadratic dX over
    # each coarse sub-interval (the control increment the reference's Euler
    # steps are a Riemann sum of).  This makes G so close to F that a single
    # parareal iteration converges to ~4e-5.
    slab_len = T1 / P
    h_c = slab_len / m
    knot_d = np.diff(ts).astype(np.float64)
    cbd, ccd, cdd = (a.astype(np.float64) for a in (cb, cc, cd))
    # full-interval integrals of dX: T_k = b*d + c*d^2 + d*d^3, prefix-summed
    Tk = (cbd * knot_d[None, :, None] + ccd * knot_d[None, :, None] ** 2
          + cdd * knot_d[None, :, None] ** 3)             # (B, 127, D)
    pref = np.concatenate([np.zeros((B, 1, D)), np.cumsum(Tk, axis=1)], axis=1)

    def antider(t):  # prefix up to t's interval + local part -> (B, D)
        idx = int(np.clip(np.searchsorted(ts, t, side="right") - 1, 0, NP_KNOTS - 2))
        u = float(t) - float(ts[idx])
        return pref[:, idx] + cbd[:, idx] * u + ccd[:, idx] * u * u + cdd[:, idx] * u ** 3

    bounds = [s * slab_len + k * h_c for s in range(P) for k in range(m)]
    bounds.append(T1)
    A = np.stack([antider(t) for t in bounds])            # (m*P+1, B, D)
    dx_coarse = (A[1:] - A[:-1]).astype(_F32) * _F32(2.0)  # (m*P, B, D)

    Wt = _host_weights(*[np.asarray(a, _F32) for a in
                         (W0, b0, W1, b1, W2, b2, F0, f0, F1, f1, F2, f2, R, rb)])
    in_maps = []
    for core in range(NCORES):
        bsl = slice(core * BS, (core + 1) * BS)
        mmap = dict(Wt)
        mmap["x0"] = np.ascontiguousarray(coeff_a[bsl, 0, :].T)
        # fine dxb: [i, (s, j), d] packed layout
        dxc = dx_fine[:, bsl, :]                          # (2000, 16, 8)
        dxp = dxc.reshape(P, NS, BS, D).transpose(1, 0, 2, 3).reshape(NS, W, D)
        arr = _pack_layout(dxp, W)                        # (NS, 128, 2W)
        arr = arr.reshape(NCH, C, 128, 2 * W).transpose(0, 2, 1, 3)
        mmap["dxbf"] = np.ascontiguousarray(arr.reshape(NCH, 128, C * 2 * W))
        # coarse dxb: [q, j, d] -> [128, q-blocks of 2*BS]
        arrc = _pack_layout(dx_coarse[:, bsl, :], BS)     # (m*P, 128, 32)
        mmap["dxbc"] = np.ascontiguousarray(
            arrc.transpose(1, 0, 2).reshape(128, m * P * 2 * BS))
        in_maps.append(mmap)
    return in_maps


def kernel(ts, coeff_d, coeff_c, coeff_b, coeff_a,
           W0, b0, W1, b1, W2, b2, F0, f0, F1, f1, F2, f2, R, rb):
    from concourse.bass_utils import run_bass_kernel_spmd

    nc = _build_nc()
    in_maps = _prepare_inputs(ts, coeff_d, coeff_c, coeff_b, coeff_a,
                              W0, b0, W1, b1, W2, b2,
                              F0, f0, F1, f1, F2, f2, R, rb)
    res = run_bass_kernel_spmd(nc, in_maps, list(range(NCORES)))
    logits = np.concatenate(
        [res.results[i]["logits"].T for i in range(NCORES)], axis=0)
    return np.ascontiguousarray(logits.astype(np.float32))


# revision 7
# speedup vs baseline: 6.4066x; 1.1990x over previous
"""Trainium2 Bass kernel for a Neural CDE forward pass — parareal edition.

Model (see reference): 2000 fixed Euler steps (h=0.01) of
    y_{t+1} = y_t + dt * einsum('bhd,bd->bh', tanh-MLP(y_t).reshape(B,H,D), dX_t)
with a 3-layer softplus MLP (32 -> 128 -> 128 -> 256/tanh), batch B=128,
followed by a linear readout.

The reference trajectory is sensitive (its own discretization error vs the
true flow is ~5e-2), so the only way to match it within 2e-2 is to reproduce
its exact discrete Euler map.  The serial 2000-step chain is latency-bound
(~3us/step: every instruction has a ~200-600ns mostly-size-independent cost).

Strategy:
  * Pure data parallel over 8 NeuronCores (16 batch elements per core).
  * PARAREAL over time inside each core: split the 2000 steps into P=25
    slabs of 80 steps.  The fine propagator (exact Euler, h=0.01) advances
    ALL slabs simultaneously, packed side by side in the free dimension
    (width 25*16=400), so each 80-step fine sweep costs barely more than 80
    narrow steps.  A serial coarse propagator (Euler with m substeps per
    slab) plus the parareal correction
        U_{s+1} <- G(U_s) + F(U_s_prev) - G(U_s_prev)
    stitches the slabs together.  J iterations converge toward the exact
    fine (reference) map.  The coarse increment is the EXACT integral of dX
    over the slab (not an Euler sample), which tracks F so well that one
    iteration (J=1, m=1) reaches rel err 3.7e-5 in float64 simulation.
  * Feature-major layout: features on partitions, (slab, batch) on the free
    dim; every layer is one PE matmul with a constant fp16 lhsT.
  * softplus(x) = Ln(Exp(x)+1): two ACT ops (natural_log_exp table).
  * tanh section via the softplus identity  sigma(2z) = exp(-ln(1+e^{-2z})):
    Exp -> Ln -> Exp on ACT (cheaper than DVE reciprocal at width 400),
    then one DVE op  g = (sigma - 1/2) * (2 dX dt).
  * The -2*f2 bias of the func-MLP output layer rides the Exp bias port, so
    layer 3 needs no bias matmul.
  * The activation-table registry is pinned so Exp/Ln/Identity resolve to
    the single natural_log_exp_and_others table (one ACT_TABLE_LOAD total).
"""

import numpy as np

B = 128
NP_KNOTS = 128
D = 8
H = 32
WID = 128
NCLS = 10
T0, T1 = 0.0, 20.0
DT0 = 0.01
NUM_STEPS = 2000
NCORES = 8
BS = B // NCORES          # 16 batch per core

# parareal configuration
P_SLABS = 25              # time slabs per core
NS = NUM_STEPS // P_SLABS  # fine steps per slab (80)
M_COARSE = 1              # coarse substeps per slab
J_ITERS = 1               # parareal iterations
CHUNK = 10                # fine steps per DMA chunk
W = P_SLABS * BS          # packed fine width (400)

_F32 = np.float32
MM_DT = np.float16


# --------------------------------------------------------------------------
# Host-side precompute
# --------------------------------------------------------------------------

def _dx_eval(ts, coeff_d, coeff_c, coeff_b, t_grid):
    """Spline derivative dX/dt at the given times.  Returns (T, B, D)."""
    idx = np.clip(np.searchsorted(ts, t_grid, side="right") - 1, 0, NP_KNOTS - 2)
    fr = (t_grid - ts[idx]).astype(_F32)[None, :, None]
    dX = (coeff_b[:, idx] + _F32(2.0) * coeff_c[:, idx] * fr
          + _F32(3.0) * coeff_d[:, idx] * fr * fr)          # (B, T, D)
    return np.transpose(dX, (1, 0, 2)).astype(_F32)         # (T, B, D)


def _pack_layout(dx_scaled, cols):
    """(T, cols, D) -> (T, 128, 2*cols) d-major, h-broadcast layout.

    Partition p in col-half cb holds (d = cb*4 + p//32, h = p%32)."""
    T = dx_scaled.shape[0]
    tmp = dx_scaled.reshape(T, cols, 2, 4)          # [t, j, cb, dblk]
    tmp = np.transpose(tmp, (0, 3, 2, 1))           # [t, dblk, cb, j]
    tmp = np.broadcast_to(tmp[:, :, None, :, :], (T, 4, 32, 2, cols))
    return np.ascontiguousarray(tmp.reshape(T, 128, 2 * cols))


def _host_weights(W0, b0, W1, b1, W2, b2, F0, f0, F1, f1, F2, f2, R, rb):
    f32 = lambda a: np.ascontiguousarray(a, dtype=_F32)
    f16 = lambda a: np.ascontiguousarray(a, dtype=MM_DT)
    p = np.arange(256)
    perm = (p % 32) * 8 + p // 32
    F2p = F2[perm]
    f2p = f2[perm]
    return {
        "ATt":   f16(np.tile(F0.T, (4, 1))),          # (128,128)
        "F1T":   f16(F1.T),
        "F2aT":  f16(F2p[:128].T),
        "F2bT":  f16(F2p[128:].T),
        "Sel":   f16(np.tile(np.eye(32, dtype=_F32), (4, 1))),  # (128,32)
        "F0T32": f32(F0.T),                            # (32,128) fp32 lhsT
        "Id32":  f32(np.eye(32, dtype=_F32)),          # (32,32)
        "W0T":   f32(W0.T), "W1T": f32(W1.T), "W2T": f32(W2.T),
        "RT":    f32(R.T),                             # (32,10)
        "b0c":   f32(b0[:, None]), "b1c": f32(b1[:, None]),
        "b2c":   f32(b2[:, None]),
        "f0c":   f32(f0[:, None]), "f1c": f32(f1[:, None]),
        "f2nA":  f32(-2.0 * f2p[:128, None]),
        "f2nB":  f32(-2.0 * f2p[128:, None]),
        "rbc":   f32(rb[:, None]),
    }


# --------------------------------------------------------------------------
# Bass kernel build
# --------------------------------------------------------------------------

_NC_CACHE = {}


def _build_nc():
    key = (P_SLABS, M_COARSE, J_ITERS, CHUNK)
    if key in _NC_CACHE:
        return _NC_CACHE[key]

    import concourse.bacc as bacc
    import concourse.mybir as mybir
    import concourse.tile as tile
    from contextlib import ExitStack

    f32 = mybir.dt.float32
    mmdt = mybir.dt.from_np(np.dtype(MM_DT))
    AF = mybir.ActivationFunctionType
    OP = mybir.AluOpType

    import concourse.hw_specs as hw_specs
    _full_tabs = hw_specs.get_activation_tables("gen3")
    _ours = {AF.Exp, AF.Ln, AF.Identity, AF.Copy}
    _pinned = {
        name: (set(funcs) if name == "natural_log_exp_and_others"
               else set(funcs) - _ours)
        for name, funcs in _full_tabs.items()
    }
    bacc.get_activation_tables = lambda arch: _pinned

    P, m, J, C = P_SLABS, M_COARSE, J_ITERS, CHUNK
    NCH = NS // C
    assert NS % C == 0

    nc = bacc.Bacc("TRN2", target_bir_lowering=False, debug=False)

    # ---- DRAM I/O ----
    wshapes = {
        "ATt": (128, 128), "F1T": (128, 128), "F2aT": (128, 128),
        "F2bT": (128, 128), "Sel": (128, 32),
        "F0T32": (32, 128), "Id32": (32, 32),
        "W0T": (8, 128), "W1T": (128, 128), "W2T": (128, 32),
        "RT": (32, 10),
        "b0c": (128, 1), "b1c": (128, 1), "b2c": (32, 1),
        "f0c": (128, 1), "f1c": (128, 1),
        "f2nA": (128, 1), "f2nB": (128, 1), "rbc": (10, 1),
    }
    mm_names = {"ATt", "F1T", "F2aT", "F2bT", "Sel"}
    dram = {}
    for name, shp in wshapes.items():
        dt_ = mmdt if name in mm_names else f32
        dram[name] = nc.dram_tensor(name, list(shp), dt_, kind="ExternalInput")
    dram["x0"] = nc.dram_tensor("x0", [8, BS], f32, kind="ExternalInput")
    dram["dxbf"] = nc.dram_tensor("dxbf", [NCH, 128, C * 2 * W], f32,
                                  kind="ExternalInput")
    dram["dxbc"] = nc.dram_tensor("dxbc", [128, m * P * 2 * BS], f32,
                                  kind="ExternalInput")
    out_dram = nc.dram_tensor("logits", [NCLS, BS], f32, kind="ExternalOutput")

    with tile.TileContext(nc) as tc, ExitStack() as ctx:
        const = ctx.enter_context(tc.tile_pool(name="const", bufs=1))
        dxbp = ctx.enter_context(tc.tile_pool(name="dxbp", bufs=2))
        work = ctx.enter_context(tc.tile_pool(name="work", bufs=2))
        state = ctx.enter_context(tc.tile_pool(name="state", bufs=1))
        psum = ctx.enter_context(tc.tile_pool(name="psum", bufs=1, space="PSUM"))

        ct = {}
        for name, shp in wshapes.items():
            dt_ = mmdt if name in mm_names else f32
            ct[name] = const.tile(list(shp), dt_, tag=name, name=f"c_{name}")
            nc.sync.dma_start(ct[name][:], dram[name][:])
        x0_t = const.tile([8, BS], f32, tag="x0")
        nc.sync.dma_start(x0_t[:], dram["x0"][:])
        dxbc_t = const.tile([128, m * P * 2 * BS], f32, tag="dxbc")
        nc.sync.dma_start(dxbc_t[:], dram["dxbc"][:])

        # ---- PSUM tiles (5 banks + readout) ----
        psum1 = psum.tile([128, W], f32, tag="psum1")
        psum2 = psum.tile([128, W], f32, tag="psum2")
        ps3a = psum.tile([128, W], f32, tag="ps3a")
        ps3b = psum.tile([128, W], f32, tag="ps3b")
        psum_y = psum.tile([32, W], f32, tag="psum_y")
        psl = psum.tile([NCLS, BS], f32, tag="psl")

        # ---- persistent state ----
        UpkA = state.tile([32, (P + 1) * BS], f32, tag="UpkA")
        UpkB = state.tile([32, (P + 1) * BS], f32, tag="UpkB")
        G_old = state.tile([32, W], f32, tag="G_old")
        F_sb = state.tile([32, W], f32, tag="F_sb")

        MM = dict(skip_group_check=True)

        def cs(t, s, n=1):
            """column slice of packed tile: slab block s, n blocks"""
            return t[:, s * BS:(s + n) * BS]

        # ================= shared eval bodies =================
        def eval_packed(dxb_ap, w, tagsfx, last):
            """One packed Euler eval at width w reading psum1[:, :w].
            Returns g [128, 2w] fp16; accumulates psum1 += A@g, psum_y += Sel@g."""
            e1 = work.tile([128, w], f32, tag="e1" + tagsfx)
            nc.scalar.activation(e1[:], psum1[:, 0:w], AF.Exp, bias=ct["f0c"][:])
            h1 = work.tile([128, w], mmdt, tag="h1" + tagsfx)
            nc.scalar.activation(h1[:], e1[:], AF.Ln, bias=1.0)
            nc.tensor.matmul(psum2[:, 0:w], ct["F1T"][:], h1[:],
                             start=True, stop=True, **MM)
            e2 = work.tile([128, w], f32, tag="e2" + tagsfx)
            nc.scalar.activation(e2[:], psum2[:, 0:w], AF.Exp, bias=ct["f1c"][:])
            h2 = work.tile([128, w], mmdt, tag="h2" + tagsfx)
            nc.scalar.activation(h2[:], e2[:], AF.Ln, bias=1.0)
            nc.tensor.matmul(ps3a[:, 0:w], ct["F2aT"][:], h2[:],
                             start=True, stop=True, **MM)
            nc.tensor.matmul(ps3b[:, 0:w], ct["F2bT"][:], h2[:],
                             start=True, stop=True, **MM)
            u = work.tile([128, 2 * w], f32, tag="u" + tagsfx)
            nc.scalar.activation(u[:, 0:w], ps3a[:, 0:w], AF.Exp,
                                 scale=-2.0, bias=ct["f2nA"][:])
            nc.scalar.activation(u[:, w:2 * w], ps3b[:, 0:w], AF.Exp,
                                 scale=-2.0, bias=ct["f2nB"][:])
            v = work.tile([128, 2 * w], f32, tag="v" + tagsfx)
            nc.scalar.activation(v[:], u[:], AF.Ln, bias=1.0)
            r = work.tile([128, 2 * w], f32, tag="r" + tagsfx)
            g = work.tile([128, 2 * w], mmdt, tag="g" + tagsfx)
            if w >= 128:
                # split halves so mm1 of half 1 overlaps ACT/DVE of half 2
                nc.scalar.activation(r[:, 0:w], v[:, 0:w], AF.Exp, scale=-1.0)
                nc.vector.scalar_tensor_tensor(g[:, 0:w], r[:, 0:w], -0.5,
                                               dxb_ap[:, 0:w], OP.add, OP.mult)
                nc.scalar.activation(r[:, w:2 * w], v[:, w:2 * w], AF.Exp,
                                     scale=-1.0)
                nc.tensor.matmul(psum1[:, 0:w], ct["ATt"][:], g[:, 0:w],
                                 start=False, stop=False, **MM)
                nc.vector.scalar_tensor_tensor(g[:, w:2 * w], r[:, w:2 * w],
                                               -0.5, dxb_ap[:, w:2 * w],
                                               OP.add, OP.mult)
                nc.tensor.matmul(psum1[:, 0:w], ct["ATt"][:], g[:, w:2 * w],
                                 start=False, stop=last, **MM)
            else:
                nc.scalar.activation(r[:], v[:], AF.Exp, scale=-1.0)
                nc.vector.scalar_tensor_tensor(g[:], r[:], -0.5, dxb_ap,
                                               OP.add, OP.mult)
                nc.tensor.matmul(psum1[:, 0:w], ct["ATt"][:], g[:, 0:w],
                                 start=False, stop=False, **MM)
                nc.tensor.matmul(psum1[:, 0:w], ct["ATt"][:], g[:, w:2 * w],
                                 start=False, stop=last, **MM)
            if w < 128:
                # coarse: track y directly via Sel accumulation
                nc.tensor.matmul(psum_y[:, 0:w], ct["Sel"][:], g[:, 0:w],
                                 start=False, stop=False, **MM)
                nc.tensor.matmul(psum_y[:, 0:w], ct["Sel"][:], g[:, w:2 * w],
                                 start=False, stop=last, **MM)

        def softplus16(ps_in, bias_ap, out_tile, tagsfx):
            e = work.tile([128, BS], f32, tag="esp" + tagsfx)
            nc.scalar.activation(e[:], ps_in, AF.Exp, bias=bias_ap)
            nc.scalar.activation(out_tile[:], e[:], AF.Ln, bias=1.0)

        def coarse_G(ys_ap, s, out_ap):
            """m coarse Euler substeps from ys_ap ([32,16]); writes G(y) to
            out_ap via ACT."""
            nc.tensor.matmul(psum1[:, 0:BS], ct["F0T32"][:], ys_ap,
                             start=True, stop=False, **MM)
            nc.tensor.matmul(psum_y[:, 0:BS], ct["Id32"][:], ys_ap,
                             start=True, stop=False, **MM)
            for k in range(m):
                q = s * m + k
                last = (k == m - 1)
                eval_packed(dxbc_t[:, q * 2 * BS:(q + 1) * 2 * BS], BS, "c", last)
            nc.scalar.activation(out_ap, psum_y[:, 0:BS], AF.Identity)

        # ================= initial MLP =================
        nc.tensor.matmul(psum1[:, 0:BS], ct["W0T"][:], x0_t[:],
                         start=True, stop=True, **MM)
        hA = work.tile([128, BS], f32, tag="hA")
        softplus16(psum1[:, 0:BS], ct["b0c"][:], hA, "A")
        nc.tensor.matmul(psum2[:, 0:BS], ct["W1T"][:], hA[:],
                         start=True, stop=True, **MM)
        hB = work.tile([128, BS], f32, tag="hB")
        softplus16(psum2[:, 0:BS], ct["b1c"][:], hB, "B")
        nc.tensor.matmul(psum_y[:, 0:BS], ct["W2T"][:], hB[:],
                         start=True, stop=True, **MM)
        nc.scalar.activation(cs(UpkA, 0), psum_y[:, 0:BS], AF.Identity,
                             bias=ct["b2c"][:])
        nc.vector.tensor_copy(cs(UpkB, 0), cs(UpkA, 0))

        # ================= coarse init sweep =================
        for s in range(P):
            coarse_G(cs(UpkA, s), s, cs(G_old, s))
            nc.vector.tensor_copy(cs(UpkA, s + 1), cs(G_old, s))

        # ================= parareal iterations =================
        cur, nxt = UpkA, UpkB
        for j in range(J):
            # ---- fine sweep on cur ----
            nc.tensor.matmul(psum1[:, 0:W], ct["F0T32"][:], cur[:, 0:W],
                             start=True, stop=False, **MM)
            nc.tensor.matmul(psum_y[:, 0:W], ct["Id32"][:], cur[:, 0:W],
                             start=True, stop=False, **MM)
            for ch in range(NCH):
                dxb_t = dxbp.tile([128, C * 2 * W], f32, tag="dxbf")
                nc.sync.dma_start(dxb_t[:], dram["dxbf"][ch])
                for c in range(C):
                    i = ch * C + c
                    eval_packed(dxb_t[:, c * 2 * W:(c + 1) * 2 * W], W, "f",
                                i == NS - 1)
            nc.scalar.activation(F_sb[:], psum_y[:, 0:W], AF.Identity)

            # ---- correction sweep: cur -> nxt ----
            for s in range(P):
                t1 = work.tile([32, BS], f32, tag="t1")
                nc.vector.tensor_tensor(t1[:], cs(F_sb, s), cs(G_old, s),
                                        OP.subtract)
                coarse_G(cs(nxt, s), s, cs(G_old, s))
                nc.vector.tensor_tensor(cs(nxt, s + 1), cs(G_old, s), t1[:],
                                        OP.add)
            cur, nxt = nxt, cur

        # ================= readout =================
        nc.tensor.matmul(psl[:], ct["RT"][:], cs(cur, P),
                         start=True, stop=True, **MM)
        out_sb = work.tile([NCLS, BS], f32, tag="out_sb")
        nc.scalar.activation(out_sb[:], psl[:], AF.Identity, bias=ct["rbc"][:])
        nc.sync.dma_start(out_dram[:], out_sb[:])

    nc.compile()
    _NC_CACHE[key] = nc
    return nc


# --------------------------------------------------------------------------
# Public entry point
# --------------------------------------------------------------------------

def _prepare_inputs(ts, coeff_d, coeff_c, coeff_b, coeff_a,
                    W0, b0, W1, b1, W2, b2, F0, f0, F1, f1, F2, f2, R, rb):
    P, m, C = P_SLABS, M_COARSE, CHUNK
    NCH = NS // C
    ts = np.asarray(ts, dtype=_F32)
    coeff_a = np.asarray(coeff_a, _F32)
    cd, cc, cb = (np.asarray(a, _F32) for a in (coeff_d, coeff_c, coeff_b))

    # fine-step times (exactly the reference's grid) and coarse times
    t_fine = (ts[0] + _F32(DT0) * np.arange(NUM_STEPS, dtype=_F32)).astype(_F32)
    dts = np.minimum(_F32(DT0), ts[-1] - t_fine).astype(_F32)
    dx_fine = _dx_eval(ts, cd, cc, cb, t_fine)            # (2000, B, D)
    dx_fine = dx_fine * (2.0 * dts[:, None, None])

    # coarse increments: EXACT integrals of the piecewise-quadratic dX over
    # each coarse sub-interval (the control increment the reference's Euler
    # steps are a Riemann sum of).  This makes G so close to F that a single
    # parareal iteration converges to ~4e-5.
    slab_len = T1 / P
    h_c = slab_len / m
    knot_d = np.diff(ts).astype(np.float64)
    cbd, ccd, cdd = (a.astype(np.float64) for a in (cb, cc, cd))
    # full-interval integrals of dX: T_k = b*d + c*d^2 + d*d^3, prefix-summed
    Tk = (cbd * knot_d[None, :, None] + ccd * knot_d[None, :, None] ** 2
          + cdd * knot_d[None, :, None] ** 3)             # (B, 127, D)
    pref = np.concatenate([np.zeros((B, 1, D)), np.cumsum(Tk, axis=1)], axis=1)

    def antider(t):  # prefix up to t's interval + local part -> (B, D)
        idx = int(np.clip(np.searchsorted(ts, t, side="right") - 1, 0, NP_KNOTS - 2))
        u = float(t) - float(ts[idx])
        return pref[:, idx] + cbd[:, idx] * u + ccd[:, idx] * u * u + cdd[:, idx] * u ** 3

    bounds = [s * slab_len + k * h_c for s in range(P) for k in range(m)]
    bounds.append(T1)
    A = np.stack([antider(t) for t in bounds])            # (m*P+1, B, D)
    dx_coarse = (A[1:] - A[:-1]).astype(_F32) * _F32(2.0)  # (m*P, B, D)

    Wt = _host_weights(*[np.asarray(a, _F32) for a in
                         (W0, b0, W1, b1, W2, b2, F0, f0, F1, f1, F2, f2, R, rb)])
    in_maps = []
    for core in range(NCORES):
        bsl = slice(core * BS, (core + 1) * BS)
        mmap = dict(Wt)
        mmap["x0"] = np.ascontiguousarray(coeff_a[bsl, 0, :].T)
        # fine dxb: [i, (s, j), d] packed layout
        dxc = dx_fine[:, bsl, :]                          # (2000, 16, 8)
        dxp = dxc.reshape(P, NS, BS, D).transpose(1, 0, 2, 3).reshape(NS, W, D)
        arr = _pack_layout(dxp, W)                        # (NS, 128, 2W)
        arr = arr.reshape(NCH, C, 128, 2 * W).transpose(0, 2, 1, 3)
        mmap["dxbf"] = np.ascontiguousarray(arr.reshape(NCH, 128, C * 2 * W))
        # coarse dxb: [q, j, d] -> [128, q-blocks of 2*BS]
        arrc = _pack_layout(dx_coarse[:, bsl, :], BS)     # (m*P, 128, 32)
        mmap["dxbc"] = np.ascontiguousarray(
            arrc.transpose(1, 0, 2).reshape(128, m * P * 2 * BS))
        in_maps.append(mmap)
    return in_maps


def kernel(ts, coeff_d, coeff_c, coeff_b, coeff_a,
           W0, b0, W1, b1, W2, b2, F0, f0, F1, f1, F2, f2, R, rb):
    from concourse.bass_utils import run_bass_kernel_spmd

    nc = _build_nc()
    in_maps = _prepare_inputs(ts, coeff_d, coeff_c, coeff_b, coeff_a,
                              W0, b0, W1, b1, W2, b2,
                              F0, f0, F1, f1, F2, f2, R, rb)
    res = run_bass_kernel_spmd(nc, in_maps, list(range(NCORES)))
    logits = np.concatenate(
        [res.results[i]["logits"].T for i in range(NCORES)], axis=0)
    return np.ascontiguousarray(logits.astype(np.float32))


# revision 9
# speedup vs baseline: 6.5991x; 1.0300x over previous
"""Trainium2 Bass kernel for a Neural CDE forward pass — parareal edition.

Model (see reference): 2000 fixed Euler steps (h=0.01) of
    y_{t+1} = y_t + dt * einsum('bhd,bd->bh', tanh-MLP(y_t).reshape(B,H,D), dX_t)
with a 3-layer softplus MLP (32 -> 128 -> 128 -> 256/tanh), batch B=128,
followed by a linear readout.

The reference trajectory is sensitive (its own discretization error vs the
true flow is ~5e-2), so the only way to match it within 2e-2 is to reproduce
its exact discrete Euler map.  The serial 2000-step chain is latency-bound
(~3us/step: every instruction has a ~200-600ns mostly-size-independent cost).

Strategy:
  * Pure data parallel over 8 NeuronCores (16 batch elements per core).
  * PARAREAL over time inside each core: split the 2000 steps into P=25
    slabs of 80 steps.  The fine propagator (exact Euler, h=0.01) advances
    ALL slabs simultaneously, packed side by side in the free dimension
    (width 25*16=400), so each 80-step fine sweep costs barely more than 80
    narrow steps.  A serial coarse propagator (Euler with m substeps per
    slab) plus the parareal correction
        U_{s+1} <- G(U_s) + F(U_s_prev) - G(U_s_prev)
    stitches the slabs together.  J iterations converge toward the exact
    fine (reference) map.  The coarse increment is the EXACT integral of dX
    over the slab (not an Euler sample), which tracks F so well that one
    iteration (J=1, m=1) reaches rel err 3.7e-5 in float64 simulation.
  * Feature-major layout: features on partitions, (slab, batch) on the free
    dim; every layer is one PE matmul with a constant fp16 lhsT.
  * softplus(x) = Ln(Exp(x)+1): two ACT ops (natural_log_exp table).
  * tanh section via the softplus identity  sigma(2z) = exp(-ln(1+e^{-2z})):
    Exp -> Ln -> Exp on ACT (cheaper than DVE reciprocal at width 400),
    then one DVE op  g = (sigma - 1/2) * (2 dX dt).
  * The -2*f2 bias of the func-MLP output layer rides the Exp bias port, so
    layer 3 needs no bias matmul.
  * The activation-table registry is pinned so Exp/Ln/Identity resolve to
    the single natural_log_exp_and_others table (one ACT_TABLE_LOAD total).
"""

import numpy as np

B = 128
NP_KNOTS = 128
D = 8
H = 32
WID = 128
NCLS = 10
T0, T1 = 0.0, 20.0
DT0 = 0.01
NUM_STEPS = 2000
NCORES = 8
BS = B // NCORES          # 16 batch per core

# parareal configuration
P_SLABS = 25              # time slabs per core
NS = NUM_STEPS // P_SLABS  # fine steps per slab (80)
M_COARSE = 1              # coarse substeps per slab
J_ITERS = 1               # parareal iterations
CHUNK = 10                # fine steps per DMA chunk
W = P_SLABS * BS          # packed fine width (400)

_F32 = np.float32
MM_DT = np.float16


# --------------------------------------------------------------------------
# Host-side precompute
# --------------------------------------------------------------------------

def _dx_eval(ts, coeff_d, coeff_c, coeff_b, t_grid):
    """Spline derivative dX/dt at the given times.  Returns (T, B, D)."""
    idx = np.clip(np.searchsorted(ts, t_grid, side="right") - 1, 0, NP_KNOTS - 2)
    fr = (t_grid - ts[idx]).astype(_F32)[None, :, None]
    dX = (coeff_b[:, idx] + _F32(2.0) * coeff_c[:, idx] * fr
          + _F32(3.0) * coeff_d[:, idx] * fr * fr)          # (B, T, D)
    return np.transpose(dX, (1, 0, 2)).astype(_F32)         # (T, B, D)


def _pack_layout(dx_scaled, cols):
    """(T, cols, D) -> (T, 128, 2*cols) d-major, h-broadcast layout.

    Partition p in col-half cb holds (d = cb*4 + p//32, h = p%32)."""
    T = dx_scaled.shape[0]
    tmp = dx_scaled.reshape(T, cols, 2, 4)          # [t, j, cb, dblk]
    tmp = np.transpose(tmp, (0, 3, 2, 1))           # [t, dblk, cb, j]
    tmp = np.broadcast_to(tmp[:, :, None, :, :], (T, 4, 32, 2, cols))
    return np.ascontiguousarray(tmp.reshape(T, 128, 2 * cols))


def _host_weights(W0, b0, W1, b1, W2, b2, F0, f0, F1, f1, F2, f2, R, rb):
    f32 = lambda a: np.ascontiguousarray(a, dtype=_F32)
    f16 = lambda a: np.ascontiguousarray(a, dtype=MM_DT)
    p = np.arange(256)
    perm = (p % 32) * 8 + p // 32
    F2p = F2[perm]
    f2p = f2[perm]
    return {
        "ATt":   f16(np.tile(F0.T, (4, 1))),          # (128,128)
        "F1T":   f16(F1.T),
        "F2aT":  f16(F2p[:128].T),
        "F2bT":  f16(F2p[128:].T),
        "Sel":   f16(np.tile(np.eye(32, dtype=_F32), (4, 1))),  # (128,32)
        "F0T32": f32(F0.T),                            # (32,128) fp32 lhsT
        "PinvT": f32(np.linalg.pinv(F0.astype(np.float64)).T.astype(np.float32)),  # (128,32)
        "Id32":  f32(np.eye(32, dtype=_F32)),          # (32,32)
        "W0T":   f32(W0.T), "W1T": f32(W1.T), "W2T": f32(W2.T),
        "RT":    f32(R.T),                             # (32,10)
        "b0c":   f32(b0[:, None]), "b1c": f32(b1[:, None]),
        "b2c":   f32(b2[:, None]),
        "f0c":   f32(f0[:, None]), "f1c": f32(f1[:, None]),
        "f2nA":  f32(-2.0 * f2p[:128, None]),
        "f2nB":  f32(-2.0 * f2p[128:, None]),
        "rbc":   f32(rb[:, None]),
    }


# --------------------------------------------------------------------------
# Bass kernel build
# --------------------------------------------------------------------------

_NC_CACHE = {}


def _build_nc():
    key = (P_SLABS, M_COARSE, J_ITERS, CHUNK)
    if key in _NC_CACHE:
        return _NC_CACHE[key]

    import concourse.bacc as bacc
    import concourse.mybir as mybir
    import concourse.tile as tile
    from contextlib import ExitStack

    f32 = mybir.dt.float32
    mmdt = mybir.dt.from_np(np.dtype(MM_DT))
    AF = mybir.ActivationFunctionType
    OP = mybir.AluOpType

    import concourse.hw_specs as hw_specs
    _full_tabs = hw_specs.get_activation_tables("gen3")
    _ours = {AF.Exp, AF.Ln, AF.Identity, AF.Copy}
    _pinned = {
        name: (set(funcs) if name == "natural_log_exp_and_others"
               else set(funcs) - _ours)
        for name, funcs in _full_tabs.items()
    }
    bacc.get_activation_tables = lambda arch: _pinned

    P, m, J, C = P_SLABS, M_COARSE, J_ITERS, CHUNK
    NCH = NS // C
    assert NS % C == 0

    nc = bacc.Bacc("TRN2", target_bir_lowering=False, debug=False)

    # ---- DRAM I/O ----
    wshapes = {
        "ATt": (128, 128), "F1T": (128, 128), "F2aT": (128, 128),
        "F2bT": (128, 128), "Sel": (128, 32),
        "F0T32": (32, 128), "Id32": (32, 32), "PinvT": (128, 32),
        "W0T": (8, 128), "W1T": (128, 128), "W2T": (128, 32),
        "RT": (32, 10),
        "b0c": (128, 1), "b1c": (128, 1), "b2c": (32, 1),
        "f0c": (128, 1), "f1c": (128, 1),
        "f2nA": (128, 1), "f2nB": (128, 1), "rbc": (10, 1),
    }
    mm_names = {"ATt", "F1T", "F2aT", "F2bT", "Sel"}
    dram = {}
    for name, shp in wshapes.items():
        dt_ = mmdt if name in mm_names else f32
        dram[name] = nc.dram_tensor(name, list(shp), dt_, kind="ExternalInput")
    dram["x0"] = nc.dram_tensor("x0", [8, BS], f32, kind="ExternalInput")
    dram["dxbf"] = nc.dram_tensor("dxbf", [NCH, 128, C * 2 * W], f32,
                                  kind="ExternalInput")
    dram["dxbc"] = nc.dram_tensor("dxbc", [128, m * P * 2 * BS], f32,
                                  kind="ExternalInput")
    out_dram = nc.dram_tensor("logits", [NCLS, BS], f32, kind="ExternalOutput")

    with tile.TileContext(nc) as tc, ExitStack() as ctx:
        const = ctx.enter_context(tc.tile_pool(name="const", bufs=1))
        dxbp = ctx.enter_context(tc.tile_pool(name="dxbp", bufs=2))
        work = ctx.enter_context(tc.tile_pool(name="work", bufs=2))
        state = ctx.enter_context(tc.tile_pool(name="state", bufs=1))
        psum = ctx.enter_context(tc.tile_pool(name="psum", bufs=1, space="PSUM"))

        ct = {}
        for name, shp in wshapes.items():
            dt_ = mmdt if name in mm_names else f32
            ct[name] = const.tile(list(shp), dt_, tag=name, name=f"c_{name}")
            nc.sync.dma_start(ct[name][:], dram[name][:])
        x0_t = const.tile([8, BS], f32, tag="x0")
        nc.sync.dma_start(x0_t[:], dram["x0"][:])
        dxbc_t = const.tile([128, m * P * 2 * BS], f32, tag="dxbc")
        nc.sync.dma_start(dxbc_t[:], dram["dxbc"][:])

        # ---- PSUM tiles (5 banks + readout) ----
        psum1 = psum.tile([128, W], f32, tag="psum1")
        psum2 = psum.tile([128, W], f32, tag="psum2")
        ps3a = psum.tile([128, W], f32, tag="ps3a")
        ps3b = psum.tile([128, W], f32, tag="ps3b")
        psum_y = psum.tile([32, W], f32, tag="psum_y")
        psl = psum.tile([NCLS, BS], f32, tag="psl")

        # ---- persistent state ----
        UpkA = state.tile([32, (P + 1) * BS], f32, tag="UpkA")
        UpkB = state.tile([32, (P + 1) * BS], f32, tag="UpkB")
        G_old = state.tile([32, W], f32, tag="G_old")
        F_sb = state.tile([32, W], f32, tag="F_sb")

        MM = dict(skip_group_check=True)

        def cs(t, s, n=1):
            """column slice of packed tile: slab block s, n blocks"""
            return t[:, s * BS:(s + n) * BS]

        # ================= shared eval bodies =================
        def eval_packed(dxb_ap, w, tagsfx, last):
            """One packed Euler eval at width w reading psum1[:, :w].
            Returns g [128, 2w] fp16; accumulates psum1 += A@g, psum_y += Sel@g."""
            e1 = work.tile([128, w], f32, tag="e1" + tagsfx)
            nc.scalar.activation(e1[:], psum1[:, 0:w], AF.Exp, bias=ct["f0c"][:])
            h1 = work.tile([128, w], mmdt, tag="h1" + tagsfx)
            nc.scalar.activation(h1[:], e1[:], AF.Ln, bias=1.0)
            nc.tensor.matmul(psum2[:, 0:w], ct["F1T"][:], h1[:],
                             start=True, stop=True, **MM)
            e2 = work.tile([128, w], f32, tag="e2" + tagsfx)
            nc.scalar.activation(e2[:], psum2[:, 0:w], AF.Exp, bias=ct["f1c"][:])
            h2 = work.tile([128, w], mmdt, tag="h2" + tagsfx)
            nc.scalar.activation(h2[:], e2[:], AF.Ln, bias=1.0)
            nc.tensor.matmul(ps3a[:, 0:w], ct["F2aT"][:], h2[:],
                             start=True, stop=True, **MM)
            nc.tensor.matmul(ps3b[:, 0:w], ct["F2bT"][:], h2[:],
                             start=True, stop=True, **MM)
            u = work.tile([128, 2 * w], f32, tag="u" + tagsfx)
            nc.scalar.activation(u[:, 0:w], ps3a[:, 0:w], AF.Exp,
                                 scale=-2.0, bias=ct["f2nA"][:])
            nc.scalar.activation(u[:, w:2 * w], ps3b[:, 0:w], AF.Exp,
                                 scale=-2.0, bias=ct["f2nB"][:])
            v = work.tile([128, 2 * w], f32, tag="v" + tagsfx)
            nc.scalar.activation(v[:], u[:], AF.Ln, bias=1.0)
            r = work.tile([128, 2 * w], f32, tag="r" + tagsfx)
            g = work.tile([128, 2 * w], mmdt, tag="g" + tagsfx)
            if w >= 128:
                # split halves so mm1 of half 1 overlaps ACT/DVE of half 2
                nc.scalar.activation(r[:, 0:w], v[:, 0:w], AF.Exp, scale=-1.0)
                nc.vector.scalar_tensor_tensor(g[:, 0:w], r[:, 0:w], -0.5,
                                               dxb_ap[:, 0:w], OP.add, OP.mult)
                nc.scalar.activation(r[:, w:2 * w], v[:, w:2 * w], AF.Exp,
                                     scale=-1.0)
                nc.tensor.matmul(psum1[:, 0:w], ct["ATt"][:], g[:, 0:w],
                                 start=False, stop=False, **MM)
                nc.vector.scalar_tensor_tensor(g[:, w:2 * w], r[:, w:2 * w],
                                               -0.5, dxb_ap[:, w:2 * w],
                                               OP.add, OP.mult)
                nc.tensor.matmul(psum1[:, 0:w], ct["ATt"][:], g[:, w:2 * w],
                                 start=False, stop=last, **MM)
            else:
                nc.scalar.activation(r[:], v[:], AF.Exp, scale=-1.0)
                nc.vector.scalar_tensor_tensor(g[:], r[:], -0.5, dxb_ap,
                                               OP.add, OP.mult)
                nc.tensor.matmul(psum1[:, 0:w], ct["ATt"][:], g[:, 0:w],
                                 start=False, stop=False, **MM)
                nc.tensor.matmul(psum1[:, 0:w], ct["ATt"][:], g[:, w:2 * w],
                                 start=False, stop=last, **MM)
            if w < 128:
                # coarse: track y directly via Sel accumulation
                nc.tensor.matmul(psum_y[:, 0:w], ct["Sel"][:], g[:, 0:w],
                                 start=False, stop=False, **MM)
                nc.tensor.matmul(psum_y[:, 0:w], ct["Sel"][:], g[:, w:2 * w],
                                 start=False, stop=last, **MM)

        def softplus16(ps_in, bias_ap, out_tile, tagsfx):
            e = work.tile([128, BS], f32, tag="esp" + tagsfx)
            nc.scalar.activation(e[:], ps_in, AF.Exp, bias=bias_ap)
            nc.scalar.activation(out_tile[:], e[:], AF.Ln, bias=1.0)

        def coarse_G(ys_ap, s, out_ap):
            """m coarse Euler substeps from ys_ap ([32,16]); writes G(y) to
            out_ap via ACT."""
            nc.tensor.matmul(psum1[:, 0:BS], ct["F0T32"][:], ys_ap,
                             start=True, stop=False, **MM)
            nc.tensor.matmul(psum_y[:, 0:BS], ct["Id32"][:], ys_ap,
                             start=True, stop=False, **MM)
            for k in range(m):
                q = s * m + k
                last = (k == m - 1)
                eval_packed(dxbc_t[:, q * 2 * BS:(q + 1) * 2 * BS], BS, "c", last)
            nc.scalar.activation(out_ap, psum_y[:, 0:BS], AF.Identity)

        # ================= initial MLP =================
        nc.tensor.matmul(psum1[:, 0:BS], ct["W0T"][:], x0_t[:],
                         start=True, stop=True, **MM)
        hA = work.tile([128, BS], f32, tag="hA")
        softplus16(psum1[:, 0:BS], ct["b0c"][:], hA, "A")
        nc.tensor.matmul(psum2[:, 0:BS], ct["W1T"][:], hA[:],
                         start=True, stop=True, **MM)
        hB = work.tile([128, BS], f32, tag="hB")
        softplus16(psum2[:, 0:BS], ct["b1c"][:], hB, "B")
        nc.tensor.matmul(psum_y[:, 0:BS], ct["W2T"][:], hB[:],
                         start=True, stop=True, **MM)
        nc.scalar.activation(cs(UpkA, 0), psum_y[:, 0:BS], AF.Identity,
                             bias=ct["b2c"][:])
        nc.vector.tensor_copy(cs(UpkB, 0), cs(UpkA, 0))

        # ================= coarse init sweep =================
        for s in range(P):
            coarse_G(cs(UpkA, s), s, cs(G_old, s))
            nc.vector.tensor_copy(cs(UpkA, s + 1), cs(G_old, s))

        # ================= parareal iterations =================
        cur, nxt = UpkA, UpkB
        for j in range(J):
            # ---- fine sweep on cur ----
            nc.tensor.matmul(psum1[:, 0:W], ct["F0T32"][:], cur[:, 0:W],
                             start=True, stop=False, **MM)
            for ch in range(NCH):
                dxb_t = dxbp.tile([128, C * 2 * W], f32, tag="dxbf")
                nc.sync.dma_start(dxb_t[:], dram["dxbf"][ch])
                for c in range(C):
                    i = ch * C + c
                    eval_packed(dxb_t[:, c * 2 * W:(c + 1) * 2 * W], W, "f",
                                i == NS - 1)
            # slab endpoints: y = pinv(F0) @ psum1  (psum1 holds F0 @ y exactly)
            ps1_sb = work.tile([128, W], f32, tag="ps1sb")
            nc.scalar.activation(ps1_sb[:], psum1[:, 0:W], AF.Identity)
            nc.tensor.matmul(psum_y[:, 0:W], ct["PinvT"][:], ps1_sb[:],
                             start=True, stop=True, **MM)
            nc.scalar.activation(F_sb[:], psum_y[:, 0:W], AF.Identity)

            # ---- correction sweep: cur -> nxt ----
            for s in range(P):
                t1 = work.tile([32, BS], f32, tag="t1")
                nc.vector.tensor_tensor(t1[:], cs(F_sb, s), cs(G_old, s),
                                        OP.subtract)
                coarse_G(cs(nxt, s), s, cs(G_old, s))
                nc.vector.tensor_tensor(cs(nxt, s + 1), cs(G_old, s), t1[:],
                                        OP.add)
            cur, nxt = nxt, cur

        # ================= readout =================
        nc.tensor.matmul(psl[:], ct["RT"][:], cs(cur, P),
                         start=True, stop=True, **MM)
        out_sb = work.tile([NCLS, BS], f32, tag="out_sb")
        nc.scalar.activation(out_sb[:], psl[:], AF.Identity, bias=ct["rbc"][:])
        nc.sync.dma_start(out_dram[:], out_sb[:])

    nc.compile()
    _NC_CACHE[key] = nc
    return nc


# --------------------------------------------------------------------------
# Public entry point
# --------------------------------------------------------------------------

def _prepare_inputs(ts, coeff_d, coeff_c, coeff_b, coeff_a,
                    W0, b0, W1, b1, W2, b2, F0, f0, F1, f1, F2, f2, R, rb):
    P, m, C = P_SLABS, M_COARSE, CHUNK
    NCH = NS // C
    ts = np.asarray(ts, dtype=_F32)
    coeff_a = np.asarray(coeff_a, _F32)
    cd, cc, cb = (np.asarray(a, _F32) for a in (coeff_d, coeff_c, coeff_b))

    # fine-step times (exactly the reference's grid) and coarse times
    t_fine = (ts[0] + _F32(DT0) * np.arange(NUM_STEPS, dtype=_F32)).astype(_F32)
    dts = np.minimum(_F32(DT0), ts[-1] - t_fine).astype(_F32)
    dx_fine = _dx_eval(ts, cd, cc, cb, t_fine)            # (2000, B, D)
    dx_fine = dx_fine * (2.0 * dts[:, None, None])

    # coarse increments: EXACT integrals of the piecewise-quadratic dX over
    # each coarse sub-interval (the control increment the reference's Euler
    # steps are a Riemann sum of).  This makes G so close to F that a single
    # parareal iteration converges to ~4e-5.
    slab_len = T1 / P
    h_c = slab_len / m
    knot_d = np.diff(ts).astype(np.float64)
    cbd, ccd, cdd = (a.astype(np.float64) for a in (cb, cc, cd))
    # full-interval integrals of dX: T_k = b*d + c*d^2 + d*d^3, prefix-summed
    Tk = (cbd * knot_d[None, :, None] + ccd * knot_d[None, :, None] ** 2
          + cdd * knot_d[None, :, None] ** 3)             # (B, 127, D)
    pref = np.concatenate([np.zeros((B, 1, D)), np.cumsum(Tk, axis=1)], axis=1)

    def antider(t):  # prefix up to t's interval + local part -> (B, D)
        idx = int(np.clip(np.searchsorted(ts, t, side="right") - 1, 0, NP_KNOTS - 2))
        u = float(t) - float(ts[idx])
        return pref[:, idx] + cbd[:, idx] * u + ccd[:, idx] * u * u + cdd[:, idx] * u ** 3

    bounds = [s * slab_len + k * h_c for s in range(P) for k in range(m)]
    bounds.append(T1)
    A = np.stack([antider(t) for t in bounds])            # (m*P+1, B, D)
    dx_coarse = (A[1:] - A[:-1]).astype(_F32) * _F32(2.0)  # (m*P, B, D)

    Wt = _host_weights(*[np.asarray(a, _F32) for a in
                         (W0, b0, W1, b1, W2, b2, F0, f0, F1, f1, F2, f2, R, rb)])
    in_maps = []
    for core in range(NCORES):
        bsl = slice(core * BS, (core + 1) * BS)
        mmap = dict(Wt)
        mmap["x0"] = np.ascontiguousarray(coeff_a[bsl, 0, :].T)
        # fine dxb: [i, (s, j), d] packed layout
        dxc = dx_fine[:, bsl, :]                          # (2000, 16, 8)
        dxp = dxc.reshape(P, NS, BS, D).transpose(1, 0, 2, 3).reshape(NS, W, D)
        arr = _pack_layout(dxp, W)                        # (NS, 128, 2W)
        arr = arr.reshape(NCH, C, 128, 2 * W).transpose(0, 2, 1, 3)
        mmap["dxbf"] = np.ascontiguousarray(arr.reshape(NCH, 128, C * 2 * W))
        # coarse dxb: [q, j, d] -> [128, q-blocks of 2*BS]
        arrc = _pack_layout(dx_coarse[:, bsl, :], BS)     # (m*P, 128, 32)
        mmap["dxbc"] = np.ascontiguousarray(
            arrc.transpose(1, 0, 2).reshape(128, m * P * 2 * BS))
        in_maps.append(mmap)
    return in_maps


def kernel(ts, coeff_d, coeff_c, coeff_b, coeff_a,
           W0, b0, W1, b1, W2, b2, F0, f0, F1, f1, F2, f2, R, rb):
    from concourse.bass_utils import run_bass_kernel_spmd

    nc = _build_nc()
    in_maps = _prepare_inputs(ts, coeff_d, coeff_c, coeff_b, coeff_a,
                              W0, b0, W1, b1, W2, b2,
                              F0, f0, F1, f1, F2, f2, R, rb)
    res = run_bass_kernel_spmd(nc, in_maps, list(range(NCORES)))
    logits = np.concatenate(
        [res.results[i]["logits"].T for i in range(NCORES)], axis=0)
    return np.ascontiguousarray(logits.astype(np.float32))


# revision 11
# speedup vs baseline: 8.0425x; 1.2187x over previous
"""Trainium2 Bass kernel for a Neural CDE forward pass — parareal edition.

Model (see reference): 2000 fixed Euler steps (h=0.01) of
    y_{t+1} = y_t + dt * einsum('bhd,bd->bh', tanh-MLP(y_t).reshape(B,H,D), dX_t)
with a 3-layer softplus MLP (32 -> 128 -> 128 -> 256/tanh), batch B=128,
followed by a linear readout.

The reference trajectory is sensitive (its own discretization error vs the
true flow is ~1e-2), so the only way to match it within 2e-2 is to reproduce
its exact discrete Euler map.  The serial step chain is latency-bound
(~200-700ns per instruction, nearly width-independent up to ~400 cols).

Strategy:
  * Pure data parallel over 8 NeuronCores (16 batch elements per core).
  * PARAREAL over time inside each core: 50 slabs of 40 fine steps.  The
    fine propagator (exact Euler, h=0.01) advances slabs 0-24 and 25-49 as
    TWO independent packed chains (width 400 each) whose instructions
    interleave on the engines — 40 serial chain-steps cover all 2000.
  * Coarse propagator G: one Euler step per slab using the EXACT integral
    of dX over the slab (host-computed piecewise-quadratic antiderivative).
    G tracks F so well that ONE parareal iteration
        U_{s+1} <- G(U_s) + F(U_s_prev) - G(U_s_prev)
    reaches ~3e-5 rel err in float64 simulation.
  * Both coarse sweeps roll the state in PSUM (psum1 = F0 @ y, updated via
    psum1 += tile(F0.T) @ g); y is never materialized on the serial chain.
    The init sweep extracts slab states off-chain via a Sel accumulator;
    the correction sweep injects the packed parareal defect D = F - G_old
    with one K=32 matmul per slab and needs no extraction at all — the
    readout is (R @ pinv(F0)) @ psum1 + rb.
  * Feature-major layout: features on partitions, (slab, batch) on the free
    dim; every layer is one PE matmul with a constant fp16 lhsT.
  * softplus(x) = Ln(Exp(x)+1): two ACT ops (natural_log_exp table, pinned
    so the chooser never inserts ACT_TABLE_LOADs).
  * tanh section on DVE: u = exp(-2z-2f2) (ACT, f2 bias on the bias port),
    w = min(1+u, 1e30), r ~ 1/w, g = (r - 1/2) * (2 dX dt) — keeps ACT
    below saturation when two fine chains interleave.
"""

import numpy as np

B = 128
NP_KNOTS = 128
D = 8
H = 32
WID = 128
NCLS = 10
T0, T1 = 0.0, 20.0
DT0 = 0.01
NUM_STEPS = 2000
NCORES = 8
BS = B // NCORES          # 16 batch per core

# parareal configuration
P_SLABS = 50              # time slabs per core (two packed chains of 25)
NG = P_SLABS // 2         # slabs per chain
NS = NUM_STEPS // P_SLABS  # fine steps per slab (40)
CHUNK = 5                 # fine steps per DMA chunk
W = NG * BS               # packed width per chain (400)

_F32 = np.float32
MM_DT = np.float16


# --------------------------------------------------------------------------
# Host-side precompute
# --------------------------------------------------------------------------

def _dx_eval(ts, coeff_d, coeff_c, coeff_b, t_grid):
    """Spline derivative dX/dt at the given times.  Returns (T, B, D)."""
    idx = np.clip(np.searchsorted(ts, t_grid, side="right") - 1, 0, NP_KNOTS - 2)
    fr = (t_grid - ts[idx]).astype(_F32)[None, :, None]
    dX = (coeff_b[:, idx] + _F32(2.0) * coeff_c[:, idx] * fr
          + _F32(3.0) * coeff_d[:, idx] * fr * fr)          # (B, T, D)
    return np.transpose(dX, (1, 0, 2)).astype(_F32)         # (T, B, D)


def _pack_layout(dx_scaled, cols):
    """(T, cols, D) -> (T, 128, 2*cols) d-major, h-broadcast layout.

    Partition p in col-half cb holds (d = cb*4 + p//32, h = p%32)."""
    T = dx_scaled.shape[0]
    tmp = dx_scaled.reshape(T, cols, 2, 4)          # [t, j, cb, dblk]
    tmp = np.transpose(tmp, (0, 3, 2, 1))           # [t, dblk, cb, j]
    tmp = np.broadcast_to(tmp[:, :, None, :, :], (T, 4, 32, 2, cols))
    return np.ascontiguousarray(tmp.reshape(T, 128, 2 * cols))


def _host_weights(W0, b0, W1, b1, W2, b2, F0, f0, F1, f1, F2, f2, R, rb):
    f32 = lambda a: np.ascontiguousarray(a, dtype=_F32)
    f16 = lambda a: np.ascontiguousarray(a, dtype=MM_DT)
    p = np.arange(256)
    perm = (p % 32) * 8 + p // 32
    F2p = F2[perm]
    f2p = f2[perm]
    pinv = np.linalg.pinv(F0.astype(np.float64))
    return {
        "ATt":   f16(np.tile(F0.T, (4, 1))),          # (128,128)
        "F1T":   f16(F1.T),
        "F2aT":  f16(F2p[:128].T),
        "F2bT":  f16(F2p[128:].T),
        "Sel":   f16(np.tile(np.eye(32, dtype=_F32), (4, 1))),  # (128,32)
        "F0T32": f32(F0.T),                            # (32,128) fp32 lhsT
        "PinvT": f32(pinv.T.astype(np.float32)),       # (128,32)
        "RPinvT": f32((R.astype(np.float64) @ pinv).T.astype(np.float32)),  # (128,10)
        "Id32":  f32(np.eye(32, dtype=_F32)),          # (32,32)
        "W0T":   f32(W0.T), "W1T": f32(W1.T), "W2T": f32(W2.T),
        "b0c":   f32(b0[:, None]), "b1c": f32(b1[:, None]),
        "b2c":   f32(b2[:, None]),
        "f0c":   f32(f0[:, None]), "f1c": f32(f1[:, None]),
        "f2nA":  f32(-2.0 * f2p[:128, None]),
        "f2nB":  f32(-2.0 * f2p[128:, None]),
        "rbc":   f32(rb[:, None]),
    }


# --------------------------------------------------------------------------
# Bass kernel build
# --------------------------------------------------------------------------

_NC_CACHE = {}


def _build_nc():
    key = (P_SLABS, CHUNK)
    if key in _NC_CACHE:
        return _NC_CACHE[key]

    import concourse.bacc as bacc
    import concourse.mybir as mybir
    import concourse.tile as tile
    from contextlib import ExitStack

    f32 = mybir.dt.float32
    mmdt = mybir.dt.from_np(np.dtype(MM_DT))
    AF = mybir.ActivationFunctionType
    OP = mybir.AluOpType

    import concourse.hw_specs as hw_specs
    _full_tabs = hw_specs.get_activation_tables("gen3")
    _ours = {AF.Exp, AF.Ln, AF.Identity, AF.Copy}
    _pinned = {
        name: (set(funcs) if name == "natural_log_exp_and_others"
               else set(funcs) - _ours)
        for name, funcs in _full_tabs.items()
    }
    bacc.get_activation_tables = lambda arch: _pinned

    P, C = P_SLABS, CHUNK
    NCH = NS // C
    assert NS % C == 0

    nc = bacc.Bacc("TRN2", target_bir_lowering=False, debug=False)

    # ---- DRAM I/O ----
    wshapes = {
        "ATt": (128, 128), "F1T": (128, 128), "F2aT": (128, 128),
        "F2bT": (128, 128), "Sel": (128, 32),
        "F0T32": (32, 128), "Id32": (32, 32), "PinvT": (128, 32),
        "RPinvT": (128, 10),
        "W0T": (8, 128), "W1T": (128, 128), "W2T": (128, 32),
        "b0c": (128, 1), "b1c": (128, 1), "b2c": (32, 1),
        "f0c": (128, 1), "f1c": (128, 1),
        "f2nA": (128, 1), "f2nB": (128, 1), "rbc": (10, 1),
    }
    mm_names = {"ATt", "F1T", "F2aT", "F2bT", "Sel"}
    dram = {}
    for name, shp in wshapes.items():
        dt_ = mmdt if name in mm_names else f32
        dram[name] = nc.dram_tensor(name, list(shp), dt_, kind="ExternalInput")
    dram["x0"] = nc.dram_tensor("x0", [8, BS], f32, kind="ExternalInput")
    dram["dxbfA"] = nc.dram_tensor("dxbfA", [NCH, 128, C * 2 * W], f32,
                                   kind="ExternalInput")
    dram["dxbfB"] = nc.dram_tensor("dxbfB", [NCH, 128, C * 2 * W], f32,
                                   kind="ExternalInput")
    dram["dxbc"] = nc.dram_tensor("dxbc", [128, P * 2 * BS], f32,
                                  kind="ExternalInput")
    out_dram = nc.dram_tensor("logits", [NCLS, BS], f32, kind="ExternalOutput")

    with tile.TileContext(nc) as tc, ExitStack() as ctx:
        const = ctx.enter_context(tc.tile_pool(name="const", bufs=1))
        dxbp = ctx.enter_context(tc.tile_pool(name="dxbp", bufs=2))
        work = ctx.enter_context(tc.tile_pool(name="work", bufs=2))
        state = ctx.enter_context(tc.tile_pool(name="state", bufs=1))
        psum = ctx.enter_context(tc.tile_pool(name="psum", bufs=1, space="PSUM"))

        ct = {}
        for name, shp in wshapes.items():
            dt_ = mmdt if name in mm_names else f32
            ct[name] = const.tile(list(shp), dt_, tag=name, name=f"c_{name}")
            nc.sync.dma_start(ct[name][:], dram[name][:])
        x0_t = const.tile([8, BS], f32, tag="x0")
        nc.sync.dma_start(x0_t[:], dram["x0"][:])
        dxbc_t = const.tile([128, P * 2 * BS], f32, tag="dxbc")
        nc.sync.dma_start(dxbc_t[:], dram["dxbc"][:])

        # ---- PSUM tiles: 8 banks, chains A and B ----
        ps = {
            k: {t: psum.tile([128, W], f32, tag=t + k, name=t + k)
                for t in ("p1", "p2", "p3a", "p3b")}
            for k in ("A", "B")
        }
        # coarse-phase aliases (slices of bank tiles, used when fine is idle)
        psc = ps["A"]
        psum_yc = ps["B"]["p1"][0:32, 0:BS]      # init-sweep y tracker
        psl = ps["B"]["p2"][0:NCLS, 0:BS]        # readout

        # ---- persistent state ----
        Upk = state.tile([32, P * BS], f32, tag="Upk")      # U^0 slab starts
        G_old = state.tile([32, P * BS], f32, tag="G_old")  # G(U^0_s)
        F_sb = state.tile([32, P * BS], f32, tag="F_sb")    # F(U^0_s)
        Dpk = state.tile([32, P * BS], f32, tag="Dpk")      # F - G_old

        MM = dict(skip_group_check=True)

        def cs(t, s, n=1):
            return t[:, s * BS:(s + n) * BS]

        def eval_body(pp, w, tagsfx, dxb_ap, last, sel_into=None):
            """One Euler eval on chain tiles `pp` at width w.

            Reads pp.p1 (= F0 @ y + f0-less bias handled via ACT bias port),
            produces g and accumulates pp.p1 += ATt @ g (exact Euler step in
            F0-space).  If sel_into is given, also accumulates Sel @ g into
            it (y tracking for the init sweep)."""
            e1 = work.tile([128, w], f32, tag="e1" + tagsfx)
            nc.scalar.activation(e1[:], pp["p1"][:, 0:w], AF.Exp, bias=ct["f0c"][:])
            h1 = work.tile([128, w], mmdt, tag="h1" + tagsfx)
            nc.scalar.activation(h1[:], e1[:], AF.Ln, bias=1.0)
            nc.tensor.matmul(pp["p2"][:, 0:w], ct["F1T"][:], h1[:],
                             start=True, stop=True, **MM)
            e2 = work.tile([128, w], f32, tag="e2" + tagsfx)
            nc.scalar.activation(e2[:], pp["p2"][:, 0:w], AF.Exp, bias=ct["f1c"][:])
            h2 = work.tile([128, w], mmdt, tag="h2" + tagsfx)
            nc.scalar.activation(h2[:], e2[:], AF.Ln, bias=1.0)
            nc.tensor.matmul(pp["p3a"][:, 0:w], ct["F2aT"][:], h2[:],
                             start=True, stop=True, **MM)
            nc.tensor.matmul(pp["p3b"][:, 0:w], ct["F2bT"][:], h2[:],
                             start=True, stop=True, **MM)
            u = work.tile([128, 2 * w], f32, tag="u" + tagsfx)
            nc.scalar.activation(u[:, 0:w], pp["p3a"][:, 0:w], AF.Exp,
                                 scale=-2.0, bias=ct["f2nA"][:])
            nc.scalar.activation(u[:, w:2 * w], pp["p3b"][:, 0:w], AF.Exp,
                                 scale=-2.0, bias=ct["f2nB"][:])
            # tanh on DVE:  r = 1/(1+u),  g = (r - 1/2) * (2 dX dt)
            wv = work.tile([128, 2 * w], f32, tag="w" + tagsfx)
            nc.vector.tensor_scalar(wv[:], u[:], 1.0, 1.0e30, OP.add, OP.min)
            r = work.tile([128, 2 * w], f32, tag="r" + tagsfx)
            nc.vector.reciprocal_approx_fast(r[:], wv[:])
            g = work.tile([128, 2 * w], mmdt, tag="g" + tagsfx)
            if w >= 128:
                nc.vector.scalar_tensor_tensor(g[:, 0:w], r[:, 0:w], -0.5,
                                               dxb_ap[:, 0:w], OP.add, OP.mult)
                nc.tensor.matmul(pp["p1"][:, 0:w], ct["ATt"][:], g[:, 0:w],
                                 start=False, stop=False, **MM)
                nc.vector.scalar_tensor_tensor(g[:, w:2 * w], r[:, w:2 * w],
                                               -0.5, dxb_ap[:, w:2 * w],
                                               OP.add, OP.mult)
                nc.tensor.matmul(pp["p1"][:, 0:w], ct["ATt"][:], g[:, w:2 * w],
                                 start=False, stop=last, **MM)
            else:
                nc.vector.scalar_tensor_tensor(g[:], r[:], -0.5, dxb_ap,
                                               OP.add, OP.mult)
                nc.tensor.matmul(pp["p1"][:, 0:w], ct["ATt"][:], g[:, 0:w],
                                 start=False, stop=False, **MM)
                nc.tensor.matmul(pp["p1"][:, 0:w], ct["ATt"][:], g[:, w:2 * w],
                                 start=False, stop=last, **MM)
            if sel_into is not None:
                nc.tensor.matmul(sel_into, ct["Sel"][:], g[:, 0:w],
                                 start=False, stop=False, **MM)
                nc.tensor.matmul(sel_into, ct["Sel"][:], g[:, w:2 * w],
                                 start=False, stop=last, **MM)

        def softplus32(ps_in, bias_ap, out_tile, tagsfx):
            e = work.tile([128, BS], f32, tag="esp" + tagsfx)
            nc.scalar.activation(e[:], ps_in, AF.Exp, bias=bias_ap)
            nc.scalar.activation(out_tile[:], e[:], AF.Ln, bias=1.0)

        # ================= initial MLP -> y0 = Upk[:, 0] =================
        nc.tensor.matmul(psc["p1"][:, 0:BS], ct["W0T"][:], x0_t[:],
                         start=True, stop=True, **MM)
        hA = work.tile([128, BS], f32, tag="hA")
        softplus32(psc["p1"][:, 0:BS], ct["b0c"][:], hA, "A0")
        nc.tensor.matmul(psc["p2"][:, 0:BS], ct["W1T"][:], hA[:],
                         start=True, stop=True, **MM)
        hB = work.tile([128, BS], f32, tag="hB")
        softplus32(psc["p2"][:, 0:BS], ct["b1c"][:], hB, "B0")
        nc.tensor.matmul(psc["p3a"][0:32, 0:BS], ct["W2T"][:], hB[:],
                         start=True, stop=True, **MM)
        nc.scalar.activation(cs(Upk, 0), psc["p3a"][0:32, 0:BS], AF.Identity,
                             bias=ct["b2c"][:])

        # ================= init coarse sweep (rolling psum) =================
        # psum1 rolls F0 @ U^0_s; psum_yc rolls U^0_s via Sel accumulation.
        nc.tensor.matmul(psc["p1"][:, 0:BS], ct["F0T32"][:], cs(Upk, 0),
                         start=True, stop=False, **MM)
        nc.tensor.matmul(psum_yc, ct["Id32"][:], cs(Upk, 0),
                         start=True, stop=False, **MM)
        for s in range(P):
            eval_body(psc, BS, "c", dxbc_t[:, s * 2 * BS:(s + 1) * 2 * BS],
                      s == P - 1, sel_into=psum_yc)
            # off-chain: extract U^0_{s+1} = psum_yc -> G_old[s] (+ Upk[s+1])
            nc.scalar.activation(cs(G_old, s), psum_yc, AF.Identity)
            if s < P - 1:
                nc.vector.tensor_copy(cs(Upk, s + 1), cs(G_old, s))

        # ================= fine sweep: two interleaved chains =================
        nc.tensor.matmul(ps["A"]["p1"][:, 0:W], ct["F0T32"][:], Upk[:, 0:W],
                         start=True, stop=False, **MM)
        nc.tensor.matmul(ps["B"]["p1"][:, 0:W], ct["F0T32"][:],
                         Upk[:, W:2 * W], start=True, stop=False, **MM)
        for ch in range(NCH):
            dxbA = dxbp.tile([128, C * 2 * W], f32, tag="dxbfA")
            nc.sync.dma_start(dxbA[:], dram["dxbfA"][ch])
            dxbB = dxbp.tile([128, C * 2 * W], f32, tag="dxbfB")
            nc.sync.dma_start(dxbB[:], dram["dxbfB"][ch])
            for c in range(C):
                i = ch * C + c
                last = i == NS - 1
                eval_body(ps["A"], W, "fA",
                          dxbA[:, c * 2 * W:(c + 1) * 2 * W], last)
                eval_body(ps["B"], W, "fB",
                          dxbB[:, c * 2 * W:(c + 1) * 2 * W], last)
        # slab endpoints: F = pinv(F0) @ psum1  (psum1 holds F0 @ y exactly)
        for key, lo in (("A", 0), ("B", W)):
            p1sb = work.tile([128, W], f32, tag="p1sb" + key)
            nc.scalar.activation(p1sb[:], ps[key]["p1"][:, 0:W], AF.Identity)
            nc.tensor.matmul(ps[key]["p3a"][0:32, 0:W], ct["PinvT"][:],
                             p1sb[:], start=True, stop=True, **MM)
            nc.scalar.activation(F_sb[:, lo:lo + W],
                                 ps[key]["p3a"][0:32, 0:W], AF.Identity)

        # ================= correction sweep (rolling psum, packed defect) ====
        nc.vector.tensor_tensor(Dpk[:], F_sb[:], G_old[:], OP.subtract)
        nc.tensor.matmul(psc["p1"][:, 0:BS], ct["F0T32"][:], cs(Upk, 0),
                         start=True, stop=False, **MM)
        for s in range(P):
            eval_body(psc, BS, "c", dxbc_t[:, s * 2 * BS:(s + 1) * 2 * BS],
                      s == P - 1)
            # psum1 += F0 @ D_s   (parareal correction, input-only)
            nc.tensor.matmul(psc["p1"][:, 0:BS], ct["F0T32"][:], cs(Dpk, s),
                             start=False, stop=(s == P - 1), **MM)

        # ================= readout: logits = R @ pinv(F0) @ psum1 + rb =======
        p1f = work.tile([128, BS], f32, tag="p1f")
        nc.scalar.activation(p1f[:], psc["p1"][:, 0:BS], AF.Identity)
        nc.tensor.matmul(psl, ct["RPinvT"][:], p1f[:], start=True, stop=True,
                         **MM)
        out_sb = work.tile([NCLS, BS], f32, tag="out_sb")
        nc.scalar.activation(out_sb[:], psl, AF.Identity, bias=ct["rbc"][:])
        nc.sync.dma_start(out_dram[:], out_sb[:])

    nc.compile()
    _NC_CACHE[key] = nc
    return nc


# --------------------------------------------------------------------------
# Public entry point
# --------------------------------------------------------------------------

def _prepare_inputs(ts, coeff_d, coeff_c, coeff_b, coeff_a,
                    W0, b0, W1, b1, W2, b2, F0, f0, F1, f1, F2, f2, R, rb):
    P, C = P_SLABS, CHUNK
    NCH = NS // C
    ts = np.asarray(ts, dtype=_F32)
    coeff_a = np.asarray(coeff_a, _F32)
    cd, cc, cb = (np.asarray(a, _F32) for a in (coeff_d, coeff_c, coeff_b))

    # fine-step times (exactly the reference's grid)
    t_fine = (ts[0] + _F32(DT0) * np.arange(NUM_STEPS, dtype=_F32)).astype(_F32)
    dts = np.minimum(_F32(DT0), ts[-1] - t_fine).astype(_F32)
    dx_fine = _dx_eval(ts, cd, cc, cb, t_fine)            # (2000, B, D)
    dx_fine = dx_fine * (2.0 * dts[:, None, None])

    # coarse increments: EXACT integrals of the piecewise-quadratic dX over
    # each slab (the control increment the reference's Euler steps are a
    # Riemann sum of).
    slab_len = T1 / P
    knot_d = np.diff(ts).astype(np.float64)
    cbd, ccd, cdd = (a.astype(np.float64) for a in (cb, cc, cd))
    Tk = (cbd * knot_d[None, :, None] + ccd * knot_d[None, :, None] ** 2
          + cdd * knot_d[None, :, None] ** 3)             # (B, 127, D)
    pref = np.concatenate([np.zeros((B, 1, D)), np.cumsum(Tk, axis=1)], axis=1)

    def antider(t):
        idx = int(np.clip(np.searchsorted(ts, t, side="right") - 1, 0, NP_KNOTS - 2))
        u = float(t) - float(ts[idx])
        return pref[:, idx] + cbd[:, idx] * u + ccd[:, idx] * u * u + cdd[:, idx] * u ** 3

    bounds = [s * slab_len for s in range(P)] + [T1]
    A = np.stack([antider(t) for t in bounds])            # (P+1, B, D)
    dx_coarse = (A[1:] - A[:-1]).astype(_F32) * _F32(2.0)  # (P, B, D)

    Wt = _host_weights(*[np.asarray(a, _F32) for a in
                         (W0, b0, W1, b1, W2, b2, F0, f0, F1, f1, F2, f2, R, rb)])
    in_maps = []
    for core in range(NCORES):
        bsl = slice(core * BS, (core + 1) * BS)
        mmap = dict(Wt)
        mmap["x0"] = np.ascontiguousarray(coeff_a[bsl, 0, :].T)
        # fine dxb per chain: [i, (slab, j), d] packed layout
        dxc = dx_fine[:, bsl, :]                          # (2000, 16, 8)
        dxp = dxc.reshape(P, NS, BS, D).transpose(1, 0, 2, 3)  # (NS, P, BS, D)
        for key, g0 in (("A", 0), ("B", NG)):
            grp = dxp[:, g0:g0 + NG].reshape(NS, W, D)
            arr = _pack_layout(grp, W)                    # (NS, 128, 2W)
            arr = arr.reshape(NCH, C, 128, 2 * W).transpose(0, 2, 1, 3)
            mmap["dxbf" + key] = np.ascontiguousarray(
                arr.reshape(NCH, 128, C * 2 * W))
        arrc = _pack_layout(dx_coarse[:, bsl, :], BS)     # (P, 128, 32)
        mmap["dxbc"] = np.ascontiguousarray(
            arrc.transpose(1, 0, 2).reshape(128, P * 2 * BS))
        in_maps.append(mmap)
    return in_maps


def kernel(ts, coeff_d, coeff_c, coeff_b, coeff_a,
           W0, b0, W1, b1, W2, b2, F0, f0, F1, f1, F2, f2, R, rb):
    from concourse.bass_utils import run_bass_kernel_spmd

    nc = _build_nc()
    in_maps = _prepare_inputs(ts, coeff_d, coeff_c, coeff_b, coeff_a,
                              W0, b0, W1, b1, W2, b2,
                              F0, f0, F1, f1, F2, f2, R, rb)
    res = run_bass_kernel_spmd(nc, in_maps, list(range(NCORES)))
    logits = np.concatenate(
        [res.results[i]["logits"].T for i in range(NCORES)], axis=0)
    return np.ascontiguousarray(logits.astype(np.float32))


# revision 19
# speedup vs baseline: 8.1311x; 1.0110x over previous
"""Trainium2 Bass kernel for a Neural CDE forward pass — parareal edition.

Model (see reference): 2000 fixed Euler steps (h=0.01) of
    y_{t+1} = y_t + dt * einsum('bhd,bd->bh', tanh-MLP(y_t).reshape(B,H,D), dX_t)
with a 3-layer softplus MLP (32 -> 128 -> 128 -> 256/tanh), batch B=128,
followed by a linear readout.

The reference trajectory is sensitive (its own discretization error vs the
true flow is ~1e-2), so the only way to match it within 2e-2 is to reproduce
its exact discrete Euler map.  The serial step chain is latency-bound
(~200-700ns per instruction, nearly width-independent up to ~400 cols).

Strategy:
  * Pure data parallel over 8 NeuronCores (16 batch elements per core).
  * PARAREAL over time inside each core: 50 slabs of 40 fine steps.  The
    fine propagator (exact Euler, h=0.01) advances slabs 0-24 and 25-49 as
    TWO independent packed chains (width 400 each) whose instructions
    interleave on the engines — 40 serial chain-steps cover all 2000.
  * Coarse propagator G: one Euler step per slab using the EXACT integral
    of dX over the slab (host-computed piecewise-quadratic antiderivative).
    G tracks F so well that ONE parareal iteration
        U_{s+1} <- G(U_s) + F(U_s_prev) - G(U_s_prev)
    reaches ~3e-5 rel err in float64 simulation.
  * Both coarse sweeps roll the state in PSUM (psum1 = F0 @ y, updated via
    psum1 += tile(F0.T) @ g); y is never materialized on the serial chain.
    The init sweep extracts slab states off-chain via a Sel accumulator;
    the correction sweep injects the packed parareal defect D = F - G_old
    with one K=32 matmul per slab and needs no extraction at all — the
    readout is (R @ pinv(F0)) @ psum1 + rb.
  * Feature-major layout: features on partitions, (slab, batch) on the free
    dim; every layer is one PE matmul with a constant fp16 lhsT.
  * softplus(x) = Ln(Exp(x)+1): two ACT ops (natural_log_exp table, pinned
    so the chooser never inserts ACT_TABLE_LOADs).
  * tanh section on DVE: u = exp(-2z-2f2) (ACT, f2 bias on the bias port),
    w = min(1+u, 1e30), r ~ 1/w, g = (r - 1/2) * (2 dX dt) — keeps ACT
    below saturation when two fine chains interleave.
"""

import numpy as np

B = 128
NP_KNOTS = 128
D = 8
H = 32
WID = 128
NCLS = 10
T0, T1 = 0.0, 20.0
DT0 = 0.01
NUM_STEPS = 2000
NCORES = 8
BS = B // NCORES          # 16 batch per core

# parareal configuration
P_SLABS = 50              # time slabs per core (two packed chains of 25)
NG = P_SLABS // 2         # slabs per chain
NS = NUM_STEPS // P_SLABS  # fine steps per slab (40)
CHUNK = 5                 # fine steps per DMA chunk
W = NG * BS               # packed width per chain (400)

_F32 = np.float32
MM_DT = np.float16


# --------------------------------------------------------------------------
# Host-side precompute
# --------------------------------------------------------------------------

def _dx_eval(ts, coeff_d, coeff_c, coeff_b, t_grid):
    """Spline derivative dX/dt at the given times.  Returns (T, B, D)."""
    idx = np.clip(np.searchsorted(ts, t_grid, side="right") - 1, 0, NP_KNOTS - 2)
    fr = (t_grid - ts[idx]).astype(_F32)[None, :, None]
    dX = (coeff_b[:, idx] + _F32(2.0) * coeff_c[:, idx] * fr
          + _F32(3.0) * coeff_d[:, idx] * fr * fr)          # (B, T, D)
    return np.transpose(dX, (1, 0, 2)).astype(_F32)         # (T, B, D)


def _pack_layout(dx_scaled, cols):
    """(T, cols, D) -> (T, 128, 2*cols) d-major, h-broadcast layout.

    Partition p in col-half cb holds (d = cb*4 + p//32, h = p%32)."""
    T = dx_scaled.shape[0]
    tmp = dx_scaled.reshape(T, cols, 2, 4)          # [t, j, cb, dblk]
    tmp = np.transpose(tmp, (0, 3, 2, 1))           # [t, dblk, cb, j]
    tmp = np.broadcast_to(tmp[:, :, None, :, :], (T, 4, 32, 2, cols))
    return np.ascontiguousarray(tmp.reshape(T, 128, 2 * cols))


def _host_weights(W0, b0, W1, b1, W2, b2, F0, f0, F1, f1, F2, f2, R, rb):
    f32 = lambda a: np.ascontiguousarray(a, dtype=_F32)
    f16 = lambda a: np.ascontiguousarray(a, dtype=MM_DT)
    p = np.arange(256)
    perm = (p % 32) * 8 + p // 32
    F2p = F2[perm]
    f2p = f2[perm]
    pinv = np.linalg.pinv(F0.astype(np.float64))
    return {
        "ATt":   f16(np.tile(F0.T, (4, 1))),          # (128,128)
        "F1T":   f16(F1.T),
        "F2aT":  f16(F2p[:128].T),
        "F2bT":  f16(F2p[128:].T),
        "Sel":   f16(np.tile(np.eye(32, dtype=_F32), (4, 1))),  # (128,32)
        "F0T32": f32(F0.T),                            # (32,128) fp32 lhsT
        "PinvT": f32(pinv.T.astype(np.float32)),       # (128,32)
        "PinvT16": f16(pinv.T.astype(np.float32)),     # (128,32) per-slab use
        "RPinvT": f32((R.astype(np.float64) @ pinv).T.astype(np.float32)),  # (128,10)
        "Id32":  f32(np.eye(32, dtype=_F32)),          # (32,32)
        "W0T":   f32(W0.T), "W1T": f32(W1.T), "W2T": f32(W2.T),
        "b0c":   f32(b0[:, None]), "b1c": f32(b1[:, None]),
        "b2c":   f32(b2[:, None]),
        "f0c":   f32(f0[:, None]), "f1c": f32(f1[:, None]),
        "f2nA":  f32(-2.0 * f2p[:128, None]),
        "f2nB":  f32(-2.0 * f2p[128:, None]),
        "rbc":   f32(rb[:, None]),
    }


# --------------------------------------------------------------------------
# Bass kernel build
# --------------------------------------------------------------------------

_NC_CACHE = {}


def _build_nc():
    key = (P_SLABS, CHUNK)
    if key in _NC_CACHE:
        return _NC_CACHE[key]

    import concourse.bacc as bacc
    import concourse.mybir as mybir
    import concourse.tile as tile
    from contextlib import ExitStack

    f32 = mybir.dt.float32
    mmdt = mybir.dt.from_np(np.dtype(MM_DT))
    AF = mybir.ActivationFunctionType
    OP = mybir.AluOpType

    import concourse.hw_specs as hw_specs
    _full_tabs = hw_specs.get_activation_tables("gen3")
    _ours = {AF.Exp, AF.Ln, AF.Identity, AF.Copy}
    _pinned = {
        name: (set(funcs) if name == "natural_log_exp_and_others"
               else set(funcs) - _ours)
        for name, funcs in _full_tabs.items()
    }
    bacc.get_activation_tables = lambda arch: _pinned

    P, C = P_SLABS, CHUNK
    NCH = NS // C
    assert NS % C == 0

    nc = bacc.Bacc("TRN2", target_bir_lowering=False, debug=False)

    # ---- DRAM I/O ----
    wshapes = {
        "ATt": (128, 128), "F1T": (128, 128), "F2aT": (128, 128),
        "F2bT": (128, 128), "Sel": (128, 32),
        "F0T32": (32, 128), "Id32": (32, 32), "PinvT": (128, 32),
        "PinvT16": (128, 32), "RPinvT": (128, 10),
        "W0T": (8, 128), "W1T": (128, 128), "W2T": (128, 32),
        "b0c": (128, 1), "b1c": (128, 1), "b2c": (32, 1),
        "f0c": (128, 1), "f1c": (128, 1),
        "f2nA": (128, 1), "f2nB": (128, 1), "rbc": (10, 1),
    }
    mm_names = {"ATt", "F1T", "F2aT", "F2bT", "Sel", "PinvT16"}
    dram = {}
    for name, shp in wshapes.items():
        dt_ = mmdt if name in mm_names else f32
        dram[name] = nc.dram_tensor(name, list(shp), dt_, kind="ExternalInput")
    dram["x0"] = nc.dram_tensor("x0", [8, BS], f32, kind="ExternalInput")
    dram["dxbfA"] = nc.dram_tensor("dxbfA", [NCH, 128, C * 2 * W], f32,
                                   kind="ExternalInput")
    dram["dxbfB"] = nc.dram_tensor("dxbfB", [NCH, 128, C * 2 * W], f32,
                                   kind="ExternalInput")
    dram["dxbc"] = nc.dram_tensor("dxbc", [128, P * 2 * BS], f32,
                                  kind="ExternalInput")
    out_dram = nc.dram_tensor("logits", [NCLS, BS], f32, kind="ExternalOutput")

    with tile.TileContext(nc) as tc, ExitStack() as ctx:
        const = ctx.enter_context(tc.tile_pool(name="const", bufs=1))
        dxbp = ctx.enter_context(tc.tile_pool(name="dxbp", bufs=2))
        work = ctx.enter_context(tc.tile_pool(name="work", bufs=2))
        state = ctx.enter_context(tc.tile_pool(name="state", bufs=1))
        psum = ctx.enter_context(tc.tile_pool(name="psum", bufs=1, space="PSUM"))

        ct = {}
        for name, shp in wshapes.items():
            dt_ = mmdt if name in mm_names else f32
            ct[name] = const.tile(list(shp), dt_, tag=name, name=f"c_{name}")
            nc.sync.dma_start(ct[name][:], dram[name][:])
        x0_t = const.tile([8, BS], f32, tag="x0")
        nc.sync.dma_start(x0_t[:], dram["x0"][:])
        dxbc_t = const.tile([128, P * 2 * BS], f32, tag="dxbc")
        nc.sync.dma_start(dxbc_t[:], dram["dxbc"][:])

        # ---- PSUM tiles: 8 banks, chains A and B ----
        ps = {
            k: {t: psum.tile([128, W], f32, tag=t + k, name=t + k)
                for t in ("p1", "p2", "p3a", "p3b")}
            for k in ("A", "B")
        }
        # coarse-phase aliases (slices of bank tiles, used when fine is idle).
        # The init sweep runs on chain-B's banks so it can overlap chain A's
        # first fine steps; the correction runs on chain-A's banks so it can
        # overlap chain B's last fine steps.  Slab states are extracted from
        # the rolling psum via short CLOSED-group pinv matmuls (never a
        # second long-open accumulation group on a busy bank).
        psi = ps["B"]
        psc = ps["A"]
        psl = ps["B"]["p2"][0:NCLS, 0:BS]        # readout

        # ---- persistent state ----
        Upk = state.tile([32, P * BS], f32, tag="Upk")      # U^0 slab starts
        G_old = state.tile([32, P * BS], f32, tag="G_old")  # G(U^0_s)
        F_sb = state.tile([32, P * BS], f32, tag="F_sb")    # F(U^0_s)
        Dpk = state.tile([32, P * BS], f32, tag="Dpk")      # F - G_old

        MM = dict(skip_group_check=True)

        def cs(t, s, n=1):
            return t[:, s * BS:(s + n) * BS]

        def eval_body(pp, w, tagsfx, dxb_ap, last, sel_into=None):
            """One Euler eval on chain tiles `pp` at width w.

            Reads pp.p1 (= F0 @ y + f0-less bias handled via ACT bias port),
            produces g and accumulates pp.p1 += ATt @ g (exact Euler step in
            F0-space).  If sel_into is given, also accumulates Sel @ g into
            it (y tracking for the init sweep)."""
            e1 = work.tile([128, w], f32, tag="e1" + tagsfx)
            nc.scalar.activation(e1[:], pp["p1"][:, 0:w], AF.Exp, bias=ct["f0c"][:])
            h1 = work.tile([128, w], mmdt, tag="h1" + tagsfx)
            nc.scalar.activation(h1[:], e1[:], AF.Ln, bias=1.0)
            nc.tensor.matmul(pp["p2"][:, 0:w], ct["F1T"][:], h1[:],
                             start=True, stop=True, **MM)
            e2 = work.tile([128, w], f32, tag="e2" + tagsfx)
            nc.scalar.activation(e2[:], pp["p2"][:, 0:w], AF.Exp, bias=ct["f1c"][:])
            h2 = work.tile([128, w], mmdt, tag="h2" + tagsfx)
            nc.scalar.activation(h2[:], e2[:], AF.Ln, bias=1.0)
            nc.tensor.matmul(pp["p3a"][:, 0:w], ct["F2aT"][:], h2[:],
                             start=True, stop=True, **MM)
            nc.tensor.matmul(pp["p3b"][:, 0:w], ct["F2bT"][:], h2[:],
                             start=True, stop=True, **MM)
            u = work.tile([128, 2 * w], f32, tag="u" + tagsfx)
            nc.scalar.activation(u[:, 0:w], pp["p3a"][:, 0:w], AF.Exp,
                                 scale=-2.0, bias=ct["f2nA"][:])
            nc.scalar.activation(u[:, w:2 * w], pp["p3b"][:, 0:w], AF.Exp,
                                 scale=-2.0, bias=ct["f2nB"][:])
            # tanh on DVE:  r = 1/(1+u),  g = (r - 1/2) * (2 dX dt)
            wv = work.tile([128, 2 * w], f32, tag="w" + tagsfx)
            nc.vector.tensor_scalar(wv[:], u[:], 1.0, 1.0e30, OP.add, OP.min)
            r = work.tile([128, 2 * w], f32, tag="r" + tagsfx)
            nc.vector.reciprocal_approx_fast(r[:], wv[:])
            g = work.tile([128, 2 * w], mmdt, tag="g" + tagsfx)
            if w >= 128:
                nc.vector.scalar_tensor_tensor(g[:, 0:w], r[:, 0:w], -0.5,
                                               dxb_ap[:, 0:w], OP.add, OP.mult)
                nc.tensor.matmul(pp["p1"][:, 0:w], ct["ATt"][:], g[:, 0:w],
                                 start=False, stop=False, **MM)
                nc.vector.scalar_tensor_tensor(g[:, w:2 * w], r[:, w:2 * w],
                                               -0.5, dxb_ap[:, w:2 * w],
                                               OP.add, OP.mult)
                nc.tensor.matmul(pp["p1"][:, 0:w], ct["ATt"][:], g[:, w:2 * w],
                                 start=False, stop=last, **MM)
            else:
                nc.vector.scalar_tensor_tensor(g[:], r[:], -0.5, dxb_ap,
                                               OP.add, OP.mult)
                nc.tensor.matmul(pp["p1"][:, 0:w], ct["ATt"][:], g[:, 0:w],
                                 start=False, stop=False, **MM)
                nc.tensor.matmul(pp["p1"][:, 0:w], ct["ATt"][:], g[:, w:2 * w],
                                 start=False, stop=last, **MM)
            if sel_into is not None:
                nc.tensor.matmul(sel_into, ct["Sel"][:], g[:, 0:w],
                                 start=False, stop=False, **MM)
                nc.tensor.matmul(sel_into, ct["Sel"][:], g[:, w:2 * w],
                                 start=False, stop=last, **MM)

        def softplus32(ps_in, bias_ap, out_tile, tagsfx):
            e = work.tile([128, BS], f32, tag="esp" + tagsfx)
            nc.scalar.activation(e[:], ps_in, AF.Exp, bias=bias_ap)
            nc.scalar.activation(out_tile[:], e[:], AF.Ln, bias=1.0)

        # ================= initial MLP -> y0 = Upk[:, 0] =================
        nc.tensor.matmul(psc["p1"][:, 0:BS], ct["W0T"][:], x0_t[:],
                         start=True, stop=True, **MM)
        hA = work.tile([128, BS], f32, tag="hA")
        softplus32(psc["p1"][:, 0:BS], ct["b0c"][:], hA, "A0")
        nc.tensor.matmul(psc["p2"][:, 0:BS], ct["W1T"][:], hA[:],
                         start=True, stop=True, **MM)
        hB = work.tile([128, BS], f32, tag="hB")
        softplus32(psc["p2"][:, 0:BS], ct["b1c"][:], hB, "B0")
        nc.tensor.matmul(psc["p3a"][0:32, 0:BS], ct["W2T"][:], hB[:],
                         start=True, stop=True, **MM)
        nc.scalar.activation(cs(Upk, 0), psc["p3a"][0:32, 0:BS], AF.Identity,
                             bias=ct["b2c"][:])

        # ======= staggered schedule =======
        # init slabs 0..24 (serial) -> chain A fine steps 0..9 overlapping
        # init slabs 25..49 -> paired fine steps (A: 10..39, B: 0..29) ->
        # A endpoints + correction slabs 0..24 overlapping B fine 30..39 ->
        # B endpoints -> correction slabs 25..49 -> readout.
        LAG = 10
        spread = [3, 2, 3, 2, 3, 2, 3, 2, 3, 2]
        dxb_cur = {}

        def get_dxb(key, i):
            """chunked dxb stream per chain; DMA on chunk boundary."""
            if i % C == 0:
                t = dxbp.tile([128, C * 2 * W], f32, tag="dxbf" + key,
                              name="dxbf" + key)
                nc.sync.dma_start(t[:], dram["dxbf" + key][i // C])
                dxb_cur[key] = t
            c = i % C
            return dxb_cur[key][:, c * 2 * W:(c + 1) * 2 * W]

        def init_slab(s):
            """one init coarse slab on chain-B banks; rolling p1B; slab
            state extracted via a closed-group pinv matmul into p3aB."""
            eval_body(psi, BS, "c", dxbc_t[:, s * 2 * BS:(s + 1) * 2 * BS],
                      s == P - 1)
            y128 = work.tile([128, BS], mmdt, tag="y128", name="y128")
            nc.scalar.activation(y128[:], psi["p1"][:, 0:BS], AF.Identity)
            nc.tensor.matmul(psi["p3a"][0:32, 0:BS], ct["PinvT16"][:],
                             y128[:], start=True, stop=True, **MM)
            nc.scalar.activation(cs(G_old, s), psi["p3a"][0:32, 0:BS],
                                 AF.Identity)
            if s < P - 1:
                nc.vector.tensor_copy(cs(Upk, s + 1), cs(G_old, s))

        def corr_slab(s):
            eval_body(psc, BS, "cr", dxbc_t[:, s * 2 * BS:(s + 1) * 2 * BS],
                      False)
            # psum1 += F0 @ D_s   (parareal correction, input-only)
            nc.tensor.matmul(psc["p1"][:, 0:BS], ct["F0T32"][:], cs(Dpk, s),
                             start=False, stop=(s == P - 1), **MM)

        def extract_F(key, lo):
            p1sb = work.tile([128, W], f32, tag="p1sb" + key,
                             name="p1sb" + key)
            nc.scalar.activation(p1sb[:], ps[key]["p1"][:, 0:W], AF.Identity)
            nc.tensor.matmul(ps[key]["p3a"][0:32, 0:W], ct["PinvT"][:],
                             p1sb[:], start=True, stop=True, **MM)
            nc.scalar.activation(F_sb[:, lo:lo + W],
                                 ps[key]["p3a"][0:32, 0:W], AF.Identity)

        # ---- init sweep start (rolling on chain-B banks) ----
        nc.tensor.matmul(psi["p1"][:, 0:BS], ct["F0T32"][:], cs(Upk, 0),
                         start=True, stop=False, **MM)
        for s in range(P // 2):
            init_slab(s)

        # ---- chain A starts; init slabs 25..49 overlap A steps 0..9 ----
        nc.tensor.matmul(ps["A"]["p1"][:, 0:W], ct["F0T32"][:], Upk[:, 0:W],
                         start=True, stop=False, **MM)
        nxt_init = P // 2
        for k in range(LAG):
            eval_body(ps["A"], W, "fA", get_dxb("A", k), False)
            for _ in range(spread[k]):
                init_slab(nxt_init)
                nxt_init += 1

        # ---- chain B starts; paired steps ----
        nc.tensor.matmul(ps["B"]["p1"][:, 0:W], ct["F0T32"][:],
                         Upk[:, W:2 * W], start=True, stop=False, **MM)
        for i in range(LAG, NS):
            eval_body(ps["A"], W, "fA", get_dxb("A", i), i == NS - 1)
            eval_body(ps["B"], W, "fB", get_dxb("B", i - LAG), False)

        # ---- A endpoints; correction slabs 0..24 overlap B steps 30..39 ----
        extract_F("A", 0)
        nc.vector.tensor_tensor(Dpk[:, 0:W], F_sb[:, 0:W], G_old[:, 0:W],
                                OP.subtract)
        nc.tensor.matmul(psc["p1"][:, 0:BS], ct["F0T32"][:], cs(Upk, 0),
                         start=True, stop=False, **MM)
        nxt_corr = 0
        for k in range(LAG):
            eval_body(ps["B"], W, "fB", get_dxb("B", NS - LAG + k),
                      k == LAG - 1)
            for _ in range(spread[k]):
                corr_slab(nxt_corr)
                nxt_corr += 1

        # ---- B endpoints; correction slabs 25..49 ----
        extract_F("B", W)
        nc.vector.tensor_tensor(Dpk[:, W:2 * W], F_sb[:, W:2 * W],
                                G_old[:, W:2 * W], OP.subtract)
        for s in range(P // 2, P):
            corr_slab(s)

        # ================= readout: logits = R @ pinv(F0) @ psum1 + rb =======
        p1f = work.tile([128, BS], f32, tag="p1f")
        nc.scalar.activation(p1f[:], psc["p1"][:, 0:BS], AF.Identity)
        nc.tensor.matmul(psl, ct["RPinvT"][:], p1f[:], start=True, stop=True,
                         **MM)
        out_sb = work.tile([NCLS, BS], f32, tag="out_sb")
        nc.scalar.activation(out_sb[:], psl, AF.Identity, bias=ct["rbc"][:])
        nc.sync.dma_start(out_dram[:], out_sb[:])

    nc.compile()
    _NC_CACHE[key] = nc
    return nc


# --------------------------------------------------------------------------
# Public entry point
# --------------------------------------------------------------------------

def _prepare_inputs(ts, coeff_d, coeff_c, coeff_b, coeff_a,
                    W0, b0, W1, b1, W2, b2, F0, f0, F1, f1, F2, f2, R, rb):
    P, C = P_SLABS, CHUNK
    NCH = NS // C
    ts = np.asarray(ts, dtype=_F32)
    coeff_a = np.asarray(coeff_a, _F32)
    cd, cc, cb = (np.asarray(a, _F32) for a in (coeff_d, coeff_c, coeff_b))

    # fine-step times (exactly the reference's grid)
    t_fine = (ts[0] + _F32(DT0) * np.arange(NUM_STEPS, dtype=_F32)).astype(_F32)
    dts = np.minimum(_F32(DT0), ts[-1] - t_fine).astype(_F32)
    dx_fine = _dx_eval(ts, cd, cc, cb, t_fine)            # (2000, B, D)
    dx_fine = dx_fine * (2.0 * dts[:, None, None])

    # coarse increments: EXACT integrals of the piecewise-quadratic dX over
    # each slab (the control increment the reference's Euler steps are a
    # Riemann sum of).
    slab_len = T1 / P
    knot_d = np.diff(ts).astype(np.float64)
    cbd, ccd, cdd = (a.astype(np.float64) for a in (cb, cc, cd))
    Tk = (cbd * knot_d[None, :, None] + ccd * knot_d[None, :, None] ** 2
          + cdd * knot_d[None, :, None] ** 3)             # (B, 127, D)
    pref = np.concatenate([np.zeros((B, 1, D)), np.cumsum(Tk, axis=1)], axis=1)

    def antider(t):
        idx = int(np.clip(np.searchsorted(ts, t, side="right") - 1, 0, NP_KNOTS - 2))
        u = float(t) - float(ts[idx])
        return pref[:, idx] + cbd[:, idx] * u + ccd[:, idx] * u * u + cdd[:, idx] * u ** 3

    bounds = [s * slab_len for s in range(P)] + [T1]
    A = np.stack([antider(t) for t in bounds])            # (P+1, B, D)
    dx_coarse = (A[1:] - A[:-1]).astype(_F32) * _F32(2.0)  # (P, B, D)

    Wt = _host_weights(*[np.asarray(a, _F32) for a in
                         (W0, b0, W1, b1, W2, b2, F0, f0, F1, f1, F2, f2, R, rb)])
    in_maps = []
    for core in range(NCORES):
        bsl = slice(core * BS, (core + 1) * BS)
        mmap = dict(Wt)
        mmap["x0"] = np.ascontiguousarray(coeff_a[bsl, 0, :].T)
        # fine dxb per chain: [i, (slab, j), d] packed layout
        dxc = dx_fine[:, bsl, :]                          # (2000, 16, 8)
        dxp = dxc.reshape(P, NS, BS, D).transpose(1, 0, 2, 3)  # (NS, P, BS, D)
        for key, g0 in (("A", 0), ("B", NG)):
            grp = dxp[:, g0:g0 + NG].reshape(NS, W, D)
            arr = _pack_layout(grp, W)                    # (NS, 128, 2W)
            arr = arr.reshape(NCH, C, 128, 2 * W).transpose(0, 2, 1, 3)
            mmap["dxbf" + key] = np.ascontiguousarray(
                arr.reshape(NCH, 128, C * 2 * W))
        arrc = _pack_layout(dx_coarse[:, bsl, :], BS)     # (P, 128, 32)
        mmap["dxbc"] = np.ascontiguousarray(
            arrc.transpose(1, 0, 2).reshape(128, P * 2 * BS))
        in_maps.append(mmap)
    return in_maps


def kernel(ts, coeff_d, coeff_c, coeff_b, coeff_a,
           W0, b0, W1, b1, W2, b2, F0, f0, F1, f1, F2, f2, R, rb):
    from concourse.bass_utils import run_bass_kernel_spmd

    nc = _build_nc()
    in_maps = _prepare_inputs(ts, coeff_d, coeff_c, coeff_b, coeff_a,
                              W0, b0, W1, b1, W2, b2,
                              F0, f0, F1, f1, F2, f2, R, rb)
    res = run_bass_kernel_spmd(nc, in_maps, list(range(NCORES)))
    logits = np.concatenate(
        [res.results[i]["logits"].T for i in range(NCORES)], axis=0)
    return np.ascontiguousarray(logits.astype(np.float32))


# revision 20
# speedup vs baseline: 8.3231x; 1.0236x over previous
"""Trainium2 Bass kernel for a Neural CDE forward pass — parareal edition.

Model (see reference): 2000 fixed Euler steps (h=0.01) of
    y_{t+1} = y_t + dt * einsum('bhd,bd->bh', tanh-MLP(y_t).reshape(B,H,D), dX_t)
with a 3-layer softplus MLP (32 -> 128 -> 128 -> 256/tanh), batch B=128,
followed by a linear readout.

The reference trajectory is sensitive (its own discretization error vs the
true flow is ~1e-2), so the only way to match it within 2e-2 is to reproduce
its exact discrete Euler map.  The serial step chain is latency-bound
(~200-700ns per instruction, nearly width-independent up to ~400 cols).

Strategy:
  * Pure data parallel over 8 NeuronCores (16 batch elements per core).
  * PARAREAL over time inside each core: 50 slabs of 40 fine steps.  The
    fine propagator (exact Euler, h=0.01) advances slabs 0-24 and 25-49 as
    TWO independent packed chains (width 400 each) whose instructions
    interleave on the engines — 40 serial chain-steps cover all 2000.
  * Coarse propagator G: one Euler step per slab using the EXACT integral
    of dX over the slab (host-computed piecewise-quadratic antiderivative).
    G tracks F so well that ONE parareal iteration
        U_{s+1} <- G(U_s) + F(U_s_prev) - G(U_s_prev)
    reaches ~3e-5 rel err in float64 simulation.
  * Both coarse sweeps roll the state in PSUM (psum1 = F0 @ y, updated via
    psum1 += tile(F0.T) @ g); y is never materialized on the serial chain.
    The init sweep extracts slab states off-chain via a Sel accumulator;
    the correction sweep injects the packed parareal defect D = F - G_old
    with one K=32 matmul per slab and needs no extraction at all — the
    readout is (R @ pinv(F0)) @ psum1 + rb.
  * Feature-major layout: features on partitions, (slab, batch) on the free
    dim; every layer is one PE matmul with a constant fp16 lhsT.
  * softplus(x) = Ln(Exp(x)+1): two ACT ops (natural_log_exp table, pinned
    so the chooser never inserts ACT_TABLE_LOADs).
  * tanh section on DVE: u = exp(-2z-2f2) (ACT, f2 bias on the bias port),
    w = min(1+u, 1e30), r ~ 1/w, g = (r - 1/2) * (2 dX dt) — keeps ACT
    below saturation when two fine chains interleave.
"""

import numpy as np

B = 128
NP_KNOTS = 128
D = 8
H = 32
WID = 128
NCLS = 10
T0, T1 = 0.0, 20.0
DT0 = 0.01
NUM_STEPS = 2000
NCORES = 8
BS = B // NCORES          # 16 batch per core

# parareal configuration
P_SLABS = 50              # time slabs per core (two packed chains of 25)
NG = P_SLABS // 2         # slabs per chain
NS = NUM_STEPS // P_SLABS  # fine steps per slab (40)
CHUNK = 5                 # fine steps per DMA chunk
W = NG * BS               # packed width per chain (400)

_F32 = np.float32
MM_DT = np.float16


# --------------------------------------------------------------------------
# Host-side precompute
# --------------------------------------------------------------------------

def _dx_eval(ts, coeff_d, coeff_c, coeff_b, t_grid):
    """Spline derivative dX/dt at the given times.  Returns (T, B, D)."""
    idx = np.clip(np.searchsorted(ts, t_grid, side="right") - 1, 0, NP_KNOTS - 2)
    fr = (t_grid - ts[idx]).astype(_F32)[None, :, None]
    dX = (coeff_b[:, idx] + _F32(2.0) * coeff_c[:, idx] * fr
          + _F32(3.0) * coeff_d[:, idx] * fr * fr)          # (B, T, D)
    return np.transpose(dX, (1, 0, 2)).astype(_F32)         # (T, B, D)


def _pack_layout(dx_scaled, cols):
    """(T, cols, D) -> (T, 128, 2*cols) d-major, h-broadcast layout.

    Partition p in col-half cb holds (d = cb*4 + p//32, h = p%32)."""
    T = dx_scaled.shape[0]
    tmp = dx_scaled.reshape(T, cols, 2, 4)          # [t, j, cb, dblk]
    tmp = np.transpose(tmp, (0, 3, 2, 1))           # [t, dblk, cb, j]
    tmp = np.broadcast_to(tmp[:, :, None, :, :], (T, 4, 32, 2, cols))
    return np.ascontiguousarray(tmp.reshape(T, 128, 2 * cols))


def _host_weights(W0, b0, W1, b1, W2, b2, F0, f0, F1, f1, F2, f2, R, rb):
    f32 = lambda a: np.ascontiguousarray(a, dtype=_F32)
    f16 = lambda a: np.ascontiguousarray(a, dtype=MM_DT)
    p = np.arange(256)
    perm = (p % 32) * 8 + p // 32
    F2p = F2[perm]
    f2p = f2[perm]
    pinv = np.linalg.pinv(F0.astype(np.float64))
    return {
        "ATt":   f16(np.tile(F0.T, (4, 1))),          # (128,128)
        "F1T":   f16(F1.T),
        "F2aT":  f16(F2p[:128].T),
        "F2bT":  f16(F2p[128:].T),
        "Sel":   f16(np.tile(np.eye(32, dtype=_F32), (4, 1))),  # (128,32)
        "F0T32": f32(F0.T),                            # (32,128) fp32 lhsT
        "F0T16": f16(F0.T),                            # (32,128) for D-injection
        "PinvT": f32(pinv.T.astype(np.float32)),       # (128,32)
        "PinvT16": f16(pinv.T.astype(np.float32)),     # (128,32) per-slab use
        "RPinvT": f32((R.astype(np.float64) @ pinv).T.astype(np.float32)),  # (128,10)
        "Id32":  f32(np.eye(32, dtype=_F32)),          # (32,32)
        "W0T":   f32(W0.T), "W1T": f32(W1.T), "W2T": f32(W2.T),
        "b0c":   f32(b0[:, None]), "b1c": f32(b1[:, None]),
        "b2c":   f32(b2[:, None]),
        "f0c":   f32(f0[:, None]), "f1c": f32(f1[:, None]),
        "f2nA":  f32(-2.0 * f2p[:128, None]),
        "f2nB":  f32(-2.0 * f2p[128:, None]),
        "rbc":   f32(rb[:, None]),
    }


# --------------------------------------------------------------------------
# Bass kernel build
# --------------------------------------------------------------------------

_NC_CACHE = {}


def _build_nc():
    key = (P_SLABS, CHUNK)
    if key in _NC_CACHE:
        return _NC_CACHE[key]

    import concourse.bacc as bacc
    import concourse.mybir as mybir
    import concourse.tile as tile
    from contextlib import ExitStack

    f32 = mybir.dt.float32
    mmdt = mybir.dt.from_np(np.dtype(MM_DT))
    AF = mybir.ActivationFunctionType
    OP = mybir.AluOpType

    import concourse.hw_specs as hw_specs
    _full_tabs = hw_specs.get_activation_tables("gen3")
    _ours = {AF.Exp, AF.Ln, AF.Identity, AF.Copy}
    _pinned = {
        name: (set(funcs) if name == "natural_log_exp_and_others"
               else set(funcs) - _ours)
        for name, funcs in _full_tabs.items()
    }
    bacc.get_activation_tables = lambda arch: _pinned

    P, C = P_SLABS, CHUNK
    NCH = NS // C
    assert NS % C == 0

    nc = bacc.Bacc("TRN2", target_bir_lowering=False, debug=False)

    # ---- DRAM I/O ----
    wshapes = {
        "ATt": (128, 128), "F1T": (128, 128), "F2aT": (128, 128),
        "F2bT": (128, 128), "Sel": (128, 32),
        "F0T32": (32, 128), "F0T16": (32, 128), "Id32": (32, 32), "PinvT": (128, 32),
        "PinvT16": (128, 32), "RPinvT": (128, 10),
        "W0T": (8, 128), "W1T": (128, 128), "W2T": (128, 32),
        "b0c": (128, 1), "b1c": (128, 1), "b2c": (32, 1),
        "f0c": (128, 1), "f1c": (128, 1),
        "f2nA": (128, 1), "f2nB": (128, 1), "rbc": (10, 1),
    }
    mm_names = {"ATt", "F1T", "F2aT", "F2bT", "Sel", "PinvT16", "F0T16"}
    dram = {}
    for name, shp in wshapes.items():
        dt_ = mmdt if name in mm_names else f32
        dram[name] = nc.dram_tensor(name, list(shp), dt_, kind="ExternalInput")
    dram["x0"] = nc.dram_tensor("x0", [8, BS], f32, kind="ExternalInput")
    dram["dxbfA"] = nc.dram_tensor("dxbfA", [NCH, 128, C * 2 * W], f32,
                                   kind="ExternalInput")
    dram["dxbfB"] = nc.dram_tensor("dxbfB", [NCH, 128, C * 2 * W], f32,
                                   kind="ExternalInput")
    dram["dxbc"] = nc.dram_tensor("dxbc", [128, P * 2 * BS], f32,
                                  kind="ExternalInput")
    out_dram = nc.dram_tensor("logits", [NCLS, BS], f32, kind="ExternalOutput")

    with tile.TileContext(nc) as tc, ExitStack() as ctx:
        const = ctx.enter_context(tc.tile_pool(name="const", bufs=1))
        dxbp = ctx.enter_context(tc.tile_pool(name="dxbp", bufs=2))
        work = ctx.enter_context(tc.tile_pool(name="work", bufs=2))
        state = ctx.enter_context(tc.tile_pool(name="state", bufs=1))
        psum = ctx.enter_context(tc.tile_pool(name="psum", bufs=1, space="PSUM"))

        ct = {}
        for name, shp in wshapes.items():
            dt_ = mmdt if name in mm_names else f32
            ct[name] = const.tile(list(shp), dt_, tag=name, name=f"c_{name}")
            nc.sync.dma_start(ct[name][:], dram[name][:])
        x0_t = const.tile([8, BS], f32, tag="x0")
        nc.sync.dma_start(x0_t[:], dram["x0"][:])
        dxbc_t = const.tile([128, P * 2 * BS], f32, tag="dxbc")
        nc.sync.dma_start(dxbc_t[:], dram["dxbc"][:])

        # ---- PSUM tiles: 8 banks, chains A and B ----
        ps = {
            k: {t: psum.tile([128, W], f32, tag=t + k, name=t + k)
                for t in ("p1", "p2", "p3a", "p3b")}
            for k in ("A", "B")
        }
        # coarse-phase aliases (slices of bank tiles, used when fine is idle).
        # The init sweep runs on chain-B's banks so it can overlap chain A's
        # first fine steps; the correction runs on chain-A's banks so it can
        # overlap chain B's last fine steps.  Slab states are extracted from
        # the rolling psum via short CLOSED-group pinv matmuls (never a
        # second long-open accumulation group on a busy bank).
        psi = ps["B"]
        psc = ps["A"]
        psl = ps["B"]["p2"][0:NCLS, 0:BS]        # readout

        # ---- persistent state ----
        Upk = state.tile([32, P * BS], f32, tag="Upk")      # U^0 slab starts
        G_old = state.tile([32, P * BS], f32, tag="G_old")  # G(U^0_s)
        F_sb = state.tile([32, P * BS], f32, tag="F_sb")    # F(U^0_s)
        Dpk = state.tile([32, P * BS], mmdt, tag="Dpk")      # F - G_old

        MM = dict(skip_group_check=True)

        def cs(t, s, n=1):
            return t[:, s * BS:(s + n) * BS]

        def eval_body(pp, w, tagsfx, dxb_ap, last, sel_into=None):
            """One Euler eval on chain tiles `pp` at width w.

            Reads pp.p1 (= F0 @ y + f0-less bias handled via ACT bias port),
            produces g and accumulates pp.p1 += ATt @ g (exact Euler step in
            F0-space).  If sel_into is given, also accumulates Sel @ g into
            it (y tracking for the init sweep)."""
            e1 = work.tile([128, w], f32, tag="e1" + tagsfx)
            nc.scalar.activation(e1[:], pp["p1"][:, 0:w], AF.Exp, bias=ct["f0c"][:])
            h1 = work.tile([128, w], mmdt, tag="h1" + tagsfx)
            nc.scalar.activation(h1[:], e1[:], AF.Ln, bias=1.0)
            nc.tensor.matmul(pp["p2"][:, 0:w], ct["F1T"][:], h1[:],
                             start=True, stop=True, **MM)
            e2 = work.tile([128, w], f32, tag="e2" + tagsfx)
            nc.scalar.activation(e2[:], pp["p2"][:, 0:w], AF.Exp, bias=ct["f1c"][:])
            h2 = work.tile([128, w], mmdt, tag="h2" + tagsfx)
            nc.scalar.activation(h2[:], e2[:], AF.Ln, bias=1.0)
            nc.tensor.matmul(pp["p3a"][:, 0:w], ct["F2aT"][:], h2[:],
                             start=True, stop=True, **MM)
            nc.tensor.matmul(pp["p3b"][:, 0:w], ct["F2bT"][:], h2[:],
                             start=True, stop=True, **MM)
            u = work.tile([128, 2 * w], f32, tag="u" + tagsfx)
            nc.scalar.activation(u[:, 0:w], pp["p3a"][:, 0:w], AF.Exp,
                                 scale=-2.0, bias=ct["f2nA"][:])
            nc.scalar.activation(u[:, w:2 * w], pp["p3b"][:, 0:w], AF.Exp,
                                 scale=-2.0, bias=ct["f2nB"][:])
            # tanh on DVE:  r = 1/(1+u),  g = (r - 1/2) * (2 dX dt)
            wv = work.tile([128, 2 * w], f32, tag="w" + tagsfx)
            nc.vector.tensor_scalar(wv[:], u[:], 1.0, 1.0e30, OP.add, OP.min)
            r = work.tile([128, 2 * w], f32, tag="r" + tagsfx)
            nc.vector.reciprocal_approx_fast(r[:], wv[:])
            g = work.tile([128, 2 * w], mmdt, tag="g" + tagsfx)
            if w >= 128:
                nc.vector.scalar_tensor_tensor(g[:, 0:w], r[:, 0:w], -0.5,
                                               dxb_ap[:, 0:w], OP.add, OP.mult)
                nc.tensor.matmul(pp["p1"][:, 0:w], ct["ATt"][:], g[:, 0:w],
                                 start=False, stop=False, **MM)
                nc.vector.scalar_tensor_tensor(g[:, w:2 * w], r[:, w:2 * w],
                                               -0.5, dxb_ap[:, w:2 * w],
                                               OP.add, OP.mult)
                nc.tensor.matmul(pp["p1"][:, 0:w], ct["ATt"][:], g[:, w:2 * w],
                                 start=False, stop=last, **MM)
            else:
                nc.vector.scalar_tensor_tensor(g[:], r[:], -0.5, dxb_ap,
                                               OP.add, OP.mult)
                nc.tensor.matmul(pp["p1"][:, 0:w], ct["ATt"][:], g[:, 0:w],
                                 start=False, stop=False, **MM)
                nc.tensor.matmul(pp["p1"][:, 0:w], ct["ATt"][:], g[:, w:2 * w],
                                 start=False, stop=last, **MM)
            if sel_into is not None:
                nc.tensor.matmul(sel_into, ct["Sel"][:], g[:, 0:w],
                                 start=False, stop=False, **MM)
                nc.tensor.matmul(sel_into, ct["Sel"][:], g[:, w:2 * w],
                                 start=False, stop=last, **MM)

        def softplus32(ps_in, bias_ap, out_tile, tagsfx):
            e = work.tile([128, BS], f32, tag="esp" + tagsfx)
            nc.scalar.activation(e[:], ps_in, AF.Exp, bias=bias_ap)
            nc.scalar.activation(out_tile[:], e[:], AF.Ln, bias=1.0)

        # ================= initial MLP -> y0 = Upk[:, 0] =================
        nc.tensor.matmul(psc["p1"][:, 0:BS], ct["W0T"][:], x0_t[:],
                         start=True, stop=True, **MM)
        hA = work.tile([128, BS], f32, tag="hA")
        softplus32(psc["p1"][:, 0:BS], ct["b0c"][:], hA, "A0")
        nc.tensor.matmul(psc["p2"][:, 0:BS], ct["W1T"][:], hA[:],
                         start=True, stop=True, **MM)
        hB = work.tile([128, BS], f32, tag="hB")
        softplus32(psc["p2"][:, 0:BS], ct["b1c"][:], hB, "B0")
        nc.tensor.matmul(psc["p3a"][0:32, 0:BS], ct["W2T"][:], hB[:],
                         start=True, stop=True, **MM)
        nc.scalar.activation(cs(Upk, 0), psc["p3a"][0:32, 0:BS], AF.Identity,
                             bias=ct["b2c"][:])

        # ======= staggered schedule =======
        # init slabs 0..24 (serial) -> chain A fine steps 0..9 overlapping
        # init slabs 25..49 -> paired fine steps (A: 10..39, B: 0..29) ->
        # A endpoints + correction slabs 0..24 overlapping B fine 30..39 ->
        # B endpoints -> correction slabs 25..49 -> readout.
        LAG = 10
        spread = [3, 2, 3, 2, 3, 2, 3, 2, 3, 2]
        dxb_cur = {}

        def get_dxb(key, i):
            """chunked dxb stream per chain; DMA on chunk boundary."""
            if i % C == 0:
                t = dxbp.tile([128, C * 2 * W], f32, tag="dxbf" + key,
                              name="dxbf" + key)
                nc.sync.dma_start(t[:], dram["dxbf" + key][i // C])
                dxb_cur[key] = t
            c = i % C
            return dxb_cur[key][:, c * 2 * W:(c + 1) * 2 * W]

        def init_slab(s):
            """one init coarse slab on chain-B banks; rolling p1B; slab
            state extracted via a closed-group pinv matmul into p3aB."""
            eval_body(psi, BS, "c", dxbc_t[:, s * 2 * BS:(s + 1) * 2 * BS],
                      s == P - 1)
            y128 = work.tile([128, BS], mmdt, tag="y128", name="y128")
            nc.scalar.activation(y128[:], psi["p1"][:, 0:BS], AF.Identity)
            nc.tensor.matmul(psi["p3a"][0:32, 0:BS], ct["PinvT16"][:],
                             y128[:], start=True, stop=True, **MM)
            nc.scalar.activation(cs(G_old, s), psi["p3a"][0:32, 0:BS],
                                 AF.Identity)
            if s < P - 1:
                nc.vector.tensor_copy(cs(Upk, s + 1), cs(G_old, s))

        def corr_slab(s):
            eval_body(psc, BS, "cr", dxbc_t[:, s * 2 * BS:(s + 1) * 2 * BS],
                      False)
            # psum1 += F0 @ D_s   (parareal correction, input-only)
            nc.tensor.matmul(psc["p1"][:, 0:BS], ct["F0T16"][:], cs(Dpk, s),
                             start=False, stop=(s == P - 1), **MM)

        def extract_F(key, lo):
            p1sb = work.tile([128, W], f32, tag="p1sb" + key,
                             name="p1sb" + key)
            nc.scalar.activation(p1sb[:], ps[key]["p1"][:, 0:W], AF.Identity)
            nc.tensor.matmul(ps[key]["p3a"][0:32, 0:W], ct["PinvT"][:],
                             p1sb[:], start=True, stop=True, **MM)
            nc.scalar.activation(F_sb[:, lo:lo + W],
                                 ps[key]["p3a"][0:32, 0:W], AF.Identity)

        # ---- init sweep start (rolling on chain-B banks) ----
        nc.tensor.matmul(psi["p1"][:, 0:BS], ct["F0T32"][:], cs(Upk, 0),
                         start=True, stop=False, **MM)
        for s in range(P // 2):
            init_slab(s)

        # ---- chain A starts; init slabs 25..49 overlap A steps 0..9 ----
        nc.tensor.matmul(ps["A"]["p1"][:, 0:W], ct["F0T32"][:], Upk[:, 0:W],
                         start=True, stop=False, **MM)
        nxt_init = P // 2
        for k in range(LAG):
            eval_body(ps["A"], W, "fA", get_dxb("A", k), False)
            for _ in range(spread[k]):
                init_slab(nxt_init)
                nxt_init += 1

        # ---- chain B starts; paired steps ----
        nc.tensor.matmul(ps["B"]["p1"][:, 0:W], ct["F0T32"][:],
                         Upk[:, W:2 * W], start=True, stop=False, **MM)
        for i in range(LAG, NS):
            eval_body(ps["A"], W, "fA", get_dxb("A", i), i == NS - 1)
            eval_body(ps["B"], W, "fB", get_dxb("B", i - LAG), False)

        # ---- A endpoints; correction slabs 0..24 overlap B steps 30..39 ----
        extract_F("A", 0)
        nc.vector.tensor_tensor(Dpk[:, 0:W], F_sb[:, 0:W], G_old[:, 0:W],
                                OP.subtract)
        nc.tensor.matmul(psc["p1"][:, 0:BS], ct["F0T32"][:], cs(Upk, 0),
                         start=True, stop=False, **MM)
        nxt_corr = 0
        for k in range(LAG):
            eval_body(ps["B"], W, "fB", get_dxb("B", NS - LAG + k),
                      k == LAG - 1)
            for _ in range(spread[k]):
                corr_slab(nxt_corr)
                nxt_corr += 1

        # ---- B endpoints; correction slabs 25..49 ----
        extract_F("B", W)
        nc.vector.tensor_tensor(Dpk[:, W:2 * W], F_sb[:, W:2 * W],
                                G_old[:, W:2 * W], OP.subtract)
        for s in range(P // 2, P):
            corr_slab(s)

        # ================= readout: logits = R @ pinv(F0) @ psum1 + rb =======
        p1f = work.tile([128, BS], f32, tag="p1f")
        nc.scalar.activation(p1f[:], psc["p1"][:, 0:BS], AF.Identity)
        nc.tensor.matmul(psl, ct["RPinvT"][:], p1f[:], start=True, stop=True,
                         **MM)
        out_sb = work.tile([NCLS, BS], f32, tag="out_sb")
        nc.scalar.activation(out_sb[:], psl, AF.Identity, bias=ct["rbc"][:])
        nc.sync.dma_start(out_dram[:], out_sb[:])

    nc.compile()
    _NC_CACHE[key] = nc
    return nc


# --------------------------------------------------------------------------
# Public entry point
# --------------------------------------------------------------------------

def _prepare_inputs(ts, coeff_d, coeff_c, coeff_b, coeff_a,
                    W0, b0, W1, b1, W2, b2, F0, f0, F1, f1, F2, f2, R, rb):
    P, C = P_SLABS, CHUNK
    NCH = NS // C
    ts = np.asarray(ts, dtype=_F32)
    coeff_a = np.asarray(coeff_a, _F32)
    cd, cc, cb = (np.asarray(a, _F32) for a in (coeff_d, coeff_c, coeff_b))

    # fine-step times (exactly the reference's grid)
    t_fine = (ts[0] + _F32(DT0) * np.arange(NUM_STEPS, dtype=_F32)).astype(_F32)
    dts = np.minimum(_F32(DT0), ts[-1] - t_fine).astype(_F32)
    dx_fine = _dx_eval(ts, cd, cc, cb, t_fine)            # (2000, B, D)
    dx_fine = dx_fine * (2.0 * dts[:, None, None])

    # coarse increments: EXACT integrals of the piecewise-quadratic dX over
    # each slab (the control increment the reference's Euler steps are a
    # Riemann sum of).
    slab_len = T1 / P
    knot_d = np.diff(ts).astype(np.float64)
    cbd, ccd, cdd = (a.astype(np.float64) for a in (cb, cc, cd))
    Tk = (cbd * knot_d[None, :, None] + ccd * knot_d[None, :, None] ** 2
          + cdd * knot_d[None, :, None] ** 3)             # (B, 127, D)
    pref = np.concatenate([np.zeros((B, 1, D)), np.cumsum(Tk, axis=1)], axis=1)

    def antider(t):
        idx = int(np.clip(np.searchsorted(ts, t, side="right") - 1, 0, NP_KNOTS - 2))
        u = float(t) - float(ts[idx])
        return pref[:, idx] + cbd[:, idx] * u + ccd[:, idx] * u * u + cdd[:, idx] * u ** 3

    bounds = [s * slab_len for s in range(P)] + [T1]
    A = np.stack([antider(t) for t in bounds])            # (P+1, B, D)
    dx_coarse = (A[1:] - A[:-1]).astype(_F32) * _F32(2.0)  # (P, B, D)

    Wt = _host_weights(*[np.asarray(a, _F32) for a in
                         (W0, b0, W1, b1, W2, b2, F0, f0, F1, f1, F2, f2, R, rb)])
    in_maps = []
    for core in range(NCORES):
        bsl = slice(core * BS, (core + 1) * BS)
        mmap = dict(Wt)
        mmap["x0"] = np.ascontiguousarray(coeff_a[bsl, 0, :].T)
        # fine dxb per chain: [i, (slab, j), d] packed layout
        dxc = dx_fine[:, bsl, :]                          # (2000, 16, 8)
        dxp = dxc.reshape(P, NS, BS, D).transpose(1, 0, 2, 3)  # (NS, P, BS, D)
        for key, g0 in (("A", 0), ("B", NG)):
            grp = dxp[:, g0:g0 + NG].reshape(NS, W, D)
            arr = _pack_layout(grp, W)                    # (NS, 128, 2W)
            arr = arr.reshape(NCH, C, 128, 2 * W).transpose(0, 2, 1, 3)
            mmap["dxbf" + key] = np.ascontiguousarray(
                arr.reshape(NCH, 128, C * 2 * W))
        arrc = _pack_layout(dx_coarse[:, bsl, :], BS)     # (P, 128, 32)
        mmap["dxbc"] = np.ascontiguousarray(
            arrc.transpose(1, 0, 2).reshape(128, P * 2 * BS))
        in_maps.append(mmap)
    return in_maps


def kernel(ts, coeff_d, coeff_c, coeff_b, coeff_a,
           W0, b0, W1, b1, W2, b2, F0, f0, F1, f1, F2, f2, R, rb):
    from concourse.bass_utils import run_bass_kernel_spmd

    nc = _build_nc()
    in_maps = _prepare_inputs(ts, coeff_d, coeff_c, coeff_b, coeff_a,
                              W0, b0, W1, b1, W2, b2,
                              F0, f0, F1, f1, F2, f2, R, rb)
    res = run_bass_kernel_spmd(nc, in_maps, list(range(NCORES)))
    logits = np.concatenate(
        [res.results[i]["logits"].T for i in range(NCORES)], axis=0)
    return np.ascontiguousarray(logits.astype(np.float32))


# revision 21
# speedup vs baseline: 8.4228x; 1.0120x over previous
"""Trainium2 Bass kernel for a Neural CDE forward pass — parareal edition.

Model (see reference): 2000 fixed Euler steps (h=0.01) of
    y_{t+1} = y_t + dt * einsum('bhd,bd->bh', tanh-MLP(y_t).reshape(B,H,D), dX_t)
with a 3-layer softplus MLP (32 -> 128 -> 128 -> 256/tanh), batch B=128,
followed by a linear readout.

The reference trajectory is sensitive (its own discretization error vs the
true flow is ~1e-2), so the only way to match it within 2e-2 is to reproduce
its exact discrete Euler map.  The serial step chain is latency-bound
(~200-700ns per instruction, nearly width-independent up to ~400 cols).

Strategy:
  * Pure data parallel over 8 NeuronCores (16 batch elements per core).
  * PARAREAL over time inside each core: 50 slabs of 40 fine steps.  The
    fine propagator (exact Euler, h=0.01) advances slabs 0-24 and 25-49 as
    TWO independent packed chains (width 400 each) whose instructions
    interleave on the engines — 40 serial chain-steps cover all 2000.
  * Coarse propagator G: one Euler step per slab using the EXACT integral
    of dX over the slab (host-computed piecewise-quadratic antiderivative).
    G tracks F so well that ONE parareal iteration
        U_{s+1} <- G(U_s) + F(U_s_prev) - G(U_s_prev)
    reaches ~3e-5 rel err in float64 simulation.
  * Both coarse sweeps roll the state in PSUM (psum1 = F0 @ y, updated via
    psum1 += tile(F0.T) @ g); y is never materialized on the serial chain.
    The init sweep extracts slab states off-chain via a Sel accumulator;
    the correction sweep injects the packed parareal defect D = F - G_old
    with one K=32 matmul per slab and needs no extraction at all — the
    readout is (R @ pinv(F0)) @ psum1 + rb.
  * Feature-major layout: features on partitions, (slab, batch) on the free
    dim; every layer is one PE matmul with a constant fp16 lhsT.
  * softplus(x) = Ln(Exp(x)+1): two ACT ops (natural_log_exp table, pinned
    so the chooser never inserts ACT_TABLE_LOADs).
  * tanh section on DVE: u = exp(-2z-2f2) (ACT, f2 bias on the bias port),
    w = min(1+u, 1e30), r ~ 1/w, g = (r - 1/2) * (2 dX dt) — keeps ACT
    below saturation when two fine chains interleave.
"""

import numpy as np

B = 128
NP_KNOTS = 128
D = 8
H = 32
WID = 128
NCLS = 10
T0, T1 = 0.0, 20.0
DT0 = 0.01
NUM_STEPS = 2000
NCORES = 8
BS = B // NCORES          # 16 batch per core

# parareal configuration
P_SLABS = 50              # time slabs per core (two packed chains of 25)
NG = P_SLABS // 2         # slabs per chain
NS = NUM_STEPS // P_SLABS  # fine steps per slab (40)
CHUNK = 5                 # fine steps per DMA chunk
W = NG * BS               # packed width per chain (400)

_F32 = np.float32
MM_DT = np.float16


# --------------------------------------------------------------------------
# Host-side precompute
# --------------------------------------------------------------------------

def _dx_eval(ts, coeff_d, coeff_c, coeff_b, t_grid):
    """Spline derivative dX/dt at the given times.  Returns (T, B, D)."""
    idx = np.clip(np.searchsorted(ts, t_grid, side="right") - 1, 0, NP_KNOTS - 2)
    fr = (t_grid - ts[idx]).astype(_F32)[None, :, None]
    dX = (coeff_b[:, idx] + _F32(2.0) * coeff_c[:, idx] * fr
          + _F32(3.0) * coeff_d[:, idx] * fr * fr)          # (B, T, D)
    return np.transpose(dX, (1, 0, 2)).astype(_F32)         # (T, B, D)


def _pack_layout(dx_scaled, cols):
    """(T, cols, D) -> (T, 128, 2*cols) d-major, h-broadcast layout.

    Partition p in col-half cb holds (d = cb*4 + p//32, h = p%32)."""
    T = dx_scaled.shape[0]
    tmp = dx_scaled.reshape(T, cols, 2, 4)          # [t, j, cb, dblk]
    tmp = np.transpose(tmp, (0, 3, 2, 1))           # [t, dblk, cb, j]
    tmp = np.broadcast_to(tmp[:, :, None, :, :], (T, 4, 32, 2, cols))
    return np.ascontiguousarray(tmp.reshape(T, 128, 2 * cols))


def _host_weights(W0, b0, W1, b1, W2, b2, F0, f0, F1, f1, F2, f2, R, rb):
    f32 = lambda a: np.ascontiguousarray(a, dtype=_F32)
    f16 = lambda a: np.ascontiguousarray(a, dtype=MM_DT)
    p = np.arange(256)
    perm = (p % 32) * 8 + p // 32
    F2p = F2[perm]
    f2p = f2[perm]
    pinv = np.linalg.pinv(F0.astype(np.float64))
    return {
        "ATt":   f16(np.tile(F0.T, (4, 1))),          # (128,128)
        "F1T":   f16(F1.T),
        "F2aT":  f16(F2p[:128].T),
        "F2bT":  f16(F2p[128:].T),
        "Sel":   f16(np.tile(np.eye(32, dtype=_F32), (4, 1))),  # (128,32)
        "F0T32": f32(F0.T),                            # (32,128) fp32 lhsT
        "F0T16": f16(F0.T),                            # (32,128) for D-injection
        "PinvT": f32(pinv.T.astype(np.float32)),       # (128,32)
        "PinvT16": f16(pinv.T.astype(np.float32)),     # (128,32) per-slab use
        "RPinvT": f32((R.astype(np.float64) @ pinv).T.astype(np.float32)),  # (128,10)
        "Id32":  f32(np.eye(32, dtype=_F32)),          # (32,32)
        "W0T":   f32(W0.T), "W1T": f32(W1.T), "W2T": f32(W2.T),
        "b0c":   f32(b0[:, None]), "b1c": f32(b1[:, None]),
        "b2c":   f32(b2[:, None]),
        "f0c":   f32(f0[:, None]), "f1c": f32(f1[:, None]),
        "f2nA":  f32(-2.0 * f2p[:128, None]),
        "f2nB":  f32(-2.0 * f2p[128:, None]),
        "rbc":   f32(rb[:, None]),
    }


# --------------------------------------------------------------------------
# Bass kernel build
# --------------------------------------------------------------------------

_NC_CACHE = {}


def _build_nc():
    key = (P_SLABS, CHUNK)
    if key in _NC_CACHE:
        return _NC_CACHE[key]

    import concourse.bacc as bacc
    import concourse.mybir as mybir
    import concourse.tile as tile
    from contextlib import ExitStack

    f32 = mybir.dt.float32
    mmdt = mybir.dt.from_np(np.dtype(MM_DT))
    AF = mybir.ActivationFunctionType
    OP = mybir.AluOpType

    import concourse.hw_specs as hw_specs
    _full_tabs = hw_specs.get_activation_tables("gen3")
    _ours = {AF.Exp, AF.Ln, AF.Identity, AF.Copy}
    _pinned = {
        name: (set(funcs) if name == "natural_log_exp_and_others"
               else set(funcs) - _ours)
        for name, funcs in _full_tabs.items()
    }
    bacc.get_activation_tables = lambda arch: _pinned

    P, C = P_SLABS, CHUNK
    NCH = NS // C
    assert NS % C == 0

    nc = bacc.Bacc("TRN2", target_bir_lowering=False, debug=False)

    # ---- DRAM I/O ----
    wshapes = {
        "ATt": (128, 128), "F1T": (128, 128), "F2aT": (128, 128),
        "F2bT": (128, 128), "Sel": (128, 32),
        "F0T32": (32, 128), "F0T16": (32, 128), "Id32": (32, 32), "PinvT": (128, 32),
        "PinvT16": (128, 32), "RPinvT": (128, 10),
        "W0T": (8, 128), "W1T": (128, 128), "W2T": (128, 32),
        "b0c": (128, 1), "b1c": (128, 1), "b2c": (32, 1),
        "f0c": (128, 1), "f1c": (128, 1),
        "f2nA": (128, 1), "f2nB": (128, 1), "rbc": (10, 1),
    }
    mm_names = {"ATt", "F1T", "F2aT", "F2bT", "Sel", "PinvT16", "F0T16"}
    dram = {}
    for name, shp in wshapes.items():
        dt_ = mmdt if name in mm_names else f32
        dram[name] = nc.dram_tensor(name, list(shp), dt_, kind="ExternalInput")
    dram["x0"] = nc.dram_tensor("x0", [8, BS], f32, kind="ExternalInput")
    dram["dxbfA"] = nc.dram_tensor("dxbfA", [NCH, 128, C * 2 * W], f32,
                                   kind="ExternalInput")
    dram["dxbfB"] = nc.dram_tensor("dxbfB", [NCH, 128, C * 2 * W], f32,
                                   kind="ExternalInput")
    dram["dxbc"] = nc.dram_tensor("dxbc", [128, P * 2 * BS], f32,
                                  kind="ExternalInput")
    out_dram = nc.dram_tensor("logits", [NCLS, BS], f32, kind="ExternalOutput")

    with tile.TileContext(nc) as tc, ExitStack() as ctx:
        const = ctx.enter_context(tc.tile_pool(name="const", bufs=1))
        dxbp = ctx.enter_context(tc.tile_pool(name="dxbp", bufs=2))
        work = ctx.enter_context(tc.tile_pool(name="work", bufs=2))
        state = ctx.enter_context(tc.tile_pool(name="state", bufs=1))
        psum = ctx.enter_context(tc.tile_pool(name="psum", bufs=1, space="PSUM"))

        ct = {}
        for name, shp in wshapes.items():
            dt_ = mmdt if name in mm_names else f32
            ct[name] = const.tile(list(shp), dt_, tag=name, name=f"c_{name}")
            nc.sync.dma_start(ct[name][:], dram[name][:])
        x0_t = const.tile([8, BS], f32, tag="x0")
        nc.sync.dma_start(x0_t[:], dram["x0"][:])
        dxbc_t = const.tile([128, P * 2 * BS], f32, tag="dxbc")
        nc.sync.dma_start(dxbc_t[:], dram["dxbc"][:])

        # ---- PSUM tiles: 8 banks, chains A and B ----
        ps = {
            k: {t: psum.tile([128, W], f32, tag=t + k, name=t + k)
                for t in ("p1", "p2", "p3a", "p3b")}
            for k in ("A", "B")
        }
        # coarse-phase aliases (slices of bank tiles, used when fine is idle).
        # The init sweep runs on chain-B's banks so it can overlap chain A's
        # first fine steps; the correction runs on chain-A's banks so it can
        # overlap chain B's last fine steps.  Slab states are extracted from
        # the rolling psum via short CLOSED-group pinv matmuls (never a
        # second long-open accumulation group on a busy bank).
        psi = ps["B"]
        psc = ps["A"]
        psl = ps["B"]["p2"][0:NCLS, 0:BS]        # readout

        # ---- persistent state ----
        Upk = state.tile([32, P * BS], f32, tag="Upk")      # U^0 slab starts
        G_old = state.tile([32, P * BS], f32, tag="G_old")  # G(U^0_s)
        F_sb = state.tile([32, P * BS], f32, tag="F_sb")    # F(U^0_s)
        Dpk = state.tile([32, P * BS], mmdt, tag="Dpk")      # F - G_old

        MM = dict(skip_group_check=True)

        def cs(t, s, n=1):
            return t[:, s * BS:(s + n) * BS]

        def eval_body(pp, w, tagsfx, dxb_ap, last, sel_into=None):
            """One Euler eval on chain tiles `pp` at width w.

            Reads pp.p1 (= F0 @ y + f0-less bias handled via ACT bias port),
            produces g and accumulates pp.p1 += ATt @ g (exact Euler step in
            F0-space).  If sel_into is given, also accumulates Sel @ g into
            it (y tracking for the init sweep)."""
            e1 = work.tile([128, w], f32, tag="e1" + tagsfx)
            nc.scalar.activation(e1[:], pp["p1"][:, 0:w], AF.Exp, bias=ct["f0c"][:])
            h1 = work.tile([128, w], mmdt, tag="h1" + tagsfx)
            nc.scalar.activation(h1[:], e1[:], AF.Ln, bias=1.0)
            nc.tensor.matmul(pp["p2"][:, 0:w], ct["F1T"][:], h1[:],
                             start=True, stop=True, **MM)
            e2 = work.tile([128, w], f32, tag="e2" + tagsfx)
            nc.scalar.activation(e2[:], pp["p2"][:, 0:w], AF.Exp, bias=ct["f1c"][:])
            h2 = work.tile([128, w], mmdt, tag="h2" + tagsfx)
            nc.scalar.activation(h2[:], e2[:], AF.Ln, bias=1.0)
            nc.tensor.matmul(pp["p3a"][:, 0:w], ct["F2aT"][:], h2[:],
                             start=True, stop=True, **MM)
            nc.tensor.matmul(pp["p3b"][:, 0:w], ct["F2bT"][:], h2[:],
                             start=True, stop=True, **MM)
            u = work.tile([128, 2 * w], f32, tag="u" + tagsfx)
            nc.scalar.activation(u[:, 0:w], pp["p3a"][:, 0:w], AF.Exp,
                                 scale=-2.0, bias=ct["f2nA"][:])
            nc.scalar.activation(u[:, w:2 * w], pp["p3b"][:, 0:w], AF.Exp,
                                 scale=-2.0, bias=ct["f2nB"][:])
            # tanh on DVE:  r = 1/(1+u),  g = (r - 1/2) * (2 dX dt)
            wv = work.tile([128, 2 * w], f32, tag="w" + tagsfx)
            nc.vector.tensor_scalar(wv[:], u[:], 1.0, 1.0e30, OP.add, OP.min)
            r = work.tile([128, 2 * w], f32, tag="r" + tagsfx)
            nc.vector.reciprocal_approx_fast(r[:], wv[:])
            g = work.tile([128, 2 * w], mmdt, tag="g" + tagsfx)
            if w >= 128:
                nc.vector.scalar_tensor_tensor(g[:, 0:w], r[:, 0:w], -0.5,
                                               dxb_ap[:, 0:w], OP.add, OP.mult)
                nc.tensor.matmul(pp["p1"][:, 0:w], ct["ATt"][:], g[:, 0:w],
                                 start=False, stop=False, **MM)
                nc.vector.scalar_tensor_tensor(g[:, w:2 * w], r[:, w:2 * w],
                                               -0.5, dxb_ap[:, w:2 * w],
                                               OP.add, OP.mult)
                nc.tensor.matmul(pp["p1"][:, 0:w], ct["ATt"][:], g[:, w:2 * w],
                                 start=False, stop=last, **MM)
            else:
                nc.vector.scalar_tensor_tensor(g[:], r[:], -0.5, dxb_ap,
                                               OP.add, OP.mult)
                nc.tensor.matmul(pp["p1"][:, 0:w], ct["ATt"][:], g[:, 0:w],
                                 start=False, stop=False, **MM)
                nc.tensor.matmul(pp["p1"][:, 0:w], ct["ATt"][:], g[:, w:2 * w],
                                 start=False, stop=last, **MM)
            if sel_into is not None:
                nc.tensor.matmul(sel_into, ct["Sel"][:], g[:, 0:w],
                                 start=False, stop=False, **MM)
                nc.tensor.matmul(sel_into, ct["Sel"][:], g[:, w:2 * w],
                                 start=False, stop=last, **MM)

        def softplus32(ps_in, bias_ap, out_tile, tagsfx):
            e = work.tile([128, BS], f32, tag="esp" + tagsfx)
            nc.scalar.activation(e[:], ps_in, AF.Exp, bias=bias_ap)
            nc.scalar.activation(out_tile[:], e[:], AF.Ln, bias=1.0)

        # ================= initial MLP -> y0 = Upk[:, 0] =================
        nc.tensor.matmul(psc["p1"][:, 0:BS], ct["W0T"][:], x0_t[:],
                         start=True, stop=True, **MM)
        hA = work.tile([128, BS], f32, tag="hA")
        softplus32(psc["p1"][:, 0:BS], ct["b0c"][:], hA, "A0")
        nc.tensor.matmul(psc["p2"][:, 0:BS], ct["W1T"][:], hA[:],
                         start=True, stop=True, **MM)
        hB = work.tile([128, BS], f32, tag="hB")
        softplus32(psc["p2"][:, 0:BS], ct["b1c"][:], hB, "B0")
        nc.tensor.matmul(psc["p3a"][0:32, 0:BS], ct["W2T"][:], hB[:],
                         start=True, stop=True, **MM)
        nc.scalar.activation(cs(Upk, 0), psc["p3a"][0:32, 0:BS], AF.Identity,
                             bias=ct["b2c"][:])

        # ======= staggered schedule =======
        # init slabs 0..24 (serial) -> chain A fine steps 0..9 overlapping
        # init slabs 25..49 -> paired fine steps (A: 10..39, B: 0..29) ->
        # A endpoints + correction slabs 0..24 overlapping B fine 30..39 ->
        # B endpoints -> correction slabs 25..49 -> readout.
        LAG = 10
        spread = [3, 2, 3, 2, 3, 2, 3, 2, 3, 2]
        dxb_cur = {}

        def get_dxb(key, i):
            """chunked dxb stream per chain; DMA on chunk boundary."""
            if i % C == 0:
                t = dxbp.tile([128, C * 2 * W], f32, tag="dxbf" + key,
                              name="dxbf" + key)
                nc.sync.dma_start(t[:], dram["dxbf" + key][i // C])
                dxb_cur[key] = t
            c = i % C
            return dxb_cur[key][:, c * 2 * W:(c + 1) * 2 * W]

        def init_slab(s):
            """one init coarse slab on chain-B banks; rolling p1B; slab
            state extracted via a closed-group pinv matmul into p3aB."""
            eval_body(psi, BS, "c", dxbc_t[:, s * 2 * BS:(s + 1) * 2 * BS],
                      s == P - 1)
            y128 = work.tile([128, BS], mmdt, tag="y128", name="y128")
            nc.vector.tensor_copy(y128[:], psi["p1"][:, 0:BS])
            nc.tensor.matmul(psi["p3a"][0:32, 0:BS], ct["PinvT16"][:],
                             y128[:], start=True, stop=True, **MM)
            nc.vector.tensor_copy(cs(G_old, s), psi["p3a"][0:32, 0:BS])
            if s < P - 1:
                nc.vector.tensor_copy(cs(Upk, s + 1), cs(G_old, s))

        def corr_slab(s):
            eval_body(psc, BS, "cr", dxbc_t[:, s * 2 * BS:(s + 1) * 2 * BS],
                      False)
            # psum1 += F0 @ D_s   (parareal correction, input-only)
            nc.tensor.matmul(psc["p1"][:, 0:BS], ct["F0T16"][:], cs(Dpk, s),
                             start=False, stop=(s == P - 1), **MM)

        def extract_F(key, lo):
            p1sb = work.tile([128, W], f32, tag="p1sb" + key,
                             name="p1sb" + key)
            nc.vector.tensor_copy(p1sb[:], ps[key]["p1"][:, 0:W])
            nc.tensor.matmul(ps[key]["p3a"][0:32, 0:W], ct["PinvT"][:],
                             p1sb[:], start=True, stop=True, **MM)
            nc.vector.tensor_copy(F_sb[:, lo:lo + W],
                                  ps[key]["p3a"][0:32, 0:W])

        # ---- init sweep start (rolling on chain-B banks) ----
        nc.tensor.matmul(psi["p1"][:, 0:BS], ct["F0T32"][:], cs(Upk, 0),
                         start=True, stop=False, **MM)
        for s in range(P // 2):
            init_slab(s)

        # ---- chain A starts; init slabs 25..49 overlap A steps 0..9 ----
        nc.tensor.matmul(ps["A"]["p1"][:, 0:W], ct["F0T32"][:], Upk[:, 0:W],
                         start=True, stop=False, **MM)
        nxt_init = P // 2
        for k in range(LAG):
            eval_body(ps["A"], W, "fA", get_dxb("A", k), False)
            for _ in range(spread[k]):
                init_slab(nxt_init)
                nxt_init += 1

        # ---- chain B starts; paired steps ----
        nc.tensor.matmul(ps["B"]["p1"][:, 0:W], ct["F0T32"][:],
                         Upk[:, W:2 * W], start=True, stop=False, **MM)
        for i in range(LAG, NS):
            eval_body(ps["A"], W, "fA", get_dxb("A", i), i == NS - 1)
            eval_body(ps["B"], W, "fB", get_dxb("B", i - LAG), False)

        # ---- A endpoints; correction slabs 0..24 overlap B steps 30..39 ----
        extract_F("A", 0)
        nc.vector.tensor_tensor(Dpk[:, 0:W], F_sb[:, 0:W], G_old[:, 0:W],
                                OP.subtract)
        nc.tensor.matmul(psc["p1"][:, 0:BS], ct["F0T32"][:], cs(Upk, 0),
                         start=True, stop=False, **MM)
        nxt_corr = 0
        for k in range(LAG):
            eval_body(ps["B"], W, "fB", get_dxb("B", NS - LAG + k),
                      k == LAG - 1)
            for _ in range(spread[k]):
                corr_slab(nxt_corr)
                nxt_corr += 1

        # ---- B endpoints; correction slabs 25..49 ----
        extract_F("B", W)
        nc.vector.tensor_tensor(Dpk[:, W:2 * W], F_sb[:, W:2 * W],
                                G_old[:, W:2 * W], OP.subtract)
        for s in range(P // 2, P):
            corr_slab(s)

        # ================= readout: logits = R @ pinv(F0) @ psum1 + rb =======
        p1f = work.tile([128, BS], f32, tag="p1f")
        nc.vector.tensor_copy(p1f[:], psc["p1"][:, 0:BS])
        nc.tensor.matmul(psl, ct["RPinvT"][:], p1f[:], start=True, stop=True,
                         **MM)
        out_sb = work.tile([NCLS, BS], f32, tag="out_sb")
        nc.scalar.activation(out_sb[:], psl, AF.Identity, bias=ct["rbc"][:])
        nc.sync.dma_start(out_dram[:], out_sb[:])

    nc.compile()
    _NC_CACHE[key] = nc
    return nc


# --------------------------------------------------------------------------
# Public entry point
# --------------------------------------------------------------------------

def _prepare_inputs(ts, coeff_d, coeff_c, coeff_b, coeff_a,
                    W0, b0, W1, b1, W2, b2, F0, f0, F1, f1, F2, f2, R, rb):
    P, C = P_SLABS, CHUNK
    NCH = NS // C
    ts = np.asarray(ts, dtype=_F32)
    coeff_a = np.asarray(coeff_a, _F32)
    cd, cc, cb = (np.asarray(a, _F32) for a in (coeff_d, coeff_c, coeff_b))

    # fine-step times (exactly the reference's grid)
    t_fine = (ts[0] + _F32(DT0) * np.arange(NUM_STEPS, dtype=_F32)).astype(_F32)
    dts = np.minimum(_F32(DT0), ts[-1] - t_fine).astype(_F32)
    dx_fine = _dx_eval(ts, cd, cc, cb, t_fine)            # (2000, B, D)
    dx_fine = dx_fine * (2.0 * dts[:, None, None])

    # coarse increments: EXACT integrals of the piecewise-quadratic dX over
    # each slab (the control increment the reference's Euler steps are a
    # Riemann sum of).
    slab_len = T1 / P
    knot_d = np.diff(ts).astype(np.float64)
    cbd, ccd, cdd = (a.astype(np.float64) for a in (cb, cc, cd))
    Tk = (cbd * knot_d[None, :, None] + ccd * knot_d[None, :, None] ** 2
          + cdd * knot_d[None, :, None] ** 3)             # (B, 127, D)
    pref = np.concatenate([np.zeros((B, 1, D)), np.cumsum(Tk, axis=1)], axis=1)

    def antider(t):
        idx = int(np.clip(np.searchsorted(ts, t, side="right") - 1, 0, NP_KNOTS - 2))
        u = float(t) - float(ts[idx])
        return pref[:, idx] + cbd[:, idx] * u + ccd[:, idx] * u * u + cdd[:, idx] * u ** 3

    bounds = [s * slab_len for s in range(P)] + [T1]
    A = np.stack([antider(t) for t in bounds])            # (P+1, B, D)
    dx_coarse = (A[1:] - A[:-1]).astype(_F32) * _F32(2.0)  # (P, B, D)

    Wt = _host_weights(*[np.asarray(a, _F32) for a in
                         (W0, b0, W1, b1, W2, b2, F0, f0, F1, f1, F2, f2, R, rb)])
    in_maps = []
    for core in range(NCORES):
        bsl = slice(core * BS, (core + 1) * BS)
        mmap = dict(Wt)
        mmap["x0"] = np.ascontiguousarray(coeff_a[bsl, 0, :].T)
        # fine dxb per chain: [i, (slab, j), d] packed layout
        dxc = dx_fine[:, bsl, :]                          # (2000, 16, 8)
        dxp = dxc.reshape(P, NS, BS, D).transpose(1, 0, 2, 3)  # (NS, P, BS, D)
        for key, g0 in (("A", 0), ("B", NG)):
            grp = dxp[:, g0:g0 + NG].reshape(NS, W, D)
            arr = _pack_layout(grp, W)                    # (NS, 128, 2W)
            arr = arr.reshape(NCH, C, 128, 2 * W).transpose(0, 2, 1, 3)
            mmap["dxbf" + key] = np.ascontiguousarray(
                arr.reshape(NCH, 128, C * 2 * W))
        arrc = _pack_layout(dx_coarse[:, bsl, :], BS)     # (P, 128, 32)
        mmap["dxbc"] = np.ascontiguousarray(
            arrc.transpose(1, 0, 2).reshape(128, P * 2 * BS))
        in_maps.append(mmap)
    return in_maps


def kernel(ts, coeff_d, coeff_c, coeff_b, coeff_a,
           W0, b0, W1, b1, W2, b2, F0, f0, F1, f1, F2, f2, R, rb):
    from concourse.bass_utils import run_bass_kernel_spmd

    nc = _build_nc()
    in_maps = _prepare_inputs(ts, coeff_d, coeff_c, coeff_b, coeff_a,
                              W0, b0, W1, b1, W2, b2,
                              F0, f0, F1, f1, F2, f2, R, rb)
    res = run_bass_kernel_spmd(nc, in_maps, list(range(NCORES)))
    logits = np.concatenate(
        [res.results[i]["logits"].T for i in range(NCORES)], axis=0)
    return np.ascontiguousarray(logits.astype(np.float32))
